# revision 50
# baseline (speedup 1.0000x reference)
"""AttentionalAggregation GNN kernel for 8 TRN2 NeuronCores.

Strategy: edges sorted by destination bucket on host; core m owns nodes
[m*NPC, (m+1)*NPC) and computes its output slice fully locally (no
collectives). Per 128-edge tile:
  - dma_gather x[src] rows (512B each) from lo/hi half tables (int16 idx)
  - score_e = sum_d V[e,d]*w[d]  (scalar_tensor_tensor accum)
  - e = exp(score + b)           (ACT)
  - P'[e,s] = (iota_s == slot_e) * e_e   (built on DVE, Pool or ACT,
    split to balance engine load; ACT path reads a host-built 0/1
    one-hot in fp8 and scales by e in one Copy op)
  - psum[bucket][:,0:128] += P'.T @ V_t   (plain f32 matmul: hardware
    f32r is tf32-grade and fails the accuracy budget)
  - psum[bucket][:,128]   += P'.T @ ones  (denominator)
Flush per group: reciprocal of denominator, scale, write agg as bf16 in
slot-major layout; host unpermutes, casts to f32 and concats with x.
"""

import math
import numpy as np

import concourse.bass as bass
import concourse.mybir as mybir
import concourse.tile as tile
from concourse import bacc

F32 = mybir.dt.float32
F32R = mybir.dt.float32r
BF16 = mybir.dt.bfloat16
FP8 = mybir.dt.float8e4
I16 = mybir.dt.int16
AF = mybir.ActivationFunctionType
OP = mybir.AluOpType

# engine assignment for building P' (one-hot * e) per tile
ENG_DVE = 0
ENG_POOL = 1
ENG_ACT = 2
# shares (DVE, POOL, ACT) of P'-build work; ACT tiles need the p01 stream.
# Pool-engine P' builds stall the in-order Pool queue behind cross-engine
# waits (gathers share it), so the Pool share stays 0.
PP_SHARES = (0.28, 0.0, 0.72)
# fraction of score STTs routed to the Pool engine. Keep 0: any non-SWDGE
# work on the in-order Pool engine delays later groups' descriptor
# generation and starves the DMA engines.
SC_POOL = 0.0


class Config:
    def __init__(self, N=50000, E=640000, D=128, NC=8, GROUP=2):
        assert D == 128
        self.N, self.E, self.D, self.NC = N, E, D, NC
        self.NPC = N // NC          # nodes per core
        assert self.NPC * NC == N
        # overlapping lo/hi gather tables (int16 index limit 32768 rows);
        # sources in the overlap may be assigned to either run, letting the
        # host pad the lo run to a tile boundary with real edges
        self.LO_MAX = min(32768, N)
        self.HI_BASE = max(0, N - 32768)
        self.NBUK = math.ceil(self.NPC / 128)   # buckets per core
        self.TAIL = self.NPC - (self.NBUK - 1) * 128  # rows in last bucket
        self.GROUP = GROUP
        self.NG = math.ceil(self.NBUK / GROUP)


def assign_engines(TOT):
    """Per-tile engine for the P' build, interleaved to the target shares."""
    eng = np.zeros(TOT, np.int8)
    acc = [0.0, 0.0, 0.0]
    for t in range(TOT):
        # pick the engine furthest below its target share
        deficits = [PP_SHARES[k] * (t + 1) - acc[k] for k in range(3)]
        k = int(np.argmax(deficits))
        eng[t] = k
        acc[k] += 1.0
    return eng


def build_schedule(cfg, src, dst):
    """Host-side: sort/pad edges into a static per-tile schedule uniform
    across cores. Returns (sched, per_core) where sched is the static
    structure (identical across cores) and per_core has the data arrays."""
    N, NC, NPC, NBUK, GROUP = (
        cfg.N, cfg.NC, cfg.NPC, cfg.NBUK, cfg.GROUP)
    LO_MAX, HI_BASE = cfg.LO_MAX, cfg.HI_BASE

    src = np.asarray(src, np.int64)
    dst = np.asarray(dst, np.int64)
    c = dst // NPC
    r = dst % NPC
    lb = r // 128
    slot = r % 128

    order = np.lexsort((src, lb, c))
    src_s, lb_s, slot_s, c_s = (
        src[order], lb[order], slot[order], c[order])

    key = c_s * NBUK + lb_s
    cnt = np.bincount(key, minlength=NC * NBUK).reshape(NC, NBUK)
    starts = np.zeros(NC * NBUK + 1, np.int64)
    np.cumsum(cnt.reshape(-1), out=starts[1:])

    # within each (core, bucket) slice (sorted by src), edges below
    # HI_BASE must use the lo table, edges >= LO_MAX must use hi, and the
    # overlap is flexible: cut the slice to fill lo tiles exactly
    n_lo_min = np.zeros((NC, NBUK), np.int64)
    n_lo_cap = np.zeros((NC, NBUK), np.int64)
    for cc in range(NC):
        for b in range(NBUK):
            k = cc * NBUK + b
            sl = src_s[starts[k]:starts[k + 1]]
            n_lo_min[cc, b] = np.searchsorted(sl, HI_BASE)
            n_lo_cap[cc, b] = np.searchsorted(sl, LO_MAX)
    # per-core needs, then sort each core's buckets by total tiles
    # descending so position-wise maxima across cores are tight
    t_lo_c = np.ceil(n_lo_min / 128.0).astype(np.int64)       # [NC, NBUK]
    lo_cap_pos = n_lo_cap
    t_hi_c = np.ceil(np.maximum(cnt - np.minimum(128 * t_lo_c, lo_cap_pos),
                                0) / 128.0).astype(np.int64)
    tot_c = t_lo_c + t_hi_c
    perm = np.argsort(-tot_c, axis=1, kind="stable")          # [NC, NBUK]
    ar = np.arange(NC)[:, None]
    T_lo = t_lo_c[ar, perm].max(axis=0)                       # [NBUK] by pos
    lo_count_pos = np.minimum(128 * T_lo[None, :], n_lo_cap[ar, perm])
    T_hi = np.ceil((cnt[ar, perm] - lo_count_pos) / 128.0
                   ).astype(np.int64).max(axis=0)
    Th = np.stack([T_lo, T_hi], axis=1)  # [NBUK, 2] by position
    # scatter position-based lo counts back to per-(core,bucket)
    lo_count = np.zeros_like(cnt)
    np.put_along_axis(lo_count, perm, lo_count_pos, axis=1)
    # move the smallest bucket to position 0: paired with a single-bucket
    # first group below, it halves the ramp-critical first gather batch
    pi = np.r_[NBUK - 1, NBUK - 2, np.arange(NBUK - 2)]
    perm = perm[:, pi]
    Th = Th[pi]
    Tb = Th.sum(axis=1)

    # static tile stream: per group g: [lo tiles of buckets][hi tiles]
    # each entry: (bucket_local_index_in_group j, bucket b, first, last)
    sizes = [1, 1]
    left = NBUK - 2
    while left > 0:
        take = min(GROUP, left)
        sizes.append(take)
        left -= take
    bounds = np.cumsum([0] + sizes)
    groups = []
    for g in range(len(sizes)):
        b0, b1 = int(bounds[g]), int(bounds[g + 1])
        tiles = []
        for h in (0, 1):
            for b in range(b0, b1):
                nt = int(Th[b, h])
                for t in range(nt):
                    pos = t if h == 0 else int(Th[b, 0]) + t
                    first = pos == 0
                    last = pos == int(Tb[b]) - 1
                    tiles.append(dict(j=b - b0, b=b, first=first, last=last))
        lo_tiles = int(Th[b0:b1, 0].sum())
        hi_tiles = int(Th[b0:b1, 1].sum())
        groups.append(dict(b0=b0, b1=b1, lo=lo_tiles, hi=hi_tiles,
                           tiles=tiles))
    TOT = sum(g["lo"] + g["hi"] for g in groups)

    # engine assignment per global tile; ACT tiles get p01 stream slots
    eng = assign_engines(TOT)
    act_idx = np.nonzero(eng == ENG_ACT)[0]
    act_pos = np.full(TOT, -1, np.int64)    # tile -> index among ACT tiles
    for i, t in enumerate(act_idx):
        act_pos[t] = i
    NACT = len(act_idx)

    # pass A: per-tile slot-span unions across cores (edges re-sorted by
    # slot within each (bucket, half) run so a tile covers a narrow
    # contiguous window; matmul partition offsets must be 32-aligned with
    # legal widths {32 @0/32/64/96, 64 @0/64, 128 @0})
    span_lo = np.full(TOT, 255, np.int64)
    span_hi = np.full(TOT, -1, np.int64)
    for core in range(NC):
        t_glob = 0
        for g in groups:
            for h in (0, 1):
                for pos in range(g["b0"], g["b1"]):
                    b = int(perm[core, pos])
                    k = core * NBUK + b
                    s0, s1 = starts[k], starts[k + 1]
                    cut = s0 + lo_count[core, b]
                    e_slot = (slot_s[s0:cut] if h == 0
                              else slot_s[cut:s1])
                    nt = int(Th[pos, h])
                    if nt > 0 and len(e_slot) > 0:
                        sl = np.sort(e_slot)
                        for kk in range(nt):
                            seg = sl[kk * 128:(kk + 1) * 128]
                            if len(seg):
                                t = t_glob + kk
                                span_lo[t] = min(span_lo[t], int(seg[0]))
                                span_hi[t] = max(span_hi[t], int(seg[-1]))
                    t_glob += nt

    def legal_window(a, b):
        # base partitions allowed by the AP layer: {0, 32, 64} only
        if b < 0:
            return 0, 32
        p = 32 * (a // 32)
        if p <= 64 and b - p < 32:
            return p, 32
        p = 64 * (a // 64)
        if b - p < 64:
            return p, 64
        return 0, 128

    win = [legal_window(int(span_lo[t]), int(span_hi[t]))
           for t in range(TOT)]
    # tiles that open a bucket's psum bank (start=True) must cover the
    # full 128 partitions so the bank-clear covers every slot
    t_glob = 0
    for g in groups:
        for i, ti in enumerate(g["tiles"]):
            if ti["first"]:
                win[t_glob + i] = (0, 128)
        t_glob += len(g["tiles"])
    wp0 = np.array([w[0] for w in win], np.int64)
    ww = np.array([w[1] for w in win], np.int64)
    # variable-width p01 offsets for ACT tiles
    act_woff = np.zeros(NACT + 1, np.int64)
    np.cumsum(ww[act_idx], out=act_woff[1:])
    P01W = int(act_woff[-1])

    # per-core data arrays
    per_core = []
    for core in range(NC):
        idx_stream = np.zeros(TOT * 128, np.int16)
        slot_stream = np.full((128, TOT), 255.0, np.float32)
        t_glob = 0
        for g in groups:
            for h in (0, 1):
                for pos in range(g["b0"], g["b1"]):
                    b = int(perm[core, pos])
                    k = core * NBUK + b
                    s0, s1 = starts[k], starts[k + 1]
                    cut = s0 + lo_count[core, b]
                    if h == 0:
                        e_src = src_s[s0:cut]
                        e_slot = slot_s[s0:cut]
                    else:
                        e_src = src_s[cut:s1] - HI_BASE
                        e_slot = slot_s[cut:s1]
                    o2 = np.argsort(e_slot, kind="stable")
                    e_src = e_src[o2]
                    e_slot = e_slot[o2]
                    n = len(e_src)
                    nt = int(Th[pos, h])
                    base = t_glob * 128
                    if n > 0:
                        idx_stream[base:base + n] = e_src.astype(np.int16)
                        fl = np.full(nt * 128, 255.0, np.float32)
                        fl[:n] = e_slot.astype(np.float32)
                        slot_stream[:, t_glob:t_glob + nt] = (
                            fl.reshape(nt, 128).T)
                    t_glob += nt
        assert t_glob == TOT
        # window-relative slots (255 padding stays out of range for any W)
        real = slot_stream < 255.0
        slot_stream = slot_stream - wp0[None, :].astype(np.float32) * real
        # wrap-16 the index stream, replicate across the 8 16-part groups
        wrapped = idx_stream.reshape(-1, 16).T  # [16, TOT*8]
        idx_arr = np.tile(wrapped, (8, 1)).copy()  # [128, TOT*8]
        # host-built 0/1 one-hot for ACT tiles, window-relative and
        # variable-width (width ww[t] per tile, concatenated)
        f8 = mybir.dt.np(FP8)
        p01 = np.zeros((128, max(P01W, 1)), f8)
        for i, t in enumerate(act_idx):
            W = int(ww[t])
            off = int(act_woff[i])
            oh = (slot_stream[:, t][:, None] ==
                  np.arange(W, dtype=np.float32)[None, :])
            p01[:, off:off + W] = oh.astype(f8)
        per_core.append(dict(idx=idx_arr, slots=slot_stream, p01=p01))

    sched = dict(groups=groups, TOT=TOT, Th=Th, Tb=Tb, perm=perm,
                 eng=eng, act_pos=act_pos, NACT=NACT,
                 wp0=wp0, ww=ww, act_woff=act_woff, P01W=P01W)
    return sched, per_core


def build_consts(cfg, gate_w, gate_b):
    """[128, 258] f32: iota | W_bcast | ones | b"""
    C = np.zeros((128, 258), np.float32)
    C[:, 0:128] = np.arange(128, dtype=np.float32)[None, :]
    C[:, 128:256] = np.asarray(gate_w, np.float32).reshape(1, 128)
    C[:, 256] = 1.0
    # gate bias is a uniform score shift and cancels in the softmax, so
    # the device never applies it; column 257 is kept but unused
    C[:, 257] = np.float32(np.asarray(gate_b).reshape(-1)[0])
    return C


def build_program(cfg, sched, do_main=True, do_compute=True):
    nc = bacc.Bacc("TRN2", num_devices=cfg.NC,
                   dynamic_dma_scratch_size=49152)
    NPC, NBUK, GROUP = cfg.NPC, cfg.NBUK, cfg.GROUP
    TOT = sched["TOT"]
    groups = sched["groups"]
    eng_map = sched["eng"]
    act_pos = sched["act_pos"]
    NACT = sched["NACT"]
    wp0 = sched["wp0"]
    ww = sched["ww"]
    act_woff = sched["act_woff"]

    xlo = nc.dram_tensor("xlo", [cfg.LO_MAX, 128], F32,
                         kind="ExternalInput")
    xhi = nc.dram_tensor("xhi", [cfg.N - cfg.HI_BASE, 128], F32,
                         kind="ExternalInput")
    idx = nc.dram_tensor("idx", [128, TOT * 8], I16, kind="ExternalInput")
    slt = nc.dram_tensor("slt", [128, TOT], F32, kind="ExternalInput")
    p01 = nc.dram_tensor("p01", [128, max(sched["P01W"], 1)], FP8,
                         kind="ExternalInput")
    cst = nc.dram_tensor("cst", [128, 258], F32, kind="ExternalInput")
    # agg output, slot-major: out[p, b*128 + d] = agg[b*128 + p, d], bf16
    out = nc.dram_tensor("out", [128, NBUK * 128], BF16,
                         kind="ExternalOutput")

    with tile.TileContext(nc) as tc:
        with (
            tc.tile_pool(name="const", bufs=1) as cpool,
            tc.tile_pool(name="meta", bufs=1) as mpool,
            tc.tile_pool(name="gather", bufs=3) as gpool,
            tc.tile_pool(name="sc", bufs=6) as scpool,
            tc.tile_pool(name="grp", bufs=3) as grpool,
            tc.tile_pool(name="pp", bufs=8) as ppool,
            tc.tile_pool(name="fl", bufs=4) as flpool,
            tc.tile_pool(name="ag", bufs=3) as agpool,
            tc.tile_pool(name="pnum", bufs=8, space="PSUM") as pnum,
        ):
            C = cpool.tile([128, 258], F32)
            iota_ap = C[:, 0:128]
            w_ap = C[:, 128:256]
            ones_ap = C[:, 256:257]
            b_ap = C[:, 257:258]

            # the idx stream is uploaded in chunks emitted just ahead of
            # the gathers that read them: a single monolithic upload held
            # the first gather batch (and the whole pipeline) back ~10us
            idx_sb = mpool.tile([128, TOT * 8], I16)
            slt_sb = mpool.tile([128, TOT], F32)

            # ---- main loop over groups (gathers prefetched one group
            # ahead so the in-order Pool queue never parks later groups'
            # gathers behind P'-builds that wait on cross-engine deps) ----
            # each dma_gather is capped at GMAX tiles: the SWDGE ring
            # holds scratch/16 descriptors and one instruction's
            # descriptor set must fit entirely
            GMAX = 16
            work = [g for g in (groups if do_main else [])
                    if g["lo"] + g["hi"] > 0]
            tg_of = {}
            acc = 0
            for g in (groups if do_main else []):
                tg_of[id(g)] = acc
                acc += g["lo"] + g["hi"]

            gbufs = {}

            # idx chunks cover CHUNK_GROUPS consecutive work groups (kept
            # >=512B per partition to dodge the small-transfer penalty)
            CHUNK_GROUPS = 2
            chunk_of = {}
            chunk_rng = []
            for wi, g in enumerate(work):
                ci = wi // CHUNK_GROUPS
                chunk_of[id(g)] = ci
                tg0 = tg_of[id(g)]
                TGg = g["lo"] + g["hi"]
                if ci == len(chunk_rng):
                    chunk_rng.append([tg0, tg0 + TGg])
                else:
                    chunk_rng[ci][1] = tg0 + TGg
            idx_chunk_done = set()

            def emit_idx_chunk(ci):
                if ci in idx_chunk_done:
                    return
                idx_chunk_done.add(ci)
                r0, r1 = chunk_rng[ci]
                nc.sync.dma_start(idx_sb[:, r0 * 8:r1 * 8],
                                  idx[:, r0 * 8:r1 * 8])

            def emit_gathers(g, first_small=False):
                emit_idx_chunk(chunk_of[id(g)])
                TG = g["lo"] + g["hi"]
                tg0 = tg_of[id(g)]
                # extra zeroed tile at the end lets the last tile's matmul
                # read a 256-wide rhs (junk cols accumulate into psum cols
                # 128:256, never read)
                gbuf = gpool.tile([128, TG, 128], F32, tag="gbuf")
                for half, n_t, base in ((0, g["lo"], 0),
                                        (1, g["hi"], g["lo"])):
                    tbl = xlo if half == 0 else xhi
                    # a small leading batch lets the first scores start
                    # as soon as 4 tiles land instead of a full batch
                    cuts = list(range(0, n_t, GMAX))
                    if first_small and half == 0 and n_t > 4:
                        cuts = [0, 4] + [c for c in cuts if c > 4]
                    for ci, q0 in enumerate(cuts):
                        q1 = cuts[ci + 1] if ci + 1 < len(cuts) else min(
                            q0 + GMAX, n_t)
                        b0t = base + q0
                        g0 = tg0 + b0t
                        nc.gpsimd.dma_gather(
                            out_ap=gbuf[:, b0t:base + q1, :],
                            in_ap=tbl[:],
                            idxs_ap=idx_sb[:, g0 * 8:(g0 + q1 - q0) * 8],
                            num_idxs=(q1 - q0) * 128,
                            num_idxs_reg=(q1 - q0) * 128,
                            elem_size=128,
                            single_packet=False,
                        )
                gbufs[id(g)] = gbuf

            # zero-fill output for empty groups up front
            for g in (groups if do_main else []):
                if g["lo"] + g["hi"] == 0:
                    nbk = g["b1"] - g["b0"]
                    aggg = agpool.tile([128, nbk * 128], BF16, tag="aggg")
                    nc.vector.memset(aggg[:], 0.0)
                    nc.sync.dma_start(
                        out[:, g["b0"] * 128:(g["b0"] + nbk) * 128], aggg[:])

            PREFETCH = 2  # groups of gathers in flight ahead (gbuf bufs-1)
            for wi, g in enumerate(work[:PREFETCH]):
                emit_gathers(g)
            # consts and slot stream issue after the pipeline-critical
            # first gathers (each DMA issue serializes ~0.6us on HWDGE)
            nc.sync.dma_start(C[:], cst[:])
            nc.sync.dma_start(slt_sb[:], slt[:])
            for wi, g in enumerate(work):
                if wi + PREFETCH < len(work):
                    emit_gathers(work[wi + PREFETCH])
                TG = g["lo"] + g["hi"]
                nbk = g["b1"] - g["b0"]
                t_glob = tg_of[id(g)]
                gbuf = gbufs.pop(id(g))
                aggg = agpool.tile([128, nbk * 128], BF16, tag="aggg")

                # p01 stream for this group's ACT-assigned tiles
                # (variable widths, contiguous in the global ACT ordering)
                acts = [t for t in range(TG) if eng_map[t_glob + t] == ENG_ACT]
                nact = len(acts)
                if nact > 0:
                    ai0 = int(act_pos[t_glob + acts[0]])
                    w0 = int(act_woff[ai0])
                    w1 = int(act_woff[ai0 + nact])
                    p01_sb = grpool.tile([128, w1 - w0], FP8, tag="p01sb")
                    nc.sync.dma_start(p01_sb[:], p01[:, w0:w1])

                if not do_compute:
                    # still consume gbuf so Tile keeps the gathers
                    sink = flpool.tile([128, 1], F32, tag="sink")
                    nc.vector.tensor_scalar(
                        out=sink[:], in0=gbuf[:, 0, 0:1], scalar1=1.0,
                        scalar2=None, op0=OP.mult)
                    nc.sync.dma_start(out[0:128, 0:1], sink[:].bitcast(BF16)[:, 0:1])
                    continue
                # scores for all tiles of the group
                sg = grpool.tile([128, TG], F32, tag="sg")
                eg = grpool.tile([128, TG], F32, tag="eg")
                for t in range(TG):
                    scr = scpool.tile([128, 128], F32, tag="scr",
                                      name="scr_v")
                    sc_eng = (nc.gpsimd if (t_glob + t) % 100 <
                              SC_POOL * 100 else nc.vector)
                    sc_eng.scalar_tensor_tensor(
                        out=scr[:], in0=gbuf[:, t, :], scalar=1.0,
                        in1=w_ap, op0=OP.mult, op1=OP.mult,
                        accum_out=sg[:, t:t + 1])
                    # per-16-tile exp keeps the pipeline fine-grained
                    if t % 16 == 15 or t == TG - 1:
                        lo8 = (t // 16) * 16
                        nc.scalar.activation(out=eg[:, lo8:t + 1],
                                             in_=sg[:, lo8:t + 1],
                                             func=AF.Exp,
                                             bias=0.0, scale=1.0)

                # per-bucket psum tiles: cols 0:128 numerator, col 128 denom
                psums = {}
                for j in range(nbk):
                    if sched["Tb"][g["b0"] + j] > 0:
                        psums[j] = pnum.tile([128, 129], F32, tag="pn",
                                             name=f"pn_{g['b0']}_{j}")

                for t, ti in enumerate(g["tiles"]):
                    j = ti["j"]
                    tg = t_glob + t
                    p0 = int(wp0[tg])
                    W = int(ww[tg])
                    Pp = ppool.tile([128, W], F32, tag=f"pp{W}")
                    ek = eng_map[tg]
                    if ek == ENG_ACT:
                        # ACT path: P' = host-built 0/1 one-hot (fp8,
                        # exact, window-relative) scaled by e in one Copy
                        ai = int(act_pos[tg])
                        o0 = int(act_woff[ai]) - int(
                            act_woff[int(act_pos[t_glob + acts[0]])])
                        nc.scalar.activation(
                            out=Pp[:], in_=p01_sb[:, o0:o0 + W],
                            func=AF.Copy, scale=eg[:, t:t + 1])
                    else:
                        nc.vector.tensor_scalar(
                            out=Pp[:], in0=iota_ap[:, 0:W],
                            scalar1=slt_sb[:, tg:tg + 1],
                            scalar2=eg[:, t:t + 1],
                            op0=OP.is_equal, op1=OP.mult)
                    # slot-sorted tiles cover a narrow aligned window
                    # [p0, p0+W) of the bucket's 128 slots; the matmul
                    # writes just those psum partitions. The bucket's
                    # first tile is forced to the full window so its
                    # start=True clears every partition of the bank.
                    nc.tensor.matmul(
                        out=psums[j][p0:p0 + W, 0:128], lhsT=Pp[:],
                        rhs=gbuf[:, t, :],
                        start=ti["first"], stop=False)
                    nc.tensor.matmul(
                        out=psums[j][p0:p0 + W, 128:129], lhsT=Pp[:],
                        rhs=ones_ap,
                        start=False, stop=ti["last"])

                # flush group: per-bucket reciprocal + scale into the
                # group's bf16 slot-major agg tile, then one DMA out
                for j in range(nbk):
                    if j in psums:
                        # no epsilon guard: slots with zero in-degree give
                        # den=0 -> inf/NaN rows, which the host overwrites
                        # with zeros (it knows the in-degrees from
                        # edge_index); skipping the max() op saves ~6us DVE
                        rcp = flpool.tile([128, 1], F32, tag="rcp")
                        nc.vector.reciprocal(out=rcp[:],
                                             in_=psums[j][:, 128:129])
                        nc.scalar.activation(
                            out=aggg[:, j * 128:(j + 1) * 128],
                            in_=psums[j][:, 0:128],
                            func=AF.Copy, scale=rcp[:, 0:1])
                    else:
                        nc.vector.memset(aggg[:, j * 128:(j + 1) * 128], 0.0)
                nc.sync.dma_start(
                    out[:, g["b0"] * 128:(g["b0"] + nbk) * 128], aggg[:])

    nc.compile()
    return nc


def make_in_maps(cfg, sched, per_core, x, gate_w, gate_b):
    x = np.asarray(x, np.float32)
    consts = build_consts(cfg, gate_w, gate_b)
    in_maps = []
    for core in range(cfg.NC):
        in_maps.append({
            "xlo": x[:cfg.LO_MAX],
            "xhi": x[cfg.HI_BASE:],
            "idx": per_core[core]["idx"],
            "slt": per_core[core]["slots"],
            "p01": per_core[core]["p01"],
            "cst": consts,
        })
    return in_maps


def _kernel_impl(x, gate_w, gate_b, edge_index, cfg=None, return_nc=False):
    from concourse.bass_utils import run_bass_kernel_spmd
    if cfg is None:
        cfg = Config()
    sched, per_core = build_schedule(cfg, edge_index[0], edge_index[1])
    nc = build_program(cfg, sched)
    in_maps = make_in_maps(cfg, sched, per_core, x, gate_w, gate_b)
    res = run_bass_kernel_spmd(nc, in_maps, core_ids=list(range(cfg.NC)))
    perm = sched["perm"]
    outp = np.zeros((cfg.N, 256), np.float32)
    outp[:, 0:128] = x
    indeg = np.bincount(np.asarray(edge_index[1], np.int64),
                        minlength=cfg.N)
    for core in range(cfg.NC):
        # o: [128, NBUK*128] bf16 slot-major -> [NBUK, 128, 128] agg
        o = np.asarray(res.results[core]["out"], dtype=np.float32)
        o = o.reshape(128, cfg.NBUK, 128).transpose(1, 0, 2)
        base = core * cfg.NPC
        for k in range(cfg.NBUK):
            b = int(perm[core, k])
            v = min(128, cfg.NPC - b * 128)
            outp[base + b * 128:base + b * 128 + v, 128:256] = o[k, :v]
    outp[indeg == 0, 128:256] = 0.0
    if return_nc:
        return outp, nc
    return outp


def kernel(**inputs):
    """Harness entry: full unsharded inputs -> full [50000, 256] f32 output.

    Shards edges by destination-node range across the 8 NeuronCores
    (each core computes its 6250-node output slice fully locally),
    compiles the Bass program, and runs it via run_bass_kernel_spmd.
    """
    x = np.ascontiguousarray(np.asarray(inputs["x"], np.float32))
    gate_w = np.asarray(inputs["gate_w"], np.float32)
    gate_b = np.asarray(inputs["gate_b"], np.float32)
    edge_index = np.asarray(inputs["edge_index"])
    return _kernel_impl(x, gate_w, gate_b, edge_index)


# revision 51
# speedup vs baseline: 1.0114x; 1.0114x over previous
"""AttentionalAggregation GNN kernel for 8 TRN2 NeuronCores.

Strategy: edges sorted by destination bucket on host; core m owns nodes
[m*NPC, (m+1)*NPC) and computes its output slice fully locally (no
collectives). Per 128-edge tile:
  - dma_gather x[src] rows (512B each) from lo/hi half tables (int16 idx)
  - score_e = sum_d V[e,d]*w[d]  (scalar_tensor_tensor accum)
  - e = exp(score + b)           (ACT)
  - P'[e,s] = (iota_s == slot_e) * e_e   (built on DVE, Pool or ACT,
    split to balance engine load; ACT path reads a host-built 0/1
    one-hot in fp8 and scales by e in one Copy op)
  - psum[bucket][:,0:128] += P'.T @ V_t   (plain f32 matmul: hardware
    f32r is tf32-grade and fails the accuracy budget)
  - psum[bucket][:,128]   += P'.T @ ones  (denominator)
Flush per group: reciprocal of denominator, scale, write agg as bf16 in
slot-major layout; host unpermutes, casts to f32 and concats with x.
"""

import math
import numpy as np

import concourse.bass as bass
import concourse.mybir as mybir
import concourse.tile as tile
from concourse import bacc

F32 = mybir.dt.float32
F32R = mybir.dt.float32r
BF16 = mybir.dt.bfloat16
FP8 = mybir.dt.float8e4
I16 = mybir.dt.int16
AF = mybir.ActivationFunctionType
OP = mybir.AluOpType

# engine assignment for building P' (one-hot * e) per tile
ENG_DVE = 0
ENG_POOL = 1
ENG_ACT = 2
# shares (DVE, POOL, ACT) of P'-build work; ACT tiles need the p01 stream.
# Pool-engine P' builds stall the in-order Pool queue behind cross-engine
# waits (gathers share it), so the Pool share stays 0.
PP_SHARES = (0.28, 0.0, 0.72)
# fraction of score STTs routed to the Pool engine. Keep 0: any non-SWDGE
# work on the in-order Pool engine delays later groups' descriptor
# generation and starves the DMA engines.
SC_POOL = 0.0


class Config:
    def __init__(self, N=50000, E=640000, D=128, NC=8, GROUP=2):
        assert D == 128
        self.N, self.E, self.D, self.NC = N, E, D, NC
        self.NPC = N // NC          # nodes per core
        assert self.NPC * NC == N
        # overlapping lo/hi gather tables (int16 index limit 32768 rows);
        # sources in the overlap may be assigned to either run, letting the
        # host pad the lo run to a tile boundary with real edges
        self.LO_MAX = min(32768, N)
        self.HI_BASE = max(0, N - 32768)
        self.NBUK = math.ceil(self.NPC / 128)   # buckets per core
        self.TAIL = self.NPC - (self.NBUK - 1) * 128  # rows in last bucket
        self.GROUP = GROUP
        self.NG = math.ceil(self.NBUK / GROUP)


def assign_engines(TOT):
    """Per-tile engine for the P' build, interleaved to the target shares."""
    eng = np.zeros(TOT, np.int8)
    acc = [0.0, 0.0, 0.0]
    for t in range(TOT):
        # pick the engine furthest below its target share
        deficits = [PP_SHARES[k] * (t + 1) - acc[k] for k in range(3)]
        k = int(np.argmax(deficits))
        eng[t] = k
        acc[k] += 1.0
    return eng


def build_schedule(cfg, src, dst):
    """Host-side: sort/pad edges into a static per-tile schedule uniform
    across cores. Returns (sched, per_core) where sched is the static
    structure (identical across cores) and per_core has the data arrays."""
    N, NC, NPC, NBUK, GROUP = (
        cfg.N, cfg.NC, cfg.NPC, cfg.NBUK, cfg.GROUP)
    LO_MAX, HI_BASE = cfg.LO_MAX, cfg.HI_BASE

    src = np.asarray(src, np.int64)
    dst = np.asarray(dst, np.int64)
    c = dst // NPC
    r = dst % NPC
    lb = r // 128
    slot = r % 128

    order = np.lexsort((src, lb, c))
    src_s, lb_s, slot_s, c_s = (
        src[order], lb[order], slot[order], c[order])

    key = c_s * NBUK + lb_s
    cnt = np.bincount(key, minlength=NC * NBUK).reshape(NC, NBUK)
    starts = np.zeros(NC * NBUK + 1, np.int64)
    np.cumsum(cnt.reshape(-1), out=starts[1:])

    # within each (core, bucket) slice (sorted by src), edges below
    # HI_BASE must use the lo table, edges >= LO_MAX must use hi, and the
    # overlap is flexible: cut the slice to fill lo tiles exactly
    n_lo_min = np.zeros((NC, NBUK), np.int64)
    n_lo_cap = np.zeros((NC, NBUK), np.int64)
    for cc in range(NC):
        for b in range(NBUK):
            k = cc * NBUK + b
            sl = src_s[starts[k]:starts[k + 1]]
            n_lo_min[cc, b] = np.searchsorted(sl, HI_BASE)
            n_lo_cap[cc, b] = np.searchsorted(sl, LO_MAX)
    # per-core needs, then sort each core's buckets by total tiles
    # descending so position-wise maxima across cores are tight
    t_lo_c = np.ceil(n_lo_min / 128.0).astype(np.int64)       # [NC, NBUK]
    lo_cap_pos = n_lo_cap
    t_hi_c = np.ceil(np.maximum(cnt - np.minimum(128 * t_lo_c, lo_cap_pos),
                                0) / 128.0).astype(np.int64)
    tot_c = t_lo_c + t_hi_c
    perm = np.argsort(-tot_c, axis=1, kind="stable")          # [NC, NBUK]
    ar = np.arange(NC)[:, None]
    T_lo = t_lo_c[ar, perm].max(axis=0)                       # [NBUK] by pos
    lo_count_pos = np.minimum(128 * T_lo[None, :], n_lo_cap[ar, perm])
    T_hi = np.ceil((cnt[ar, perm] - lo_count_pos) / 128.0
                   ).astype(np.int64).max(axis=0)
    Th = np.stack([T_lo, T_hi], axis=1)  # [NBUK, 2] by position
    # scatter position-based lo counts back to per-(core,bucket)
    lo_count = np.zeros_like(cnt)
    np.put_along_axis(lo_count, perm, lo_count_pos, axis=1)
    # move the smallest bucket to position 0: paired with a single-bucket
    # first group below, it halves the ramp-critical first gather batch
    pi = np.r_[NBUK - 1, NBUK - 2, np.arange(NBUK - 2)]
    perm = perm[:, pi]
    Th = Th[pi]
    Tb = Th.sum(axis=1)

    # static tile stream: per group g: [lo tiles of buckets][hi tiles]
    # each entry: (bucket_local_index_in_group j, bucket b, first, last)
    sizes = [1, 1]
    left = NBUK - 2
    while left > 0:
        take = min(GROUP, left)
        sizes.append(take)
        left -= take
    bounds = np.cumsum([0] + sizes)
    groups = []
    for g in range(len(sizes)):
        b0, b1 = int(bounds[g]), int(bounds[g + 1])
        tiles = []
        for h in (0, 1):
            for b in range(b0, b1):
                nt = int(Th[b, h])
                for t in range(nt):
                    pos = t if h == 0 else int(Th[b, 0]) + t
                    first = pos == 0
                    last = pos == int(Tb[b]) - 1
                    tiles.append(dict(j=b - b0, b=b, first=first, last=last))
        lo_tiles = int(Th[b0:b1, 0].sum())
        hi_tiles = int(Th[b0:b1, 1].sum())
        groups.append(dict(b0=b0, b1=b1, lo=lo_tiles, hi=hi_tiles,
                           tiles=tiles))
    TOT = sum(g["lo"] + g["hi"] for g in groups)

    # engine assignment per global tile; ACT tiles get p01 stream slots
    eng = assign_engines(TOT)
    act_idx = np.nonzero(eng == ENG_ACT)[0]
    act_pos = np.full(TOT, -1, np.int64)    # tile -> index among ACT tiles
    for i, t in enumerate(act_idx):
        act_pos[t] = i
    NACT = len(act_idx)

    # pass A: per-tile slot-span unions across cores (edges re-sorted by
    # slot within each (bucket, half) run so a tile covers a narrow
    # contiguous window; matmul partition offsets must be 32-aligned with
    # legal widths {32 @0/32/64/96, 64 @0/64, 128 @0})
    span_lo = np.full(TOT, 255, np.int64)
    span_hi = np.full(TOT, -1, np.int64)
    for core in range(NC):
        t_glob = 0
        for g in groups:
            for h in (0, 1):
                for pos in range(g["b0"], g["b1"]):
                    b = int(perm[core, pos])
                    k = core * NBUK + b
                    s0, s1 = starts[k], starts[k + 1]
                    cut = s0 + lo_count[core, b]
                    e_slot = (slot_s[s0:cut] if h == 0
                              else slot_s[cut:s1])
                    nt = int(Th[pos, h])
                    if nt > 0 and len(e_slot) > 0:
                        sl = np.sort(e_slot)
                        for kk in range(nt):
                            seg = sl[kk * 128:(kk + 1) * 128]
                            if len(seg):
                                t = t_glob + kk
                                span_lo[t] = min(span_lo[t], int(seg[0]))
                                span_hi[t] = max(span_hi[t], int(seg[-1]))
                    t_glob += nt

    def legal_window(a, b):
        # base partitions allowed by the AP layer: {0, 32, 64} only
        if b < 0:
            return 0, 32
        p = 32 * (a // 32)
        if p <= 64 and b - p < 32:
            return p, 32
        p = 64 * (a // 64)
        if b - p < 64:
            return p, 64
        return 0, 128

    win = [legal_window(int(span_lo[t]), int(span_hi[t]))
           for t in range(TOT)]
    # the bucket-opening tile (start=True) must cover all 128 partitions:
    # hardware clears the full bank row only for partitions the matmul
    # writes (validated: narrow first tiles corrupt untouched partitions).
    # Swap a naturally-full-window slice to the front of the bucket's
    # first run when one exists; otherwise widen the first tile.
    slice_perm = {}  # (pos, h) -> permutation of the run's slice indices
    t_glob = 0
    for g in groups:
        for h in (0, 1):
            for pos in range(g["b0"], g["b1"]):
                nt = int(Th[pos, h])
                if nt == 0:
                    continue
                is_first_run = (h == 0) or int(Th[pos, 0]) == 0
                if is_first_run:
                    p = list(range(nt))
                    kfull = next((k for k in range(nt)
                                  if win[t_glob + k] == (0, 128)), None)
                    if kfull is not None and kfull != 0:
                        p[0], p[kfull] = p[kfull], p[0]
                        slice_perm[(pos, h)] = p
                        w2 = [win[t_glob + k] for k in p]
                        for k in range(nt):
                            win[t_glob + k] = w2[k]
                    elif kfull is None:
                        win[t_glob] = (0, 128)
                t_glob += nt
    wp0 = np.array([w[0] for w in win], np.int64)
    ww = np.array([w[1] for w in win], np.int64)
    # variable-width p01 offsets for ACT tiles
    act_woff = np.zeros(NACT + 1, np.int64)
    np.cumsum(ww[act_idx], out=act_woff[1:])
    P01W = int(act_woff[-1])

    # per-core data arrays
    per_core = []
    for core in range(NC):
        idx_stream = np.zeros(TOT * 128, np.int16)
        slot_stream = np.full((128, TOT), 255.0, np.float32)
        t_glob = 0
        for g in groups:
            for h in (0, 1):
                for pos in range(g["b0"], g["b1"]):
                    b = int(perm[core, pos])
                    k = core * NBUK + b
                    s0, s1 = starts[k], starts[k + 1]
                    cut = s0 + lo_count[core, b]
                    if h == 0:
                        e_src = src_s[s0:cut]
                        e_slot = slot_s[s0:cut]
                    else:
                        e_src = src_s[cut:s1] - HI_BASE
                        e_slot = slot_s[cut:s1]
                    o2 = np.argsort(e_slot, kind="stable")
                    e_src = e_src[o2]
                    e_slot = e_slot[o2]
                    sp = slice_perm.get((pos, h))
                    if sp is not None and len(e_src) > 0:
                        segs = [e_src[k * 128:(k + 1) * 128] for k in sp]
                        segt = [e_slot[k * 128:(k + 1) * 128] for k in sp]
                        e_src = np.concatenate(segs)
                        e_slot = np.concatenate(segt)
                    n = len(e_src)
                    nt = int(Th[pos, h])
                    base = t_glob * 128
                    if n > 0:
                        idx_stream[base:base + n] = e_src.astype(np.int16)
                        fl = np.full(nt * 128, 255.0, np.float32)
                        fl[:n] = e_slot.astype(np.float32)
                        slot_stream[:, t_glob:t_glob + nt] = (
                            fl.reshape(nt, 128).T)
                    t_glob += nt
        assert t_glob == TOT
        # window-relative slots (255 padding stays out of range for any W)
        real = slot_stream < 255.0
        slot_stream = slot_stream - wp0[None, :].astype(np.float32) * real
        # wrap-16 the index stream, replicate across the 8 16-part groups
        wrapped = idx_stream.reshape(-1, 16).T  # [16, TOT*8]
        idx_arr = np.tile(wrapped, (8, 1)).copy()  # [128, TOT*8]
        # host-built 0/1 one-hot for ACT tiles, window-relative and
        # variable-width (width ww[t] per tile, concatenated)
        f8 = mybir.dt.np(FP8)
        p01 = np.zeros((128, max(P01W, 1)), f8)
        for i, t in enumerate(act_idx):
            W = int(ww[t])
            off = int(act_woff[i])
            oh = (slot_stream[:, t][:, None] ==
                  np.arange(W, dtype=np.float32)[None, :])
            p01[:, off:off + W] = oh.astype(f8)
        per_core.append(dict(idx=idx_arr, slots=slot_stream, p01=p01))

    sched = dict(groups=groups, TOT=TOT, Th=Th, Tb=Tb, perm=perm,
                 eng=eng, act_pos=act_pos, NACT=NACT,
                 wp0=wp0, ww=ww, act_woff=act_woff, P01W=P01W)
    return sched, per_core


def build_consts(cfg, gate_w, gate_b):
    """[128, 258] f32: iota | W_bcast | ones | b"""
    C = np.zeros((128, 258), np.float32)
    C[:, 0:128] = np.arange(128, dtype=np.float32)[None, :]
    C[:, 128:256] = np.asarray(gate_w, np.float32).reshape(1, 128)
    C[:, 256] = 1.0
    # gate bias is a uniform score shift and cancels in the softmax, so
    # the device never applies it; column 257 is kept but unused
    C[:, 257] = np.float32(np.asarray(gate_b).reshape(-1)[0])
    return C


def build_program(cfg, sched, do_main=True, do_compute=True):
    nc = bacc.Bacc("TRN2", num_devices=cfg.NC,
                   dynamic_dma_scratch_size=49152)
    NPC, NBUK, GROUP = cfg.NPC, cfg.NBUK, cfg.GROUP
    TOT = sched["TOT"]
    groups = sched["groups"]
    eng_map = sched["eng"]
    act_pos = sched["act_pos"]
    NACT = sched["NACT"]
    wp0 = sched["wp0"]
    ww = sched["ww"]
    act_woff = sched["act_woff"]

    xlo = nc.dram_tensor("xlo", [cfg.LO_MAX, 128], F32,
                         kind="ExternalInput")
    xhi = nc.dram_tensor("xhi", [cfg.N - cfg.HI_BASE, 128], F32,
                         kind="ExternalInput")
    idx = nc.dram_tensor("idx", [128, TOT * 8], I16, kind="ExternalInput")
    slt = nc.dram_tensor("slt", [128, TOT], F32, kind="ExternalInput")
    p01 = nc.dram_tensor("p01", [128, max(sched["P01W"], 1)], FP8,
                         kind="ExternalInput")
    cst = nc.dram_tensor("cst", [128, 258], F32, kind="ExternalInput")
    # agg output, slot-major: out[p, b*128 + d] = agg[b*128 + p, d], bf16
    out = nc.dram_tensor("out", [128, NBUK * 128], BF16,
                         kind="ExternalOutput")

    with tile.TileContext(nc) as tc:
        with (
            tc.tile_pool(name="const", bufs=1) as cpool,
            tc.tile_pool(name="meta", bufs=1) as mpool,
            tc.tile_pool(name="gather", bufs=3) as gpool,
            tc.tile_pool(name="sc", bufs=6) as scpool,
            tc.tile_pool(name="grp", bufs=3) as grpool,
            tc.tile_pool(name="pp", bufs=8) as ppool,
            tc.tile_pool(name="fl", bufs=4) as flpool,
            tc.tile_pool(name="ag", bufs=3) as agpool,
            tc.tile_pool(name="pnum", bufs=8, space="PSUM") as pnum,
        ):
            C = cpool.tile([128, 258], F32)
            iota_ap = C[:, 0:128]
            w_ap = C[:, 128:256]
            ones_ap = C[:, 256:257]
            b_ap = C[:, 257:258]

            # the idx stream is uploaded in chunks emitted just ahead of
            # the gathers that read them: a single monolithic upload held
            # the first gather batch (and the whole pipeline) back ~10us
            idx_sb = mpool.tile([128, TOT * 8], I16)
            slt_sb = mpool.tile([128, TOT], F32)

            # ---- main loop over groups (gathers prefetched one group
            # ahead so the in-order Pool queue never parks later groups'
            # gathers behind P'-builds that wait on cross-engine deps) ----
            # each dma_gather is capped at GMAX tiles: the SWDGE ring
            # holds scratch/16 descriptors and one instruction's
            # descriptor set must fit entirely
            GMAX = 16
            work = [g for g in (groups if do_main else [])
                    if g["lo"] + g["hi"] > 0]
            tg_of = {}
            acc = 0
            for g in (groups if do_main else []):
                tg_of[id(g)] = acc
                acc += g["lo"] + g["hi"]

            gbufs = {}

            # idx chunks cover CHUNK_GROUPS consecutive work groups (kept
            # >=512B per partition to dodge the small-transfer penalty)
            CHUNK_GROUPS = 2
            chunk_of = {}
            chunk_rng = []
            for wi, g in enumerate(work):
                ci = wi // CHUNK_GROUPS
                chunk_of[id(g)] = ci
                tg0 = tg_of[id(g)]
                TGg = g["lo"] + g["hi"]
                if ci == len(chunk_rng):
                    chunk_rng.append([tg0, tg0 + TGg])
                else:
                    chunk_rng[ci][1] = tg0 + TGg
            idx_chunk_done = set()

            def emit_idx_chunk(ci):
                if ci in idx_chunk_done:
                    return
                idx_chunk_done.add(ci)
                r0, r1 = chunk_rng[ci]
                nc.sync.dma_start(idx_sb[:, r0 * 8:r1 * 8],
                                  idx[:, r0 * 8:r1 * 8])

            def emit_gathers(g, first_small=False):
                emit_idx_chunk(chunk_of[id(g)])
                TG = g["lo"] + g["hi"]
                tg0 = tg_of[id(g)]
                # extra zeroed tile at the end lets the last tile's matmul
                # read a 256-wide rhs (junk cols accumulate into psum cols
                # 128:256, never read)
                gbuf = gpool.tile([128, TG, 128], F32, tag="gbuf")
                for half, n_t, base in ((0, g["lo"], 0),
                                        (1, g["hi"], g["lo"])):
                    tbl = xlo if half == 0 else xhi
                    # a small leading batch lets the first scores start
                    # as soon as 4 tiles land instead of a full batch
                    cuts = list(range(0, n_t, GMAX))
                    if first_small and half == 0 and n_t > 4:
                        cuts = [0, 4] + [c for c in cuts if c > 4]
                    for ci, q0 in enumerate(cuts):
                        q1 = cuts[ci + 1] if ci + 1 < len(cuts) else min(
                            q0 + GMAX, n_t)
                        b0t = base + q0
                        g0 = tg0 + b0t
                        nc.gpsimd.dma_gather(
                            out_ap=gbuf[:, b0t:base + q1, :],
                            in_ap=tbl[:],
                            idxs_ap=idx_sb[:, g0 * 8:(g0 + q1 - q0) * 8],
                            num_idxs=(q1 - q0) * 128,
                            num_idxs_reg=(q1 - q0) * 128,
                            elem_size=128,
                            single_packet=False,
                        )
                gbufs[id(g)] = gbuf

            # zero-fill output for empty groups up front
            for g in (groups if do_main else []):
                if g["lo"] + g["hi"] == 0:
                    nbk = g["b1"] - g["b0"]
                    aggg = agpool.tile([128, nbk * 128], BF16, tag="aggg")
                    nc.vector.memset(aggg[:], 0.0)
                    nc.sync.dma_start(
                        out[:, g["b0"] * 128:(g["b0"] + nbk) * 128], aggg[:])

            PREFETCH = 2  # groups of gathers in flight ahead (gbuf bufs-1)
            for wi, g in enumerate(work[:PREFETCH]):
                emit_gathers(g)
            # consts and slot stream issue after the pipeline-critical
            # first gathers (each DMA issue serializes ~0.6us on HWDGE)
            nc.sync.dma_start(C[:], cst[:])
            nc.sync.dma_start(slt_sb[:], slt[:])
            for wi, g in enumerate(work):
                if wi + PREFETCH < len(work):
                    emit_gathers(work[wi + PREFETCH])
                TG = g["lo"] + g["hi"]
                nbk = g["b1"] - g["b0"]
                t_glob = tg_of[id(g)]
                gbuf = gbufs.pop(id(g))
                aggg = agpool.tile([128, nbk * 128], BF16, tag="aggg")

                # p01 stream for this group's ACT-assigned tiles
                # (variable widths, contiguous in the global ACT ordering)
                acts = [t for t in range(TG) if eng_map[t_glob + t] == ENG_ACT]
                nact = len(acts)
                if nact > 0:
                    ai0 = int(act_pos[t_glob + acts[0]])
                    w0 = int(act_woff[ai0])
                    w1 = int(act_woff[ai0 + nact])
                    p01_sb = grpool.tile([128, w1 - w0], FP8, tag="p01sb")
                    nc.sync.dma_start(p01_sb[:], p01[:, w0:w1])

                if not do_compute:
                    # still consume gbuf so Tile keeps the gathers
                    sink = flpool.tile([128, 1], F32, tag="sink")
                    nc.vector.tensor_scalar(
                        out=sink[:], in0=gbuf[:, 0, 0:1], scalar1=1.0,
                        scalar2=None, op0=OP.mult)
                    nc.sync.dma_start(out[0:128, 0:1], sink[:].bitcast(BF16)[:, 0:1])
                    continue
                # scores for all tiles of the group
                sg = grpool.tile([128, TG], F32, tag="sg")
                eg = grpool.tile([128, TG], F32, tag="eg")
                for t in range(TG):
                    scr = scpool.tile([128, 128], F32, tag="scr",
                                      name="scr_v")
                    sc_eng = (nc.gpsimd if (t_glob + t) % 100 <
                              SC_POOL * 100 else nc.vector)
                    sc_eng.scalar_tensor_tensor(
                        out=scr[:], in0=gbuf[:, t, :], scalar=1.0,
                        in1=w_ap, op0=OP.mult, op1=OP.mult,
                        accum_out=sg[:, t:t + 1])
                    # per-16-tile exp keeps the pipeline fine-grained
                    if t % 16 == 15 or t == TG - 1:
                        lo8 = (t // 16) * 16
                        nc.scalar.activation(out=eg[:, lo8:t + 1],
                                             in_=sg[:, lo8:t + 1],
                                             func=AF.Exp,
                                             bias=0.0, scale=1.0)

                # per-bucket psum tiles: cols 0:128 numerator, col 128 denom
                psums = {}
                for j in range(nbk):
                    if sched["Tb"][g["b0"] + j] > 0:
                        psums[j] = pnum.tile([128, 129], F32, tag="pn",
                                             name=f"pn_{g['b0']}_{j}")

                for t, ti in enumerate(g["tiles"]):
                    j = ti["j"]
                    tg = t_glob + t
                    p0 = int(wp0[tg])
                    W = int(ww[tg])
                    Pp = ppool.tile([128, W], F32, tag=f"pp{W}")
                    ek = eng_map[tg]
                    if ek == ENG_ACT:
                        # ACT path: P' = host-built 0/1 one-hot (fp8,
                        # exact, window-relative) scaled by e in one Copy
                        ai = int(act_pos[tg])
                        o0 = int(act_woff[ai]) - int(
                            act_woff[int(act_pos[t_glob + acts[0]])])
                        nc.scalar.activation(
                            out=Pp[:], in_=p01_sb[:, o0:o0 + W],
                            func=AF.Copy, scale=eg[:, t:t + 1])
                    else:
                        nc.vector.tensor_scalar(
                            out=Pp[:], in0=iota_ap[:, 0:W],
                            scalar1=slt_sb[:, tg:tg + 1],
                            scalar2=eg[:, t:t + 1],
                            op0=OP.is_equal, op1=OP.mult)
                    # slot-sorted tiles cover a narrow aligned window
                    # [p0, p0+W) of the bucket's 128 slots; the matmul
                    # writes just those psum partitions. The bucket's
                    # first tile is forced to the full window so its
                    # start=True clears every partition of the bank.
                    nc.tensor.matmul(
                        out=psums[j][p0:p0 + W, 0:128], lhsT=Pp[:],
                        rhs=gbuf[:, t, :],
                        start=ti["first"], stop=False)
                    nc.tensor.matmul(
                        out=psums[j][p0:p0 + W, 128:129], lhsT=Pp[:],
                        rhs=ones_ap,
                        start=False, stop=ti["last"])

                # flush group: per-bucket reciprocal + scale into the
                # group's bf16 slot-major agg tile, then one DMA out
                for j in range(nbk):
                    if j in psums:
                        # no epsilon guard: slots with zero in-degree give
                        # den=0 -> inf/NaN rows, which the host overwrites
                        # with zeros (it knows the in-degrees from
                        # edge_index); skipping the max() op saves ~6us DVE
                        rcp = flpool.tile([128, 1], F32, tag="rcp")
                        nc.vector.reciprocal(out=rcp[:],
                                             in_=psums[j][:, 128:129])
                        nc.scalar.activation(
                            out=aggg[:, j * 128:(j + 1) * 128],
                            in_=psums[j][:, 0:128],
                            func=AF.Copy, scale=rcp[:, 0:1])
                    else:
                        nc.vector.memset(aggg[:, j * 128:(j + 1) * 128], 0.0)
                nc.sync.dma_start(
                    out[:, g["b0"] * 128:(g["b0"] + nbk) * 128], aggg[:])

    nc.compile()
    return nc


def make_in_maps(cfg, sched, per_core, x, gate_w, gate_b):
    x = np.asarray(x, np.float32)
    consts = build_consts(cfg, gate_w, gate_b)
    in_maps = []
    for core in range(cfg.NC):
        in_maps.append({
            "xlo": x[:cfg.LO_MAX],
            "xhi": x[cfg.HI_BASE:],
            "idx": per_core[core]["idx"],
            "slt": per_core[core]["slots"],
            "p01": per_core[core]["p01"],
            "cst": consts,
        })
    return in_maps


def _kernel_impl(x, gate_w, gate_b, edge_index, cfg=None, return_nc=False):
    from concourse.bass_utils import run_bass_kernel_spmd
    if cfg is None:
        cfg = Config()
    sched, per_core = build_schedule(cfg, edge_index[0], edge_index[1])
    nc = build_program(cfg, sched)
    in_maps = make_in_maps(cfg, sched, per_core, x, gate_w, gate_b)
    res = run_bass_kernel_spmd(nc, in_maps, core_ids=list(range(cfg.NC)))
    perm = sched["perm"]
    outp = np.zeros((cfg.N, 256), np.float32)
    outp[:, 0:128] = x
    indeg = np.bincount(np.asarray(edge_index[1], np.int64),
                        minlength=cfg.N)
    for core in range(cfg.NC):
        # o: [128, NBUK*128] bf16 slot-major -> [NBUK, 128, 128] agg
        o = np.asarray(res.results[core]["out"], dtype=np.float32)
        o = o.reshape(128, cfg.NBUK, 128).transpose(1, 0, 2)
        base = core * cfg.NPC
        for k in range(cfg.NBUK):
            b = int(perm[core, k])
            v = min(128, cfg.NPC - b * 128)
            outp[base + b * 128:base + b * 128 + v, 128:256] = o[k, :v]
    outp[indeg == 0, 128:256] = 0.0
    if return_nc:
        return outp, nc
    return outp


def kernel(**inputs):
    """Harness entry: full unsharded inputs -> full [50000, 256] f32 output.

    Shards edges by destination-node range across the 8 NeuronCores
    (each core computes its 6250-node output slice fully locally),
    compiles the Bass program, and runs it via run_bass_kernel_spmd.
    """
    x = np.ascontiguousarray(np.asarray(inputs["x"], np.float32))
    gate_w = np.asarray(inputs["gate_w"], np.float32)
    gate_b = np.asarray(inputs["gate_b"], np.float32)
    edge_index = np.asarray(inputs["edge_index"])
    return _kernel_impl(x, gate_w, gate_b, edge_index)


# revision 52
# speedup vs baseline: 1.0118x; 1.0004x over previous
"""AttentionalAggregation GNN kernel for 8 TRN2 NeuronCores.

Strategy: edges sorted by destination bucket on host; core m owns nodes
[m*NPC, (m+1)*NPC) and computes its output slice fully locally (no
collectives). Per 128-edge tile:
  - dma_gather x[src] rows (512B each) from lo/hi half tables (int16 idx)
  - score_e = sum_d V[e,d]*w[d]  (scalar_tensor_tensor accum)
  - e = exp(score + b)           (ACT)
  - P'[e,s] = (iota_s == slot_e) * e_e   (built on DVE, Pool or ACT,
    split to balance engine load; ACT path reads a host-built 0/1
    one-hot in fp8 and scales by e in one Copy op)
  - psum[bucket][:,0:128] += P'.T @ V_t   (plain f32 matmul: hardware
    f32r is tf32-grade and fails the accuracy budget)
  - psum[bucket][:,128]   += P'.T @ ones  (denominator)
Flush per group: reciprocal of denominator, scale, write agg as bf16 in
slot-major layout; host unpermutes, casts to f32 and concats with x.
"""

import math
import numpy as np

import concourse.bass as bass
import concourse.mybir as mybir
import concourse.tile as tile
from concourse import bacc

F32 = mybir.dt.float32
F32R = mybir.dt.float32r
BF16 = mybir.dt.bfloat16
FP8 = mybir.dt.float8e4
I16 = mybir.dt.int16
AF = mybir.ActivationFunctionType
OP = mybir.AluOpType

# engine assignment for building P' (one-hot * e) per tile
ENG_DVE = 0
ENG_POOL = 1
ENG_ACT = 2
# shares (DVE, POOL, ACT) of P'-build work; ACT tiles need the p01 stream.
# Pool-engine P' builds stall the in-order Pool queue behind cross-engine
# waits (gathers share it), so the Pool share stays 0.
PP_SHARES = (0.28, 0.0, 0.72)
# fraction of score STTs routed to the Pool engine. Keep 0: any non-SWDGE
# work on the in-order Pool engine delays later groups' descriptor
# generation and starves the DMA engines.
SC_POOL = 0.0


class Config:
    def __init__(self, N=50000, E=640000, D=128, NC=8, GROUP=2):
        assert D == 128
        self.N, self.E, self.D, self.NC = N, E, D, NC
        self.NPC = N // NC          # nodes per core
        assert self.NPC * NC == N
        # overlapping lo/hi gather tables (int16 index limit 32768 rows);
        # sources in the overlap may be assigned to either run, letting the
        # host pad the lo run to a tile boundary with real edges
        self.LO_MAX = min(32768, N)
        self.HI_BASE = max(0, N - 32768)
        self.NBUK = math.ceil(self.NPC / 128)   # buckets per core
        self.TAIL = self.NPC - (self.NBUK - 1) * 128  # rows in last bucket
        self.GROUP = GROUP
        self.NG = math.ceil(self.NBUK / GROUP)


def assign_engines(TOT):
    """Per-tile engine for the P' build, interleaved to the target shares."""
    eng = np.zeros(TOT, np.int8)
    acc = [0.0, 0.0, 0.0]
    for t in range(TOT):
        # pick the engine furthest below its target share
        deficits = [PP_SHARES[k] * (t + 1) - acc[k] for k in range(3)]
        k = int(np.argmax(deficits))
        eng[t] = k
        acc[k] += 1.0
    return eng


def build_schedule(cfg, src, dst):
    """Host-side: sort/pad edges into a static per-tile schedule uniform
    across cores. Returns (sched, per_core) where sched is the static
    structure (identical across cores) and per_core has the data arrays."""
    N, NC, NPC, NBUK, GROUP = (
        cfg.N, cfg.NC, cfg.NPC, cfg.NBUK, cfg.GROUP)
    LO_MAX, HI_BASE = cfg.LO_MAX, cfg.HI_BASE

    src = np.asarray(src, np.int64)
    dst = np.asarray(dst, np.int64)
    c = dst // NPC
    r = dst % NPC
    lb = r // 128
    slot = r % 128

    order = np.lexsort((src, lb, c))
    src_s, lb_s, slot_s, c_s = (
        src[order], lb[order], slot[order], c[order])

    key = c_s * NBUK + lb_s
    cnt = np.bincount(key, minlength=NC * NBUK).reshape(NC, NBUK)
    starts = np.zeros(NC * NBUK + 1, np.int64)
    np.cumsum(cnt.reshape(-1), out=starts[1:])

    # within each (core, bucket) slice (sorted by src), edges below
    # HI_BASE must use the lo table, edges >= LO_MAX must use hi, and the
    # overlap is flexible: cut the slice to fill lo tiles exactly
    n_lo_min = np.zeros((NC, NBUK), np.int64)
    n_lo_cap = np.zeros((NC, NBUK), np.int64)
    for cc in range(NC):
        for b in range(NBUK):
            k = cc * NBUK + b
            sl = src_s[starts[k]:starts[k + 1]]
            n_lo_min[cc, b] = np.searchsorted(sl, HI_BASE)
            n_lo_cap[cc, b] = np.searchsorted(sl, LO_MAX)
    # per-core needs, then sort each core's buckets by total tiles
    # descending so position-wise maxima across cores are tight
    t_lo_c = np.ceil(n_lo_min / 128.0).astype(np.int64)       # [NC, NBUK]
    lo_cap_pos = n_lo_cap
    t_hi_c = np.ceil(np.maximum(cnt - np.minimum(128 * t_lo_c, lo_cap_pos),
                                0) / 128.0).astype(np.int64)
    tot_c = t_lo_c + t_hi_c
    perm = np.argsort(-tot_c, axis=1, kind="stable")          # [NC, NBUK]
    ar = np.arange(NC)[:, None]
    T_lo = t_lo_c[ar, perm].max(axis=0)                       # [NBUK] by pos
    lo_count_pos = np.minimum(128 * T_lo[None, :], n_lo_cap[ar, perm])
    T_hi = np.ceil((cnt[ar, perm] - lo_count_pos) / 128.0
                   ).astype(np.int64).max(axis=0)
    Th = np.stack([T_lo, T_hi], axis=1)  # [NBUK, 2] by position
    # scatter position-based lo counts back to per-(core,bucket)
    lo_count = np.zeros_like(cnt)
    np.put_along_axis(lo_count, perm, lo_count_pos, axis=1)
    # move the smallest bucket to position 0: paired with a single-bucket
    # first group below, it halves the ramp-critical first gather batch
    pi = np.r_[NBUK - 1, NBUK - 2, np.arange(NBUK - 2)]
    perm = perm[:, pi]
    Th = Th[pi]
    Tb = Th.sum(axis=1)

    # static tile stream: per group g: [lo tiles of buckets][hi tiles]
    # each entry: (bucket_local_index_in_group j, bucket b, first, last)
    sizes = [1, 1]
    left = NBUK - 2
    while left > 0:
        take = min(GROUP, left)
        sizes.append(take)
        left -= take
    bounds = np.cumsum([0] + sizes)
    groups = []
    for g in range(len(sizes)):
        b0, b1 = int(bounds[g]), int(bounds[g + 1])
        tiles = []
        for h in (0, 1):
            for b in range(b0, b1):
                nt = int(Th[b, h])
                for t in range(nt):
                    pos = t if h == 0 else int(Th[b, 0]) + t
                    first = pos == 0
                    last = pos == int(Tb[b]) - 1
                    tiles.append(dict(j=b - b0, b=b, first=first, last=last))
        lo_tiles = int(Th[b0:b1, 0].sum())
        hi_tiles = int(Th[b0:b1, 1].sum())
        groups.append(dict(b0=b0, b1=b1, lo=lo_tiles, hi=hi_tiles,
                           tiles=tiles))
    TOT = sum(g["lo"] + g["hi"] for g in groups)

    # engine assignment per global tile; ACT tiles get p01 stream slots
    eng = assign_engines(TOT)
    act_idx = np.nonzero(eng == ENG_ACT)[0]
    act_pos = np.full(TOT, -1, np.int64)    # tile -> index among ACT tiles
    for i, t in enumerate(act_idx):
        act_pos[t] = i
    NACT = len(act_idx)

    # pass A: per-tile slot-span unions across cores (edges re-sorted by
    # slot within each (bucket, half) run so a tile covers a narrow
    # contiguous window; matmul partition offsets must be 32-aligned with
    # legal widths {32 @0/32/64/96, 64 @0/64, 128 @0})
    span_lo = np.full(TOT, 255, np.int64)
    span_hi = np.full(TOT, -1, np.int64)
    for core in range(NC):
        t_glob = 0
        for g in groups:
            for h in (0, 1):
                for pos in range(g["b0"], g["b1"]):
                    b = int(perm[core, pos])
                    k = core * NBUK + b
                    s0, s1 = starts[k], starts[k + 1]
                    cut = s0 + lo_count[core, b]
                    e_slot = (slot_s[s0:cut] if h == 0
                              else slot_s[cut:s1])
                    nt = int(Th[pos, h])
                    if nt > 0 and len(e_slot) > 0:
                        sl = np.sort(e_slot)
                        for kk in range(nt):
                            seg = sl[kk * 128:(kk + 1) * 128]
                            if len(seg):
                                t = t_glob + kk
                                span_lo[t] = min(span_lo[t], int(seg[0]))
                                span_hi[t] = max(span_hi[t], int(seg[-1]))
                    t_glob += nt

    def legal_window(a, b):
        # base partitions allowed by the AP layer: {0, 32, 64} only
        if b < 0:
            return 0, 32
        p = 32 * (a // 32)
        if p <= 64 and b - p < 32:
            return p, 32
        p = 64 * (a // 64)
        if b - p < 64:
            return p, 64
        return 0, 128

    win = [legal_window(int(span_lo[t]), int(span_hi[t]))
           for t in range(TOT)]
    # the bucket-opening tile (start=True) must cover all 128 partitions:
    # hardware clears the full bank row only for partitions the matmul
    # writes (validated: narrow first tiles corrupt untouched partitions).
    # Swap a naturally-full-window slice to the front of the bucket's
    # first run when one exists; otherwise widen the first tile.
    slice_perm = {}  # (pos, h) -> permutation of the run's slice indices
    t_glob = 0
    for g in groups:
        for h in (0, 1):
            for pos in range(g["b0"], g["b1"]):
                nt = int(Th[pos, h])
                if nt == 0:
                    continue
                is_first_run = (h == 0) or int(Th[pos, 0]) == 0
                if is_first_run:
                    p = list(range(nt))
                    kfull = next((k for k in range(nt)
                                  if win[t_glob + k] == (0, 128)), None)
                    if kfull is not None and kfull != 0:
                        p[0], p[kfull] = p[kfull], p[0]
                        slice_perm[(pos, h)] = p
                        w2 = [win[t_glob + k] for k in p]
                        for k in range(nt):
                            win[t_glob + k] = w2[k]
                    elif kfull is None:
                        win[t_glob] = (0, 128)
                t_glob += nt
    wp0 = np.array([w[0] for w in win], np.int64)
    ww = np.array([w[1] for w in win], np.int64)
    # variable-width p01 offsets for ACT tiles
    act_woff = np.zeros(NACT + 1, np.int64)
    np.cumsum(ww[act_idx], out=act_woff[1:])
    P01W = int(act_woff[-1])

    # per-core data arrays
    per_core = []
    for core in range(NC):
        idx_stream = np.zeros(TOT * 128, np.int16)
        slot_stream = np.full((128, TOT), 255.0, np.float32)
        t_glob = 0
        for g in groups:
            for h in (0, 1):
                for pos in range(g["b0"], g["b1"]):
                    b = int(perm[core, pos])
                    k = core * NBUK + b
                    s0, s1 = starts[k], starts[k + 1]
                    cut = s0 + lo_count[core, b]
                    if h == 0:
                        e_src = src_s[s0:cut]
                        e_slot = slot_s[s0:cut]
                    else:
                        e_src = src_s[cut:s1] - HI_BASE
                        e_slot = slot_s[cut:s1]
                    o2 = np.argsort(e_slot, kind="stable")
                    e_src = e_src[o2]
                    e_slot = e_slot[o2]
                    sp = slice_perm.get((pos, h))
                    if sp is not None and len(e_src) > 0:
                        segs = [e_src[k * 128:(k + 1) * 128] for k in sp]
                        segt = [e_slot[k * 128:(k + 1) * 128] for k in sp]
                        e_src = np.concatenate(segs)
                        e_slot = np.concatenate(segt)
                    n = len(e_src)
                    nt = int(Th[pos, h])
                    base = t_glob * 128
                    if n > 0:
                        idx_stream[base:base + n] = e_src.astype(np.int16)
                        fl = np.full(nt * 128, 255.0, np.float32)
                        fl[:n] = e_slot.astype(np.float32)
                        slot_stream[:, t_glob:t_glob + nt] = (
                            fl.reshape(nt, 128).T)
                    t_glob += nt
        assert t_glob == TOT
        # window-relative slots (255 padding stays out of range for any W)
        real = slot_stream < 255.0
        slot_stream = slot_stream - wp0[None, :].astype(np.float32) * real
        # wrap-16 the index stream, replicate across the 8 16-part groups
        wrapped = idx_stream.reshape(-1, 16).T  # [16, TOT*8]
        idx_arr = np.tile(wrapped, (8, 1)).copy()  # [128, TOT*8]
        # host-built 0/1 one-hot for ACT tiles, window-relative and
        # variable-width (width ww[t] per tile, concatenated)
        f8 = mybir.dt.np(FP8)
        p01 = np.zeros((128, max(P01W, 1)), f8)
        for i, t in enumerate(act_idx):
            W = int(ww[t])
            off = int(act_woff[i])
            oh = (slot_stream[:, t][:, None] ==
                  np.arange(W, dtype=np.float32)[None, :])
            p01[:, off:off + W] = oh.astype(f8)
        per_core.append(dict(idx=idx_arr, slots=slot_stream, p01=p01))

    sched = dict(groups=groups, TOT=TOT, Th=Th, Tb=Tb, perm=perm,
                 eng=eng, act_pos=act_pos, NACT=NACT,
                 wp0=wp0, ww=ww, act_woff=act_woff, P01W=P01W)
    return sched, per_core


def build_consts(cfg, gate_w, gate_b):
    """[128, 258] f32: iota | W_bcast | ones | b"""
    C = np.zeros((128, 258), np.float32)
    C[:, 0:128] = np.arange(128, dtype=np.float32)[None, :]
    C[:, 128:256] = np.asarray(gate_w, np.float32).reshape(1, 128)
    C[:, 256] = 1.0
    # gate bias is a uniform score shift and cancels in the softmax, so
    # the device never applies it; column 257 is kept but unused
    C[:, 257] = np.float32(np.asarray(gate_b).reshape(-1)[0])
    return C


def build_program(cfg, sched, do_main=True, do_compute=True):
    nc = bacc.Bacc("TRN2", num_devices=cfg.NC,
                   dynamic_dma_scratch_size=49152)
    NPC, NBUK, GROUP = cfg.NPC, cfg.NBUK, cfg.GROUP
    TOT = sched["TOT"]
    groups = sched["groups"]
    eng_map = sched["eng"]
    act_pos = sched["act_pos"]
    NACT = sched["NACT"]
    wp0 = sched["wp0"]
    ww = sched["ww"]
    act_woff = sched["act_woff"]

    xlo = nc.dram_tensor("xlo", [cfg.LO_MAX, 128], F32,
                         kind="ExternalInput")
    xhi = nc.dram_tensor("xhi", [cfg.N - cfg.HI_BASE, 128], F32,
                         kind="ExternalInput")
    idx = nc.dram_tensor("idx", [128, TOT * 8], I16, kind="ExternalInput")
    slt = nc.dram_tensor("slt", [128, TOT], F32, kind="ExternalInput")
    p01 = nc.dram_tensor("p01", [128, max(sched["P01W"], 1)], FP8,
                         kind="ExternalInput")
    cst = nc.dram_tensor("cst", [128, 258], F32, kind="ExternalInput")
    # agg output, slot-major: out[p, b*128 + d] = agg[b*128 + p, d], bf16
    out = nc.dram_tensor("out", [128, NBUK * 128], BF16,
                         kind="ExternalOutput")

    with tile.TileContext(nc) as tc:
        with (
            tc.tile_pool(name="const", bufs=1) as cpool,
            tc.tile_pool(name="meta", bufs=1) as mpool,
            tc.tile_pool(name="gather", bufs=3) as gpool,
            tc.tile_pool(name="sc", bufs=6) as scpool,
            tc.tile_pool(name="grp", bufs=3) as grpool,
            tc.tile_pool(name="pp", bufs=8) as ppool,
            tc.tile_pool(name="fl", bufs=4) as flpool,
            tc.tile_pool(name="ag", bufs=3) as agpool,
            tc.tile_pool(name="pnum", bufs=8, space="PSUM") as pnum,
        ):
            C = cpool.tile([128, 258], F32)
            iota_ap = C[:, 0:128]
            w_ap = C[:, 128:256]
            ones_ap = C[:, 256:257]
            b_ap = C[:, 257:258]

            # the idx stream is uploaded in chunks emitted just ahead of
            # the gathers that read them: a single monolithic upload held
            # the first gather batch (and the whole pipeline) back ~10us
            idx_sb = mpool.tile([128, TOT * 8], I16)
            slt_sb = mpool.tile([128, TOT], F32)

            # ---- main loop over groups (gathers prefetched one group
            # ahead so the in-order Pool queue never parks later groups'
            # gathers behind P'-builds that wait on cross-engine deps) ----
            # each dma_gather is capped at GMAX tiles: the SWDGE ring
            # holds scratch/16 descriptors and one instruction's
            # descriptor set must fit entirely
            GMAX = 16
            work = [g for g in (groups if do_main else [])
                    if g["lo"] + g["hi"] > 0]
            tg_of = {}
            acc = 0
            for g in (groups if do_main else []):
                tg_of[id(g)] = acc
                acc += g["lo"] + g["hi"]

            gbufs = {}

            # idx chunks cover CHUNK_GROUPS consecutive work groups (kept
            # >=512B per partition to dodge the small-transfer penalty)
            CHUNK_GROUPS = 2
            chunk_of = {}
            chunk_rng = []
            for wi, g in enumerate(work):
                ci = wi // CHUNK_GROUPS
                chunk_of[id(g)] = ci
                tg0 = tg_of[id(g)]
                TGg = g["lo"] + g["hi"]
                if ci == len(chunk_rng):
                    chunk_rng.append([tg0, tg0 + TGg])
                else:
                    chunk_rng[ci][1] = tg0 + TGg
            idx_chunk_done = set()

            def emit_idx_chunk(ci):
                if ci in idx_chunk_done:
                    return
                idx_chunk_done.add(ci)
                r0, r1 = chunk_rng[ci]
                nc.sync.dma_start(idx_sb[:, r0 * 8:r1 * 8],
                                  idx[:, r0 * 8:r1 * 8])

            def emit_gathers(g, first_small=False):
                emit_idx_chunk(chunk_of[id(g)])
                TG = g["lo"] + g["hi"]
                tg0 = tg_of[id(g)]
                # extra zeroed tile at the end lets the last tile's matmul
                # read a 256-wide rhs (junk cols accumulate into psum cols
                # 128:256, never read)
                gbuf = gpool.tile([128, TG, 128], F32, tag="gbuf")
                for half, n_t, base in ((0, g["lo"], 0),
                                        (1, g["hi"], g["lo"])):
                    tbl = xlo if half == 0 else xhi
                    # a small leading batch lets the first scores start
                    # as soon as 4 tiles land instead of a full batch
                    cuts = list(range(0, n_t, GMAX))
                    if first_small and half == 0 and n_t > 4:
                        cuts = [0, 4] + [c for c in cuts if c > 4]
                    for ci, q0 in enumerate(cuts):
                        q1 = cuts[ci + 1] if ci + 1 < len(cuts) else min(
                            q0 + GMAX, n_t)
                        b0t = base + q0
                        g0 = tg0 + b0t
                        nc.gpsimd.dma_gather(
                            out_ap=gbuf[:, b0t:base + q1, :],
                            in_ap=tbl[:],
                            idxs_ap=idx_sb[:, g0 * 8:(g0 + q1 - q0) * 8],
                            num_idxs=(q1 - q0) * 128,
                            num_idxs_reg=(q1 - q0) * 128,
                            elem_size=128,
                            single_packet=False,
                        )
                gbufs[id(g)] = gbuf

            # zero-fill output for empty groups up front
            for g in (groups if do_main else []):
                if g["lo"] + g["hi"] == 0:
                    nbk = g["b1"] - g["b0"]
                    aggg = agpool.tile([128, nbk * 128], BF16, tag="aggg")
                    nc.vector.memset(aggg[:], 0.0)
                    nc.sync.dma_start(
                        out[:, g["b0"] * 128:(g["b0"] + nbk) * 128], aggg[:])

            PREFETCH = 1  # groups of gathers in flight ahead (gbuf bufs-1)
            for wi, g in enumerate(work[:PREFETCH]):
                emit_gathers(g)
            # consts and slot stream issue after the pipeline-critical
            # first gathers (each DMA issue serializes ~0.6us on HWDGE)
            nc.sync.dma_start(C[:], cst[:])
            nc.sync.dma_start(slt_sb[:], slt[:])
            for wi, g in enumerate(work):
                if wi + PREFETCH < len(work):
                    emit_gathers(work[wi + PREFETCH])
                TG = g["lo"] + g["hi"]
                nbk = g["b1"] - g["b0"]
                t_glob = tg_of[id(g)]
                gbuf = gbufs.pop(id(g))
                aggg = agpool.tile([128, nbk * 128], BF16, tag="aggg")

                # p01 stream for this group's ACT-assigned tiles
                # (variable widths, contiguous in the global ACT ordering)
                acts = [t for t in range(TG) if eng_map[t_glob + t] == ENG_ACT]
                nact = len(acts)
                if nact > 0:
                    ai0 = int(act_pos[t_glob + acts[0]])
                    w0 = int(act_woff[ai0])
                    w1 = int(act_woff[ai0 + nact])
                    p01_sb = grpool.tile([128, w1 - w0], FP8, tag="p01sb")
                    nc.sync.dma_start(p01_sb[:], p01[:, w0:w1])

                if not do_compute:
                    # still consume gbuf so Tile keeps the gathers
                    sink = flpool.tile([128, 1], F32, tag="sink")
                    nc.vector.tensor_scalar(
                        out=sink[:], in0=gbuf[:, 0, 0:1], scalar1=1.0,
                        scalar2=None, op0=OP.mult)
                    nc.sync.dma_start(out[0:128, 0:1], sink[:].bitcast(BF16)[:, 0:1])
                    continue
                # scores for all tiles of the group
                sg = grpool.tile([128, TG], F32, tag="sg")
                eg = grpool.tile([128, TG], F32, tag="eg")
                for t in range(TG):
                    scr = scpool.tile([128, 128], F32, tag="scr",
                                      name="scr_v")
                    sc_eng = (nc.gpsimd if (t_glob + t) % 100 <
                              SC_POOL * 100 else nc.vector)
                    sc_eng.scalar_tensor_tensor(
                        out=scr[:], in0=gbuf[:, t, :], scalar=1.0,
                        in1=w_ap, op0=OP.mult, op1=OP.mult,
                        accum_out=sg[:, t:t + 1])
                    # per-16-tile exp keeps the pipeline fine-grained
                    if t % 16 == 15 or t == TG - 1:
                        lo8 = (t // 16) * 16
                        nc.scalar.activation(out=eg[:, lo8:t + 1],
                                             in_=sg[:, lo8:t + 1],
                                             func=AF.Exp,
                                             bias=0.0, scale=1.0)

                # per-bucket psum tiles: cols 0:128 numerator, col 128 denom
                psums = {}
                for j in range(nbk):
                    if sched["Tb"][g["b0"] + j] > 0:
                        psums[j] = pnum.tile([128, 129], F32, tag="pn",
                                             name=f"pn_{g['b0']}_{j}")

                for t, ti in enumerate(g["tiles"]):
                    j = ti["j"]
                    tg = t_glob + t
                    p0 = int(wp0[tg])
                    W = int(ww[tg])
                    Pp = ppool.tile([128, W], F32, tag=f"pp{W}")
                    ek = eng_map[tg]
                    if ek == ENG_ACT:
                        # ACT path: P' = host-built 0/1 one-hot (fp8,
                        # exact, window-relative) scaled by e in one Copy
                        ai = int(act_pos[tg])
                        o0 = int(act_woff[ai]) - int(
                            act_woff[int(act_pos[t_glob + acts[0]])])
                        nc.scalar.activation(
                            out=Pp[:], in_=p01_sb[:, o0:o0 + W],
                            func=AF.Copy, scale=eg[:, t:t + 1])
                    else:
                        nc.vector.tensor_scalar(
                            out=Pp[:], in0=iota_ap[:, 0:W],
                            scalar1=slt_sb[:, tg:tg + 1],
                            scalar2=eg[:, t:t + 1],
                            op0=OP.is_equal, op1=OP.mult)
                    # slot-sorted tiles cover a narrow aligned window
                    # [p0, p0+W) of the bucket's 128 slots; the matmul
                    # writes just those psum partitions. The bucket's
                    # first tile is forced to the full window so its
                    # start=True clears every partition of the bank.
                    nc.tensor.matmul(
                        out=psums[j][p0:p0 + W, 0:128], lhsT=Pp[:],
                        rhs=gbuf[:, t, :],
                        start=ti["first"], stop=False)
                    nc.tensor.matmul(
                        out=psums[j][p0:p0 + W, 128:129], lhsT=Pp[:],
                        rhs=ones_ap,
                        start=False, stop=ti["last"])

                # flush group: per-bucket reciprocal + scale into the
                # group's bf16 slot-major agg tile, then one DMA out
                for j in range(nbk):
                    if j in psums:
                        # no epsilon guard: slots with zero in-degree give
                        # den=0 -> inf/NaN rows, which the host overwrites
                        # with zeros (it knows the in-degrees from
                        # edge_index); skipping the max() op saves ~6us DVE
                        rcp = flpool.tile([128, 1], F32, tag="rcp")
                        nc.vector.reciprocal(out=rcp[:],
                                             in_=psums[j][:, 128:129])
                        nc.scalar.activation(
                            out=aggg[:, j * 128:(j + 1) * 128],
                            in_=psums[j][:, 0:128],
                            func=AF.Copy, scale=rcp[:, 0:1])
                    else:
                        nc.vector.memset(aggg[:, j * 128:(j + 1) * 128], 0.0)
                nc.sync.dma_start(
                    out[:, g["b0"] * 128:(g["b0"] + nbk) * 128], aggg[:])

    nc.compile()
    return nc


def make_in_maps(cfg, sched, per_core, x, gate_w, gate_b):
    x = np.asarray(x, np.float32)
    consts = build_consts(cfg, gate_w, gate_b)
    in_maps = []
    for core in range(cfg.NC):
        in_maps.append({
            "xlo": x[:cfg.LO_MAX],
            "xhi": x[cfg.HI_BASE:],
            "idx": per_core[core]["idx"],
            "slt": per_core[core]["slots"],
            "p01": per_core[core]["p01"],
            "cst": consts,
        })
    return in_maps


def _kernel_impl(x, gate_w, gate_b, edge_index, cfg=None, return_nc=False):
    from concourse.bass_utils import run_bass_kernel_spmd
    if cfg is None:
        cfg = Config()
    sched, per_core = build_schedule(cfg, edge_index[0], edge_index[1])
    nc = build_program(cfg, sched)
    in_maps = make_in_maps(cfg, sched, per_core, x, gate_w, gate_b)
    res = run_bass_kernel_spmd(nc, in_maps, core_ids=list(range(cfg.NC)))
    perm = sched["perm"]
    outp = np.zeros((cfg.N, 256), np.float32)
    outp[:, 0:128] = x
    indeg = np.bincount(np.asarray(edge_index[1], np.int64),
                        minlength=cfg.N)
    for core in range(cfg.NC):
        # o: [128, NBUK*128] bf16 slot-major -> [NBUK, 128, 128] agg
        o = np.asarray(res.results[core]["out"], dtype=np.float32)
        o = o.reshape(128, cfg.NBUK, 128).transpose(1, 0, 2)
        base = core * cfg.NPC
        for k in range(cfg.NBUK):
            b = int(perm[core, k])
            v = min(128, cfg.NPC - b * 128)
            outp[base + b * 128:base + b * 128 + v, 128:256] = o[k, :v]
    outp[indeg == 0, 128:256] = 0.0
    if return_nc:
        return outp, nc
    return outp


def kernel(**inputs):
    """Harness entry: full unsharded inputs -> full [50000, 256] f32 output.

    Shards edges by destination-node range across the 8 NeuronCores
    (each core computes its 6250-node output slice fully locally),
    compiles the Bass program, and runs it via run_bass_kernel_spmd.
    """
    x = np.ascontiguousarray(np.asarray(inputs["x"], np.float32))
    gate_w = np.asarray(inputs["gate_w"], np.float32)
    gate_b = np.asarray(inputs["gate_b"], np.float32)
    edge_index = np.asarray(inputs["edge_index"])
    return _kernel_impl(x, gate_w, gate_b, edge_index)


# revision 53
# speedup vs baseline: 1.0187x; 1.0067x over previous
"""AttentionalAggregation GNN kernel for 8 TRN2 NeuronCores.

Strategy: edges sorted by destination bucket on host; core m owns nodes
[m*NPC, (m+1)*NPC) and computes its output slice fully locally (no
collectives). Per 128-edge tile:
  - dma_gather x[src] rows (512B each) from lo/hi half tables (int16 idx)
  - score_e = sum_d V[e,d]*w[d]  (scalar_tensor_tensor accum)
  - e = exp(score + b)           (ACT)
  - P'[e,s] = (iota_s == slot_e) * e_e   (built on DVE, Pool or ACT,
    split to balance engine load; ACT path reads a host-built 0/1
    one-hot in fp8 and scales by e in one Copy op)
  - psum[bucket][:,0:128] += P'.T @ V_t   (plain f32 matmul: hardware
    f32r is tf32-grade and fails the accuracy budget)
  - psum[bucket][:,128]   += P'.T @ ones  (denominator)
Flush per group: reciprocal of denominator, scale, write agg as bf16 in
slot-major layout; host unpermutes, casts to f32 and concats with x.
"""

import math
import numpy as np

import concourse.bass as bass
import concourse.mybir as mybir
import concourse.tile as tile
from concourse import bacc

F32 = mybir.dt.float32
F32R = mybir.dt.float32r
BF16 = mybir.dt.bfloat16
FP8 = mybir.dt.float8e4
I16 = mybir.dt.int16
AF = mybir.ActivationFunctionType
OP = mybir.AluOpType

# engine assignment for building P' (one-hot * e) per tile
ENG_DVE = 0
ENG_POOL = 1
ENG_ACT = 2
# shares (DVE, POOL, ACT) of P'-build work; ACT tiles need the p01 stream.
# Pool-engine P' builds stall the in-order Pool queue behind cross-engine
# waits (gathers share it), so the Pool share stays 0.
PP_SHARES = (0.28, 0.0, 0.72)
# fraction of score STTs routed to the Pool engine. Keep 0: any non-SWDGE
# work on the in-order Pool engine delays later groups' descriptor
# generation and starves the DMA engines.
SC_POOL = 0.0


class Config:
    def __init__(self, N=50000, E=640000, D=128, NC=8, GROUP=2):
        assert D == 128
        self.N, self.E, self.D, self.NC = N, E, D, NC
        self.NPC = N // NC          # nodes per core
        assert self.NPC * NC == N
        # overlapping lo/hi gather tables (int16 index limit 32768 rows);
        # sources in the overlap may be assigned to either run, letting the
        # host pad the lo run to a tile boundary with real edges
        self.LO_MAX = min(32768, N)
        self.HI_BASE = max(0, N - 32768)
        self.NBUK = math.ceil(self.NPC / 128)   # buckets per core
        self.TAIL = self.NPC - (self.NBUK - 1) * 128  # rows in last bucket
        self.GROUP = GROUP
        self.NG = math.ceil(self.NBUK / GROUP)


def assign_engines(TOT):
    """Per-tile engine for the P' build, interleaved to the target shares."""
    eng = np.zeros(TOT, np.int8)
    acc = [0.0, 0.0, 0.0]
    for t in range(TOT):
        # pick the engine furthest below its target share
        deficits = [PP_SHARES[k] * (t + 1) - acc[k] for k in range(3)]
        k = int(np.argmax(deficits))
        eng[t] = k
        acc[k] += 1.0
    return eng


def build_schedule(cfg, src, dst):
    """Host-side: sort/pad edges into a static per-tile schedule uniform
    across cores. Returns (sched, per_core) where sched is the static
    structure (identical across cores) and per_core has the data arrays."""
    N, NC, NPC, NBUK, GROUP = (
        cfg.N, cfg.NC, cfg.NPC, cfg.NBUK, cfg.GROUP)
    LO_MAX, HI_BASE = cfg.LO_MAX, cfg.HI_BASE

    src = np.asarray(src, np.int64)
    dst = np.asarray(dst, np.int64)
    c = dst // NPC
    r = dst % NPC
    lb = r // 128
    slot = r % 128

    order = np.lexsort((src, lb, c))
    src_s, lb_s, slot_s, c_s = (
        src[order], lb[order], slot[order], c[order])

    key = c_s * NBUK + lb_s
    cnt = np.bincount(key, minlength=NC * NBUK).reshape(NC, NBUK)
    starts = np.zeros(NC * NBUK + 1, np.int64)
    np.cumsum(cnt.reshape(-1), out=starts[1:])

    # within each (core, bucket) slice (sorted by src), edges below
    # HI_BASE must use the lo table, edges >= LO_MAX must use hi, and the
    # overlap is flexible: cut the slice to fill lo tiles exactly
    n_lo_min = np.zeros((NC, NBUK), np.int64)
    n_lo_cap = np.zeros((NC, NBUK), np.int64)
    for cc in range(NC):
        for b in range(NBUK):
            k = cc * NBUK + b
            sl = src_s[starts[k]:starts[k + 1]]
            n_lo_min[cc, b] = np.searchsorted(sl, HI_BASE)
            n_lo_cap[cc, b] = np.searchsorted(sl, LO_MAX)
    # per-core needs, then sort each core's buckets by total tiles
    # descending so position-wise maxima across cores are tight
    t_lo_c = np.ceil(n_lo_min / 128.0).astype(np.int64)       # [NC, NBUK]
    lo_cap_pos = n_lo_cap
    t_hi_c = np.ceil(np.maximum(cnt - np.minimum(128 * t_lo_c, lo_cap_pos),
                                0) / 128.0).astype(np.int64)
    tot_c = t_lo_c + t_hi_c
    perm = np.argsort(-tot_c, axis=1, kind="stable")          # [NC, NBUK]
    ar = np.arange(NC)[:, None]
    T_lo = t_lo_c[ar, perm].max(axis=0)                       # [NBUK] by pos
    lo_count_pos = np.minimum(128 * T_lo[None, :], n_lo_cap[ar, perm])
    T_hi = np.ceil((cnt[ar, perm] - lo_count_pos) / 128.0
                   ).astype(np.int64).max(axis=0)
    Th = np.stack([T_lo, T_hi], axis=1)  # [NBUK, 2] by position
    # scatter position-based lo counts back to per-(core,bucket)
    lo_count = np.zeros_like(cnt)
    np.put_along_axis(lo_count, perm, lo_count_pos, axis=1)
    # move the smallest bucket to position 0: paired with a single-bucket
    # first group below, it halves the ramp-critical first gather batch
    pi = np.r_[NBUK - 1, NBUK - 2, np.arange(NBUK - 2)]
    perm = perm[:, pi]
    Th = Th[pi]
    Tb = Th.sum(axis=1)

    # static tile stream: per group g: [lo tiles of buckets][hi tiles]
    # each entry: (bucket_local_index_in_group j, bucket b, first, last)
    sizes = [1, 1]
    left = NBUK - 2
    while left > 0:
        take = min(GROUP, left)
        sizes.append(take)
        left -= take
    bounds = np.cumsum([0] + sizes)
    groups = []
    for g in range(len(sizes)):
        b0, b1 = int(bounds[g]), int(bounds[g + 1])
        tiles = []
        for h in (0, 1):
            for b in range(b0, b1):
                nt = int(Th[b, h])
                for t in range(nt):
                    pos = t if h == 0 else int(Th[b, 0]) + t
                    first = pos == 0
                    last = pos == int(Tb[b]) - 1
                    tiles.append(dict(j=b - b0, b=b, first=first, last=last))
        lo_tiles = int(Th[b0:b1, 0].sum())
        hi_tiles = int(Th[b0:b1, 1].sum())
        groups.append(dict(b0=b0, b1=b1, lo=lo_tiles, hi=hi_tiles,
                           tiles=tiles))
    TOT = sum(g["lo"] + g["hi"] for g in groups)

    # engine assignment per global tile; ACT tiles get p01 stream slots
    eng = assign_engines(TOT)
    act_idx = np.nonzero(eng == ENG_ACT)[0]
    act_pos = np.full(TOT, -1, np.int64)    # tile -> index among ACT tiles
    for i, t in enumerate(act_idx):
        act_pos[t] = i
    NACT = len(act_idx)

    # pass A: per-tile slot-span unions across cores (edges re-sorted by
    # slot within each (bucket, half) run so a tile covers a narrow
    # contiguous window; matmul partition offsets must be 32-aligned with
    # legal widths {32 @0/32/64/96, 64 @0/64, 128 @0})
    span_lo = np.full(TOT, 255, np.int64)
    span_hi = np.full(TOT, -1, np.int64)
    for core in range(NC):
        t_glob = 0
        for g in groups:
            for h in (0, 1):
                for pos in range(g["b0"], g["b1"]):
                    b = int(perm[core, pos])
                    k = core * NBUK + b
                    s0, s1 = starts[k], starts[k + 1]
                    cut = s0 + lo_count[core, b]
                    e_slot = (slot_s[s0:cut] if h == 0
                              else slot_s[cut:s1])
                    nt = int(Th[pos, h])
                    if nt > 0 and len(e_slot) > 0:
                        sl = np.sort(e_slot)
                        for kk in range(nt):
                            seg = sl[kk * 128:(kk + 1) * 128]
                            if len(seg):
                                t = t_glob + kk
                                span_lo[t] = min(span_lo[t], int(seg[0]))
                                span_hi[t] = max(span_hi[t], int(seg[-1]))
                    t_glob += nt

    def legal_window(a, b):
        # base partitions allowed by the AP layer: {0, 32, 64} only
        if b < 0:
            return 0, 32
        p = 32 * (a // 32)
        if p <= 64 and b - p < 32:
            return p, 32
        p = 64 * (a // 64)
        if b - p < 64:
            return p, 64
        return 0, 128

    win = [legal_window(int(span_lo[t]), int(span_hi[t]))
           for t in range(TOT)]
    # the bucket-opening tile (start=True) must cover all 128 partitions:
    # hardware clears the full bank row only for partitions the matmul
    # writes (validated: narrow first tiles corrupt untouched partitions).
    # Swap a naturally-full-window slice to the front of the bucket's
    # first run when one exists; otherwise widen the first tile.
    slice_perm = {}  # (pos, h) -> permutation of the run's slice indices
    t_glob = 0
    for g in groups:
        for h in (0, 1):
            for pos in range(g["b0"], g["b1"]):
                nt = int(Th[pos, h])
                if nt == 0:
                    continue
                is_first_run = (h == 0) or int(Th[pos, 0]) == 0
                if is_first_run:
                    p = list(range(nt))
                    kfull = next((k for k in range(nt)
                                  if win[t_glob + k] == (0, 128)), None)
                    if kfull is not None and kfull != 0:
                        p[0], p[kfull] = p[kfull], p[0]
                        slice_perm[(pos, h)] = p
                        w2 = [win[t_glob + k] for k in p]
                        for k in range(nt):
                            win[t_glob + k] = w2[k]
                    elif kfull is None:
                        win[t_glob] = (0, 128)
                t_glob += nt
    wp0 = np.array([w[0] for w in win], np.int64)
    ww = np.array([w[1] for w in win], np.int64)
    # variable-width p01 offsets for ACT tiles
    act_woff = np.zeros(NACT + 1, np.int64)
    np.cumsum(ww[act_idx], out=act_woff[1:])
    P01W = int(act_woff[-1])

    # per-core data arrays
    per_core = []
    for core in range(NC):
        idx_stream = np.zeros(TOT * 128, np.int16)
        slot_stream = np.full((128, TOT), 255.0, np.float32)
        t_glob = 0
        for g in groups:
            for h in (0, 1):
                for pos in range(g["b0"], g["b1"]):
                    b = int(perm[core, pos])
                    k = core * NBUK + b
                    s0, s1 = starts[k], starts[k + 1]
                    cut = s0 + lo_count[core, b]
                    if h == 0:
                        e_src = src_s[s0:cut]
                        e_slot = slot_s[s0:cut]
                    else:
                        e_src = src_s[cut:s1] - HI_BASE
                        e_slot = slot_s[cut:s1]
                    o2 = np.argsort(e_slot, kind="stable")
                    e_src = e_src[o2]
                    e_slot = e_slot[o2]
                    sp = slice_perm.get((pos, h))
                    if sp is not None and len(e_src) > 0:
                        segs = [e_src[k * 128:(k + 1) * 128] for k in sp]
                        segt = [e_slot[k * 128:(k + 1) * 128] for k in sp]
                        e_src = np.concatenate(segs)
                        e_slot = np.concatenate(segt)
                    n = len(e_src)
                    nt = int(Th[pos, h])
                    base = t_glob * 128
                    if n > 0:
                        idx_stream[base:base + n] = e_src.astype(np.int16)
                        fl = np.full(nt * 128, 255.0, np.float32)
                        fl[:n] = e_slot.astype(np.float32)
                        slot_stream[:, t_glob:t_glob + nt] = (
                            fl.reshape(nt, 128).T)
                    t_glob += nt
        assert t_glob == TOT
        # window-relative slots (255 padding stays out of range for any W)
        real = slot_stream < 255.0
        slot_stream = slot_stream - wp0[None, :].astype(np.float32) * real
        # wrap-16 the index stream, replicate across the 8 16-part groups
        wrapped = idx_stream.reshape(-1, 16).T  # [16, TOT*8]
        idx_arr = np.tile(wrapped, (8, 1)).copy()  # [128, TOT*8]
        # host-built 0/1 one-hot for ACT tiles, window-relative and
        # variable-width (width ww[t] per tile, concatenated)
        f8 = mybir.dt.np(FP8)
        p01 = np.zeros((128, max(P01W, 1)), f8)
        for i, t in enumerate(act_idx):
            W = int(ww[t])
            off = int(act_woff[i])
            oh = (slot_stream[:, t][:, None] ==
                  np.arange(W, dtype=np.float32)[None, :])
            p01[:, off:off + W] = oh.astype(f8)
        per_core.append(dict(idx=idx_arr, slots=slot_stream, p01=p01))

    sched = dict(groups=groups, TOT=TOT, Th=Th, Tb=Tb, perm=perm,
                 eng=eng, act_pos=act_pos, NACT=NACT,
                 wp0=wp0, ww=ww, act_woff=act_woff, P01W=P01W)
    return sched, per_core


def build_consts(cfg, gate_w, gate_b):
    """[128, 258] f32: iota | W_bcast | ones | b"""
    C = np.zeros((128, 258), np.float32)
    C[:, 0:128] = np.arange(128, dtype=np.float32)[None, :]
    C[:, 128:256] = np.asarray(gate_w, np.float32).reshape(1, 128)
    C[:, 256] = 1.0
    # gate bias is a uniform score shift and cancels in the softmax, so
    # the device never applies it; column 257 is kept but unused
    C[:, 257] = np.float32(np.asarray(gate_b).reshape(-1)[0])
    return C


def build_program(cfg, sched, do_main=True, do_compute=True):
    nc = bacc.Bacc("TRN2", num_devices=cfg.NC,
                   dynamic_dma_scratch_size=49152)
    NPC, NBUK, GROUP = cfg.NPC, cfg.NBUK, cfg.GROUP
    TOT = sched["TOT"]
    groups = sched["groups"]
    eng_map = sched["eng"]
    act_pos = sched["act_pos"]
    NACT = sched["NACT"]
    wp0 = sched["wp0"]
    ww = sched["ww"]
    act_woff = sched["act_woff"]

    xlo = nc.dram_tensor("xlo", [cfg.LO_MAX, 128], F32,
                         kind="ExternalInput")
    xhi = nc.dram_tensor("xhi", [cfg.N - cfg.HI_BASE, 128], F32,
                         kind="ExternalInput")
    idx = nc.dram_tensor("idx", [128, TOT * 8], I16, kind="ExternalInput")
    slt = nc.dram_tensor("slt", [128, TOT], F32, kind="ExternalInput")
    p01 = nc.dram_tensor("p01", [128, max(sched["P01W"], 1)], FP8,
                         kind="ExternalInput")
    cst = nc.dram_tensor("cst", [128, 258], F32, kind="ExternalInput")
    # agg output, slot-major: out[p, b*128 + d] = agg[b*128 + p, d], bf16
    out = nc.dram_tensor("out", [128, NBUK * 128], BF16,
                         kind="ExternalOutput")

    with tile.TileContext(nc) as tc:
        with (
            tc.tile_pool(name="const", bufs=1) as cpool,
            tc.tile_pool(name="meta", bufs=1) as mpool,
            tc.tile_pool(name="gather", bufs=3) as gpool,
            tc.tile_pool(name="sc", bufs=6) as scpool,
            tc.tile_pool(name="grp", bufs=3) as grpool,
            tc.tile_pool(name="pp", bufs=8) as ppool,
            tc.tile_pool(name="fl", bufs=4) as flpool,
            tc.tile_pool(name="ag", bufs=3) as agpool,
            tc.tile_pool(name="pnum", bufs=8, space="PSUM") as pnum,
        ):
            C = cpool.tile([128, 258], F32)
            iota_ap = C[:, 0:128]
            w_ap = C[:, 128:256]
            ones_ap = C[:, 256:257]
            b_ap = C[:, 257:258]

            # the idx stream is uploaded in chunks emitted just ahead of
            # the gathers that read them: a single monolithic upload held
            # the first gather batch (and the whole pipeline) back ~10us
            idx_sb = mpool.tile([128, TOT * 8], I16)
            slt_sb = mpool.tile([128, TOT], F32)

            # ---- main loop over groups (gathers prefetched one group
            # ahead so the in-order Pool queue never parks later groups'
            # gathers behind P'-builds that wait on cross-engine deps) ----
            # each dma_gather is capped at GMAX tiles: the SWDGE ring
            # holds scratch/16 descriptors and one instruction's
            # descriptor set must fit entirely
            GMAX = 8
            work = [g for g in (groups if do_main else [])
                    if g["lo"] + g["hi"] > 0]
            tg_of = {}
            acc = 0
            for g in (groups if do_main else []):
                tg_of[id(g)] = acc
                acc += g["lo"] + g["hi"]

            gbufs = {}

            # idx chunks cover CHUNK_GROUPS consecutive work groups (kept
            # >=512B per partition to dodge the small-transfer penalty)
            CHUNK_GROUPS = 2
            chunk_of = {}
            chunk_rng = []
            for wi, g in enumerate(work):
                ci = wi // CHUNK_GROUPS
                chunk_of[id(g)] = ci
                tg0 = tg_of[id(g)]
                TGg = g["lo"] + g["hi"]
                if ci == len(chunk_rng):
                    chunk_rng.append([tg0, tg0 + TGg])
                else:
                    chunk_rng[ci][1] = tg0 + TGg
            idx_chunk_done = set()

            def emit_idx_chunk(ci):
                if ci in idx_chunk_done:
                    return
                idx_chunk_done.add(ci)
                r0, r1 = chunk_rng[ci]
                nc.sync.dma_start(idx_sb[:, r0 * 8:r1 * 8],
                                  idx[:, r0 * 8:r1 * 8])

            def emit_gathers(g, first_small=False):
                emit_idx_chunk(chunk_of[id(g)])
                TG = g["lo"] + g["hi"]
                tg0 = tg_of[id(g)]
                # extra zeroed tile at the end lets the last tile's matmul
                # read a 256-wide rhs (junk cols accumulate into psum cols
                # 128:256, never read)
                gbuf = gpool.tile([128, TG, 128], F32, tag="gbuf")
                for half, n_t, base in ((0, g["lo"], 0),
                                        (1, g["hi"], g["lo"])):
                    tbl = xlo if half == 0 else xhi
                    # a small leading batch lets the first scores start
                    # as soon as 4 tiles land instead of a full batch
                    cuts = list(range(0, n_t, GMAX))
                    if first_small and half == 0 and n_t > 4:
                        cuts = [0, 4] + [c for c in cuts if c > 4]
                    for ci, q0 in enumerate(cuts):
                        q1 = cuts[ci + 1] if ci + 1 < len(cuts) else min(
                            q0 + GMAX, n_t)
                        b0t = base + q0
                        g0 = tg0 + b0t
                        nc.gpsimd.dma_gather(
                            out_ap=gbuf[:, b0t:base + q1, :],
                            in_ap=tbl[:],
                            idxs_ap=idx_sb[:, g0 * 8:(g0 + q1 - q0) * 8],
                            num_idxs=(q1 - q0) * 128,
                            num_idxs_reg=(q1 - q0) * 128,
                            elem_size=128,
                            single_packet=False,
                        )
                gbufs[id(g)] = gbuf

            # zero-fill output for empty groups up front
            for g in (groups if do_main else []):
                if g["lo"] + g["hi"] == 0:
                    nbk = g["b1"] - g["b0"]
                    aggg = agpool.tile([128, nbk * 128], BF16, tag="aggg")
                    nc.vector.memset(aggg[:], 0.0)
                    nc.sync.dma_start(
                        out[:, g["b0"] * 128:(g["b0"] + nbk) * 128], aggg[:])

            PREFETCH = 1  # groups of gathers in flight ahead (gbuf bufs-1)
            for wi, g in enumerate(work[:PREFETCH]):
                emit_gathers(g)
            # consts and slot stream issue after the pipeline-critical
            # first gathers (each DMA issue serializes ~0.6us on HWDGE)
            nc.sync.dma_start(C[:], cst[:])
            nc.sync.dma_start(slt_sb[:], slt[:])
            for wi, g in enumerate(work):
                if wi + PREFETCH < len(work):
                    emit_gathers(work[wi + PREFETCH])
                TG = g["lo"] + g["hi"]
                nbk = g["b1"] - g["b0"]
                t_glob = tg_of[id(g)]
                gbuf = gbufs.pop(id(g))
                aggg = agpool.tile([128, nbk * 128], BF16, tag="aggg")

                # p01 stream for this group's ACT-assigned tiles
                # (variable widths, contiguous in the global ACT ordering)
                acts = [t for t in range(TG) if eng_map[t_glob + t] == ENG_ACT]
                nact = len(acts)
                if nact > 0:
                    ai0 = int(act_pos[t_glob + acts[0]])
                    w0 = int(act_woff[ai0])
                    w1 = int(act_woff[ai0 + nact])
                    p01_sb = grpool.tile([128, w1 - w0], FP8, tag="p01sb")
                    nc.sync.dma_start(p01_sb[:], p01[:, w0:w1])

                if not do_compute:
                    # still consume gbuf so Tile keeps the gathers
                    sink = flpool.tile([128, 1], F32, tag="sink")
                    nc.vector.tensor_scalar(
                        out=sink[:], in0=gbuf[:, 0, 0:1], scalar1=1.0,
                        scalar2=None, op0=OP.mult)
                    nc.sync.dma_start(out[0:128, 0:1], sink[:].bitcast(BF16)[:, 0:1])
                    continue
                # scores for all tiles of the group
                sg = grpool.tile([128, TG], F32, tag="sg")
                eg = grpool.tile([128, TG], F32, tag="eg")
                for t in range(TG):
                    scr = scpool.tile([128, 128], F32, tag="scr",
                                      name="scr_v")
                    sc_eng = (nc.gpsimd if (t_glob + t) % 100 <
                              SC_POOL * 100 else nc.vector)
                    sc_eng.scalar_tensor_tensor(
                        out=scr[:], in0=gbuf[:, t, :], scalar=1.0,
                        in1=w_ap, op0=OP.mult, op1=OP.mult,
                        accum_out=sg[:, t:t + 1])
                    # per-16-tile exp keeps the pipeline fine-grained
                    if t % 16 == 15 or t == TG - 1:
                        lo8 = (t // 16) * 16
                        nc.scalar.activation(out=eg[:, lo8:t + 1],
                                             in_=sg[:, lo8:t + 1],
                                             func=AF.Exp,
                                             bias=0.0, scale=1.0)

                # per-bucket psum tiles: cols 0:128 numerator, col 128 denom
                psums = {}
                for j in range(nbk):
                    if sched["Tb"][g["b0"] + j] > 0:
                        psums[j] = pnum.tile([128, 129], F32, tag="pn",
                                             name=f"pn_{g['b0']}_{j}")

                for t, ti in enumerate(g["tiles"]):
                    j = ti["j"]
                    tg = t_glob + t
                    p0 = int(wp0[tg])
                    W = int(ww[tg])
                    Pp = ppool.tile([128, W], F32, tag=f"pp{W}")
                    ek = eng_map[tg]
                    if ek == ENG_ACT:
                        # ACT path: P' = host-built 0/1 one-hot (fp8,
                        # exact, window-relative) scaled by e in one Copy
                        ai = int(act_pos[tg])
                        o0 = int(act_woff[ai]) - int(
                            act_woff[int(act_pos[t_glob + acts[0]])])
                        nc.scalar.activation(
                            out=Pp[:], in_=p01_sb[:, o0:o0 + W],
                            func=AF.Copy, scale=eg[:, t:t + 1])
                    else:
                        nc.vector.tensor_scalar(
                            out=Pp[:], in0=iota_ap[:, 0:W],
                            scalar1=slt_sb[:, tg:tg + 1],
                            scalar2=eg[:, t:t + 1],
                            op0=OP.is_equal, op1=OP.mult)
                    # slot-sorted tiles cover a narrow aligned window
                    # [p0, p0+W) of the bucket's 128 slots; the matmul
                    # writes just those psum partitions. The bucket's
                    # first tile is forced to the full window so its
                    # start=True clears every partition of the bank.
                    nc.tensor.matmul(
                        out=psums[j][p0:p0 + W, 0:128], lhsT=Pp[:],
                        rhs=gbuf[:, t, :],
                        start=ti["first"], stop=False)
                    nc.tensor.matmul(
                        out=psums[j][p0:p0 + W, 128:129], lhsT=Pp[:],
                        rhs=ones_ap,
                        start=False, stop=ti["last"])

                # flush group: per-bucket reciprocal + scale into the
                # group's bf16 slot-major agg tile, then one DMA out
                for j in range(nbk):
                    if j in psums:
                        # no epsilon guard: slots with zero in-degree give
                        # den=0 -> inf/NaN rows, which the host overwrites
                        # with zeros (it knows the in-degrees from
                        # edge_index); skipping the max() op saves ~6us DVE
                        rcp = flpool.tile([128, 1], F32, tag="rcp")
                        nc.vector.reciprocal(out=rcp[:],
                                             in_=psums[j][:, 128:129])
                        nc.scalar.activation(
                            out=aggg[:, j * 128:(j + 1) * 128],
                            in_=psums[j][:, 0:128],
                            func=AF.Copy, scale=rcp[:, 0:1])
                    else:
                        nc.vector.memset(aggg[:, j * 128:(j + 1) * 128], 0.0)
                nc.sync.dma_start(
                    out[:, g["b0"] * 128:(g["b0"] + nbk) * 128], aggg[:])

    nc.compile()
    return nc


def make_in_maps(cfg, sched, per_core, x, gate_w, gate_b):
    x = np.asarray(x, np.float32)
    consts = build_consts(cfg, gate_w, gate_b)
    in_maps = []
    for core in range(cfg.NC):
        in_maps.append({
            "xlo": x[:cfg.LO_MAX],
            "xhi": x[cfg.HI_BASE:],
            "idx": per_core[core]["idx"],
            "slt": per_core[core]["slots"],
            "p01": per_core[core]["p01"],
            "cst": consts,
        })
    return in_maps


def _kernel_impl(x, gate_w, gate_b, edge_index, cfg=None, return_nc=False):
    from concourse.bass_utils import run_bass_kernel_spmd
    if cfg is None:
        cfg = Config()
    sched, per_core = build_schedule(cfg, edge_index[0], edge_index[1])
    nc = build_program(cfg, sched)
    in_maps = make_in_maps(cfg, sched, per_core, x, gate_w, gate_b)
    res = run_bass_kernel_spmd(nc, in_maps, core_ids=list(range(cfg.NC)))
    perm = sched["perm"]
    outp = np.zeros((cfg.N, 256), np.float32)
    outp[:, 0:128] = x
    indeg = np.bincount(np.asarray(edge_index[1], np.int64),
                        minlength=cfg.N)
    for core in range(cfg.NC):
        # o: [128, NBUK*128] bf16 slot-major -> [NBUK, 128, 128] agg
        o = np.asarray(res.results[core]["out"], dtype=np.float32)
        o = o.reshape(128, cfg.NBUK, 128).transpose(1, 0, 2)
        base = core * cfg.NPC
        for k in range(cfg.NBUK):
            b = int(perm[core, k])
            v = min(128, cfg.NPC - b * 128)
            outp[base + b * 128:base + b * 128 + v, 128:256] = o[k, :v]
    outp[indeg == 0, 128:256] = 0.0
    if return_nc:
        return outp, nc
    return outp


def kernel(**inputs):
    """Harness entry: full unsharded inputs -> full [50000, 256] f32 output.

    Shards edges by destination-node range across the 8 NeuronCores
    (each core computes its 6250-node output slice fully locally),
    compiles the Bass program, and runs it via run_bass_kernel_spmd.
    """
    x = np.ascontiguousarray(np.asarray(inputs["x"], np.float32))
    gate_w = np.asarray(inputs["gate_w"], np.float32)
    gate_b = np.asarray(inputs["gate_b"], np.float32)
    edge_index = np.asarray(inputs["edge_index"])
    return _kernel_impl(x, gate_w, gate_b, edge_index)


# revision 54
# speedup vs baseline: 1.0213x; 1.0026x over previous
"""AttentionalAggregation GNN kernel for 8 TRN2 NeuronCores.

Strategy: edges sorted by destination bucket on host; core m owns nodes
[m*NPC, (m+1)*NPC) and computes its output slice fully locally (no
collectives). Per 128-edge tile:
  - dma_gather x[src] rows (512B each) from lo/hi half tables (int16 idx)
  - score_e = sum_d V[e,d]*w[d]  (scalar_tensor_tensor accum)
  - e = exp(score + b)           (ACT)
  - P'[e,s] = (iota_s == slot_e) * e_e   (built on DVE, Pool or ACT,
    split to balance engine load; ACT path reads a host-built 0/1
    one-hot in fp8 and scales by e in one Copy op)
  - psum[bucket][:,0:128] += P'.T @ V_t   (plain f32 matmul: hardware
    f32r is tf32-grade and fails the accuracy budget)
  - psum[bucket][:,128]   += P'.T @ ones  (denominator)
Flush per group: reciprocal of denominator, scale, write agg as bf16 in
slot-major layout; host unpermutes, casts to f32 and concats with x.
"""

import math
import numpy as np

import concourse.bass as bass
import concourse.mybir as mybir
import concourse.tile as tile
from concourse import bacc

F32 = mybir.dt.float32
F32R = mybir.dt.float32r
BF16 = mybir.dt.bfloat16
FP8 = mybir.dt.float8e4
I16 = mybir.dt.int16
AF = mybir.ActivationFunctionType
OP = mybir.AluOpType

# engine assignment for building P' (one-hot * e) per tile
ENG_DVE = 0
ENG_POOL = 1
ENG_ACT = 2
# shares (DVE, POOL, ACT) of P'-build work; ACT tiles need the p01 stream.
# Pool-engine P' builds stall the in-order Pool queue behind cross-engine
# waits (gathers share it), so the Pool share stays 0.
PP_SHARES = (0.28, 0.0, 0.72)
# fraction of score STTs routed to the Pool engine. Keep 0: any non-SWDGE
# work on the in-order Pool engine delays later groups' descriptor
# generation and starves the DMA engines.
SC_POOL = 0.0


class Config:
    def __init__(self, N=50000, E=640000, D=128, NC=8, GROUP=2):
        assert D == 128
        self.N, self.E, self.D, self.NC = N, E, D, NC
        self.NPC = N // NC          # nodes per core
        assert self.NPC * NC == N
        # overlapping lo/hi gather tables (int16 index limit 32768 rows);
        # sources in the overlap may be assigned to either run, letting the
        # host pad the lo run to a tile boundary with real edges
        self.LO_MAX = min(32768, N)
        self.HI_BASE = max(0, N - 32768)
        self.NBUK = math.ceil(self.NPC / 128)   # buckets per core
        self.TAIL = self.NPC - (self.NBUK - 1) * 128  # rows in last bucket
        self.GROUP = GROUP
        self.NG = math.ceil(self.NBUK / GROUP)


def assign_engines(TOT):
    """Per-tile engine for the P' build, interleaved to the target shares."""
    eng = np.zeros(TOT, np.int8)
    acc = [0.0, 0.0, 0.0]
    for t in range(TOT):
        # pick the engine furthest below its target share
        deficits = [PP_SHARES[k] * (t + 1) - acc[k] for k in range(3)]
        k = int(np.argmax(deficits))
        eng[t] = k
        acc[k] += 1.0
    return eng


def build_schedule(cfg, src, dst):
    """Host-side: sort/pad edges into a static per-tile schedule uniform
    across cores. Returns (sched, per_core) where sched is the static
    structure (identical across cores) and per_core has the data arrays."""
    N, NC, NPC, NBUK, GROUP = (
        cfg.N, cfg.NC, cfg.NPC, cfg.NBUK, cfg.GROUP)
    LO_MAX, HI_BASE = cfg.LO_MAX, cfg.HI_BASE

    src = np.asarray(src, np.int64)
    dst = np.asarray(dst, np.int64)
    c = dst // NPC
    r = dst % NPC
    lb = r // 128
    slot = r % 128

    order = np.lexsort((src, lb, c))
    src_s, lb_s, slot_s, c_s = (
        src[order], lb[order], slot[order], c[order])

    key = c_s * NBUK + lb_s
    cnt = np.bincount(key, minlength=NC * NBUK).reshape(NC, NBUK)
    starts = np.zeros(NC * NBUK + 1, np.int64)
    np.cumsum(cnt.reshape(-1), out=starts[1:])

    # within each (core, bucket) slice (sorted by src), edges below
    # HI_BASE must use the lo table, edges >= LO_MAX must use hi, and the
    # overlap is flexible: cut the slice to fill lo tiles exactly
    n_lo_min = np.zeros((NC, NBUK), np.int64)
    n_lo_cap = np.zeros((NC, NBUK), np.int64)
    for cc in range(NC):
        for b in range(NBUK):
            k = cc * NBUK + b
            sl = src_s[starts[k]:starts[k + 1]]
            n_lo_min[cc, b] = np.searchsorted(sl, HI_BASE)
            n_lo_cap[cc, b] = np.searchsorted(sl, LO_MAX)
    # per-core needs, then sort each core's buckets by total tiles
    # descending so position-wise maxima across cores are tight
    t_lo_c = np.ceil(n_lo_min / 128.0).astype(np.int64)       # [NC, NBUK]
    lo_cap_pos = n_lo_cap
    t_hi_c = np.ceil(np.maximum(cnt - np.minimum(128 * t_lo_c, lo_cap_pos),
                                0) / 128.0).astype(np.int64)
    tot_c = t_lo_c + t_hi_c
    perm = np.argsort(-tot_c, axis=1, kind="stable")          # [NC, NBUK]
    ar = np.arange(NC)[:, None]
    T_lo = t_lo_c[ar, perm].max(axis=0)                       # [NBUK] by pos
    lo_count_pos = np.minimum(128 * T_lo[None, :], n_lo_cap[ar, perm])
    T_hi = np.ceil((cnt[ar, perm] - lo_count_pos) / 128.0
                   ).astype(np.int64).max(axis=0)
    Th = np.stack([T_lo, T_hi], axis=1)  # [NBUK, 2] by position
    # scatter position-based lo counts back to per-(core,bucket)
    lo_count = np.zeros_like(cnt)
    np.put_along_axis(lo_count, perm, lo_count_pos, axis=1)
    # move the smallest bucket to position 0: paired with a single-bucket
    # first group below, it halves the ramp-critical first gather batch
    pi = np.r_[NBUK - 1, NBUK - 2, np.arange(NBUK - 2)]
    perm = perm[:, pi]
    Th = Th[pi]
    Tb = Th.sum(axis=1)

    # static tile stream: per group g: [lo tiles of buckets][hi tiles]
    # each entry: (bucket_local_index_in_group j, bucket b, first, last)
    sizes = [1, 1]
    left = NBUK - 2
    while left > 0:
        take = min(GROUP, left)
        sizes.append(take)
        left -= take
    bounds = np.cumsum([0] + sizes)
    groups = []
    for g in range(len(sizes)):
        b0, b1 = int(bounds[g]), int(bounds[g + 1])
        tiles = []
        for h in (0, 1):
            for b in range(b0, b1):
                nt = int(Th[b, h])
                for t in range(nt):
                    pos = t if h == 0 else int(Th[b, 0]) + t
                    first = pos == 0
                    last = pos == int(Tb[b]) - 1
                    tiles.append(dict(j=b - b0, b=b, first=first, last=last))
        lo_tiles = int(Th[b0:b1, 0].sum())
        hi_tiles = int(Th[b0:b1, 1].sum())
        groups.append(dict(b0=b0, b1=b1, lo=lo_tiles, hi=hi_tiles,
                           tiles=tiles))
    TOT = sum(g["lo"] + g["hi"] for g in groups)

    # engine assignment per global tile; ACT tiles get p01 stream slots
    eng = assign_engines(TOT)
    act_idx = np.nonzero(eng == ENG_ACT)[0]
    act_pos = np.full(TOT, -1, np.int64)    # tile -> index among ACT tiles
    for i, t in enumerate(act_idx):
        act_pos[t] = i
    NACT = len(act_idx)

    # pass A: per-tile slot-span unions across cores (edges re-sorted by
    # slot within each (bucket, half) run so a tile covers a narrow
    # contiguous window; matmul partition offsets must be 32-aligned with
    # legal widths {32 @0/32/64/96, 64 @0/64, 128 @0})
    span_lo = np.full(TOT, 255, np.int64)
    span_hi = np.full(TOT, -1, np.int64)
    for core in range(NC):
        t_glob = 0
        for g in groups:
            for h in (0, 1):
                for pos in range(g["b0"], g["b1"]):
                    b = int(perm[core, pos])
                    k = core * NBUK + b
                    s0, s1 = starts[k], starts[k + 1]
                    cut = s0 + lo_count[core, b]
                    e_slot = (slot_s[s0:cut] if h == 0
                              else slot_s[cut:s1])
                    nt = int(Th[pos, h])
                    if nt > 0 and len(e_slot) > 0:
                        sl = np.sort(e_slot)
                        for kk in range(nt):
                            seg = sl[kk * 128:(kk + 1) * 128]
                            if len(seg):
                                t = t_glob + kk
                                span_lo[t] = min(span_lo[t], int(seg[0]))
                                span_hi[t] = max(span_hi[t], int(seg[-1]))
                    t_glob += nt

    def legal_window(a, b):
        # base partitions allowed by the AP layer: {0, 32, 64} only
        if b < 0:
            return 0, 32
        p = 32 * (a // 32)
        if p <= 64 and b - p < 32:
            return p, 32
        p = 64 * (a // 64)
        if b - p < 64:
            return p, 64
        return 0, 128

    win = [legal_window(int(span_lo[t]), int(span_hi[t]))
           for t in range(TOT)]
    # the bucket-opening tile (start=True) must cover all 128 partitions:
    # hardware clears the full bank row only for partitions the matmul
    # writes (validated: narrow first tiles corrupt untouched partitions).
    # Swap a naturally-full-window slice to the front of the bucket's
    # first run when one exists; otherwise widen the first tile.
    slice_perm = {}  # (pos, h) -> permutation of the run's slice indices
    t_glob = 0
    for g in groups:
        for h in (0, 1):
            for pos in range(g["b0"], g["b1"]):
                nt = int(Th[pos, h])
                if nt == 0:
                    continue
                is_first_run = (h == 0) or int(Th[pos, 0]) == 0
                if is_first_run:
                    p = list(range(nt))
                    kfull = next((k for k in range(nt)
                                  if win[t_glob + k] == (0, 128)), None)
                    if kfull is not None and kfull != 0:
                        p[0], p[kfull] = p[kfull], p[0]
                        slice_perm[(pos, h)] = p
                        w2 = [win[t_glob + k] for k in p]
                        for k in range(nt):
                            win[t_glob + k] = w2[k]
                    elif kfull is None:
                        win[t_glob] = (0, 128)
                t_glob += nt
    wp0 = np.array([w[0] for w in win], np.int64)
    ww = np.array([w[1] for w in win], np.int64)
    # variable-width p01 offsets for ACT tiles
    act_woff = np.zeros(NACT + 1, np.int64)
    np.cumsum(ww[act_idx], out=act_woff[1:])
    P01W = int(act_woff[-1])

    # per-core data arrays
    per_core = []
    for core in range(NC):
        idx_stream = np.zeros(TOT * 128, np.int16)
        slot_stream = np.full((128, TOT), 255.0, np.float32)
        t_glob = 0
        for g in groups:
            for h in (0, 1):
                for pos in range(g["b0"], g["b1"]):
                    b = int(perm[core, pos])
                    k = core * NBUK + b
                    s0, s1 = starts[k], starts[k + 1]
                    cut = s0 + lo_count[core, b]
                    if h == 0:
                        e_src = src_s[s0:cut]
                        e_slot = slot_s[s0:cut]
                    else:
                        e_src = src_s[cut:s1] - HI_BASE
                        e_slot = slot_s[cut:s1]
                    o2 = np.argsort(e_slot, kind="stable")
                    e_src = e_src[o2]
                    e_slot = e_slot[o2]
                    sp = slice_perm.get((pos, h))
                    if sp is not None and len(e_src) > 0:
                        segs = [e_src[k * 128:(k + 1) * 128] for k in sp]
                        segt = [e_slot[k * 128:(k + 1) * 128] for k in sp]
                        e_src = np.concatenate(segs)
                        e_slot = np.concatenate(segt)
                    n = len(e_src)
                    nt = int(Th[pos, h])
                    base = t_glob * 128
                    if n > 0:
                        idx_stream[base:base + n] = e_src.astype(np.int16)
                        fl = np.full(nt * 128, 255.0, np.float32)
                        fl[:n] = e_slot.astype(np.float32)
                        slot_stream[:, t_glob:t_glob + nt] = (
                            fl.reshape(nt, 128).T)
                    t_glob += nt
        assert t_glob == TOT
        # window-relative slots (255 padding stays out of range for any W)
        real = slot_stream < 255.0
        slot_stream = slot_stream - wp0[None, :].astype(np.float32) * real
        # wrap-16 the index stream, replicate across the 8 16-part groups
        wrapped = idx_stream.reshape(-1, 16).T  # [16, TOT*8]
        idx_arr = np.tile(wrapped, (8, 1)).copy()  # [128, TOT*8]
        # host-built 0/1 one-hot for ACT tiles, window-relative and
        # variable-width (width ww[t] per tile, concatenated)
        f8 = mybir.dt.np(FP8)
        p01 = np.zeros((128, max(P01W, 1)), f8)
        for i, t in enumerate(act_idx):
            W = int(ww[t])
            off = int(act_woff[i])
            oh = (slot_stream[:, t][:, None] ==
                  np.arange(W, dtype=np.float32)[None, :])
            p01[:, off:off + W] = oh.astype(f8)
        per_core.append(dict(idx=idx_arr, slots=slot_stream, p01=p01))

    sched = dict(groups=groups, TOT=TOT, Th=Th, Tb=Tb, perm=perm,
                 eng=eng, act_pos=act_pos, NACT=NACT,
                 wp0=wp0, ww=ww, act_woff=act_woff, P01W=P01W)
    return sched, per_core


def build_consts(cfg, gate_w, gate_b):
    """[128, 258] f32: iota | W_bcast | ones | b"""
    C = np.zeros((128, 258), np.float32)
    C[:, 0:128] = np.arange(128, dtype=np.float32)[None, :]
    C[:, 128:256] = np.asarray(gate_w, np.float32).reshape(1, 128)
    C[:, 256] = 1.0
    # gate bias is a uniform score shift and cancels in the softmax, so
    # the device never applies it; column 257 is kept but unused
    C[:, 257] = np.float32(np.asarray(gate_b).reshape(-1)[0])
    return C


def build_program(cfg, sched, do_main=True, do_compute=True):
    nc = bacc.Bacc("TRN2", num_devices=cfg.NC,
                   dynamic_dma_scratch_size=49152)
    NPC, NBUK, GROUP = cfg.NPC, cfg.NBUK, cfg.GROUP
    TOT = sched["TOT"]
    groups = sched["groups"]
    eng_map = sched["eng"]
    act_pos = sched["act_pos"]
    NACT = sched["NACT"]
    wp0 = sched["wp0"]
    ww = sched["ww"]
    act_woff = sched["act_woff"]

    xlo = nc.dram_tensor("xlo", [cfg.LO_MAX, 128], F32,
                         kind="ExternalInput")
    xhi = nc.dram_tensor("xhi", [cfg.N - cfg.HI_BASE, 128], F32,
                         kind="ExternalInput")
    idx = nc.dram_tensor("idx", [128, TOT * 8], I16, kind="ExternalInput")
    slt = nc.dram_tensor("slt", [128, TOT], F32, kind="ExternalInput")
    p01 = nc.dram_tensor("p01", [128, max(sched["P01W"], 1)], FP8,
                         kind="ExternalInput")
    cst = nc.dram_tensor("cst", [128, 258], F32, kind="ExternalInput")
    # agg output, slot-major: out[p, b*128 + d] = agg[b*128 + p, d], bf16
    out = nc.dram_tensor("out", [128, NBUK * 128], BF16,
                         kind="ExternalOutput")

    with tile.TileContext(nc) as tc:
        with (
            tc.tile_pool(name="const", bufs=1) as cpool,
            tc.tile_pool(name="meta", bufs=1) as mpool,
            tc.tile_pool(name="gather", bufs=3) as gpool,
            tc.tile_pool(name="sc", bufs=6) as scpool,
            tc.tile_pool(name="grp", bufs=3) as grpool,
            tc.tile_pool(name="pp", bufs=8) as ppool,
            tc.tile_pool(name="fl", bufs=4) as flpool,
            tc.tile_pool(name="ag", bufs=3) as agpool,
            tc.tile_pool(name="pnum", bufs=8, space="PSUM") as pnum,
        ):
            C = cpool.tile([128, 258], F32)
            iota_ap = C[:, 0:128]
            w_ap = C[:, 128:256]
            ones_ap = C[:, 256:257]
            b_ap = C[:, 257:258]

            # the idx stream is uploaded in chunks emitted just ahead of
            # the gathers that read them: a single monolithic upload held
            # the first gather batch (and the whole pipeline) back ~10us
            idx_sb = mpool.tile([128, TOT * 8], I16)
            slt_sb = mpool.tile([128, TOT], F32)

            # ---- main loop over groups (gathers prefetched one group
            # ahead so the in-order Pool queue never parks later groups'
            # gathers behind P'-builds that wait on cross-engine deps) ----
            # each dma_gather is capped at GMAX tiles: the SWDGE ring
            # holds scratch/16 descriptors and one instruction's
            # descriptor set must fit entirely
            GMAX = 8
            work = [g for g in (groups if do_main else [])
                    if g["lo"] + g["hi"] > 0]
            tg_of = {}
            acc = 0
            for g in (groups if do_main else []):
                tg_of[id(g)] = acc
                acc += g["lo"] + g["hi"]

            gbufs = {}

            # idx chunks cover CHUNK_GROUPS consecutive work groups (kept
            # >=512B per partition to dodge the small-transfer penalty)
            CHUNK_GROUPS = 3
            chunk_of = {}
            chunk_rng = []
            for wi, g in enumerate(work):
                ci = wi // CHUNK_GROUPS
                chunk_of[id(g)] = ci
                tg0 = tg_of[id(g)]
                TGg = g["lo"] + g["hi"]
                if ci == len(chunk_rng):
                    chunk_rng.append([tg0, tg0 + TGg])
                else:
                    chunk_rng[ci][1] = tg0 + TGg
            idx_chunk_done = set()

            def emit_idx_chunk(ci):
                if ci in idx_chunk_done:
                    return
                idx_chunk_done.add(ci)
                r0, r1 = chunk_rng[ci]
                nc.sync.dma_start(idx_sb[:, r0 * 8:r1 * 8],
                                  idx[:, r0 * 8:r1 * 8])

            def emit_gathers(g, first_small=False):
                emit_idx_chunk(chunk_of[id(g)])
                TG = g["lo"] + g["hi"]
                tg0 = tg_of[id(g)]
                # extra zeroed tile at the end lets the last tile's matmul
                # read a 256-wide rhs (junk cols accumulate into psum cols
                # 128:256, never read)
                gbuf = gpool.tile([128, TG, 128], F32, tag="gbuf")
                for half, n_t, base in ((0, g["lo"], 0),
                                        (1, g["hi"], g["lo"])):
                    tbl = xlo if half == 0 else xhi
                    # a small leading batch lets the first scores start
                    # as soon as 4 tiles land instead of a full batch
                    cuts = list(range(0, n_t, GMAX))
                    if first_small and half == 0 and n_t > 4:
                        cuts = [0, 4] + [c for c in cuts if c > 4]
                    for ci, q0 in enumerate(cuts):
                        q1 = cuts[ci + 1] if ci + 1 < len(cuts) else min(
                            q0 + GMAX, n_t)
                        b0t = base + q0
                        g0 = tg0 + b0t
                        nc.gpsimd.dma_gather(
                            out_ap=gbuf[:, b0t:base + q1, :],
                            in_ap=tbl[:],
                            idxs_ap=idx_sb[:, g0 * 8:(g0 + q1 - q0) * 8],
                            num_idxs=(q1 - q0) * 128,
                            num_idxs_reg=(q1 - q0) * 128,
                            elem_size=128,
                            single_packet=False,
                        )
                gbufs[id(g)] = gbuf

            # zero-fill output for empty groups up front
            for g in (groups if do_main else []):
                if g["lo"] + g["hi"] == 0:
                    nbk = g["b1"] - g["b0"]
                    aggg = agpool.tile([128, nbk * 128], BF16, tag="aggg")
                    nc.vector.memset(aggg[:], 0.0)
                    nc.sync.dma_start(
                        out[:, g["b0"] * 128:(g["b0"] + nbk) * 128], aggg[:])

            PREFETCH = 1  # groups of gathers in flight ahead (gbuf bufs-1)
            for wi, g in enumerate(work[:PREFETCH]):
                emit_gathers(g)
            # consts and slot stream issue after the pipeline-critical
            # first gathers (each DMA issue serializes ~0.6us on HWDGE)
            nc.sync.dma_start(C[:], cst[:])
            nc.sync.dma_start(slt_sb[:], slt[:])
            for wi, g in enumerate(work):
                if wi + PREFETCH < len(work):
                    emit_gathers(work[wi + PREFETCH])
                TG = g["lo"] + g["hi"]
                nbk = g["b1"] - g["b0"]
                t_glob = tg_of[id(g)]
                gbuf = gbufs.pop(id(g))
                aggg = agpool.tile([128, nbk * 128], BF16, tag="aggg")

                # p01 stream for this group's ACT-assigned tiles
                # (variable widths, contiguous in the global ACT ordering)
                acts = [t for t in range(TG) if eng_map[t_glob + t] == ENG_ACT]
                nact = len(acts)
                if nact > 0:
                    ai0 = int(act_pos[t_glob + acts[0]])
                    w0 = int(act_woff[ai0])
                    w1 = int(act_woff[ai0 + nact])
                    p01_sb = grpool.tile([128, w1 - w0], FP8, tag="p01sb")
                    nc.sync.dma_start(p01_sb[:], p01[:, w0:w1])

                if not do_compute:
                    # still consume gbuf so Tile keeps the gathers
                    sink = flpool.tile([128, 1], F32, tag="sink")
                    nc.vector.tensor_scalar(
                        out=sink[:], in0=gbuf[:, 0, 0:1], scalar1=1.0,
                        scalar2=None, op0=OP.mult)
                    nc.sync.dma_start(out[0:128, 0:1], sink[:].bitcast(BF16)[:, 0:1])
                    continue
                # scores for all tiles of the group
                sg = grpool.tile([128, TG], F32, tag="sg")
                eg = grpool.tile([128, TG], F32, tag="eg")
                for t in range(TG):
                    scr = scpool.tile([128, 128], F32, tag="scr",
                                      name="scr_v")
                    sc_eng = (nc.gpsimd if (t_glob + t) % 100 <
                              SC_POOL * 100 else nc.vector)
                    sc_eng.scalar_tensor_tensor(
                        out=scr[:], in0=gbuf[:, t, :], scalar=1.0,
                        in1=w_ap, op0=OP.mult, op1=OP.mult,
                        accum_out=sg[:, t:t + 1])
                    # per-16-tile exp keeps the pipeline fine-grained
                    if t % 16 == 15 or t == TG - 1:
                        lo8 = (t // 16) * 16
                        nc.scalar.activation(out=eg[:, lo8:t + 1],
                                             in_=sg[:, lo8:t + 1],
                                             func=AF.Exp,
                                             bias=0.0, scale=1.0)

                # per-bucket psum tiles: cols 0:128 numerator, col 128 denom
                psums = {}
                for j in range(nbk):
                    if sched["Tb"][g["b0"] + j] > 0:
                        psums[j] = pnum.tile([128, 129], F32, tag="pn",
                                             name=f"pn_{g['b0']}_{j}")

                for t, ti in enumerate(g["tiles"]):
                    j = ti["j"]
                    tg = t_glob + t
                    p0 = int(wp0[tg])
                    W = int(ww[tg])
                    Pp = ppool.tile([128, W], F32, tag=f"pp{W}")
                    ek = eng_map[tg]
                    if ek == ENG_ACT:
                        # ACT path: P' = host-built 0/1 one-hot (fp8,
                        # exact, window-relative) scaled by e in one Copy
                        ai = int(act_pos[tg])
                        o0 = int(act_woff[ai]) - int(
                            act_woff[int(act_pos[t_glob + acts[0]])])
                        nc.scalar.activation(
                            out=Pp[:], in_=p01_sb[:, o0:o0 + W],
                            func=AF.Copy, scale=eg[:, t:t + 1])
                    else:
                        nc.vector.tensor_scalar(
                            out=Pp[:], in0=iota_ap[:, 0:W],
                            scalar1=slt_sb[:, tg:tg + 1],
                            scalar2=eg[:, t:t + 1],
                            op0=OP.is_equal, op1=OP.mult)
                    # slot-sorted tiles cover a narrow aligned window
                    # [p0, p0+W) of the bucket's 128 slots; the matmul
                    # writes just those psum partitions. The bucket's
                    # first tile is forced to the full window so its
                    # start=True clears every partition of the bank.
                    nc.tensor.matmul(
                        out=psums[j][p0:p0 + W, 0:128], lhsT=Pp[:],
                        rhs=gbuf[:, t, :],
                        start=ti["first"], stop=False)
                    nc.tensor.matmul(
                        out=psums[j][p0:p0 + W, 128:129], lhsT=Pp[:],
                        rhs=ones_ap,
                        start=False, stop=ti["last"])

                # flush group: per-bucket reciprocal + scale into the
                # group's bf16 slot-major agg tile, then one DMA out
                for j in range(nbk):
                    if j in psums:
                        # no epsilon guard: slots with zero in-degree give
                        # den=0 -> inf/NaN rows, which the host overwrites
                        # with zeros (it knows the in-degrees from
                        # edge_index); skipping the max() op saves ~6us DVE
                        rcp = flpool.tile([128, 1], F32, tag="rcp")
                        nc.vector.reciprocal(out=rcp[:],
                                             in_=psums[j][:, 128:129])
                        nc.scalar.activation(
                            out=aggg[:, j * 128:(j + 1) * 128],
                            in_=psums[j][:, 0:128],
                            func=AF.Copy, scale=rcp[:, 0:1])
                    else:
                        nc.vector.memset(aggg[:, j * 128:(j + 1) * 128], 0.0)
                nc.sync.dma_start(
                    out[:, g["b0"] * 128:(g["b0"] + nbk) * 128], aggg[:])

    nc.compile()
    return nc


def make_in_maps(cfg, sched, per_core, x, gate_w, gate_b):
    x = np.asarray(x, np.float32)
    consts = build_consts(cfg, gate_w, gate_b)
    in_maps = []
    for core in range(cfg.NC):
        in_maps.append({
            "xlo": x[:cfg.LO_MAX],
            "xhi": x[cfg.HI_BASE:],
            "idx": per_core[core]["idx"],
            "slt": per_core[core]["slots"],
            "p01": per_core[core]["p01"],
            "cst": consts,
        })
    return in_maps


def _kernel_impl(x, gate_w, gate_b, edge_index, cfg=None, return_nc=False):
    from concourse.bass_utils import run_bass_kernel_spmd
    if cfg is None:
        cfg = Config()
    sched, per_core = build_schedule(cfg, edge_index[0], edge_index[1])
    nc = build_program(cfg, sched)
    in_maps = make_in_maps(cfg, sched, per_core, x, gate_w, gate_b)
    res = run_bass_kernel_spmd(nc, in_maps, core_ids=list(range(cfg.NC)))
    perm = sched["perm"]
    outp = np.zeros((cfg.N, 256), np.float32)
    outp[:, 0:128] = x
    indeg = np.bincount(np.asarray(edge_index[1], np.int64),
                        minlength=cfg.N)
    for core in range(cfg.NC):
        # o: [128, NBUK*128] bf16 slot-major -> [NBUK, 128, 128] agg
        o = np.asarray(res.results[core]["out"], dtype=np.float32)
        o = o.reshape(128, cfg.NBUK, 128).transpose(1, 0, 2)
        base = core * cfg.NPC
        for k in range(cfg.NBUK):
            b = int(perm[core, k])
            v = min(128, cfg.NPC - b * 128)
            outp[base + b * 128:base + b * 128 + v, 128:256] = o[k, :v]
    outp[indeg == 0, 128:256] = 0.0
    if return_nc:
        return outp, nc
    return outp


def kernel(**inputs):
    """Harness entry: full unsharded inputs -> full [50000, 256] f32 output.

    Shards edges by destination-node range across the 8 NeuronCores
    (each core computes its 6250-node output slice fully locally),
    compiles the Bass program, and runs it via run_bass_kernel_spmd.
    """
    x = np.ascontiguousarray(np.asarray(inputs["x"], np.float32))
    gate_w = np.asarray(inputs["gate_w"], np.float32)
    gate_b = np.asarray(inputs["gate_b"], np.float32)
    edge_index = np.asarray(inputs["edge_index"])
    return _kernel_impl(x, gate_w, gate_b, edge_index)


# revision 55
# speedup vs baseline: 1.0302x; 1.0087x over previous
"""AttentionalAggregation GNN kernel for 8 TRN2 NeuronCores.

Strategy: edges sorted by destination bucket on host; core m owns nodes
[m*NPC, (m+1)*NPC) and computes its output slice fully locally (no
collectives). Per 128-edge tile:
  - dma_gather x[src] rows (512B each) from lo/hi half tables (int16 idx)
  - score_e = sum_d V[e,d]*w[d]  (scalar_tensor_tensor accum)
  - e = exp(score + b)           (ACT)
  - P'[e,s] = (iota_s == slot_e) * e_e   (built on DVE, Pool or ACT,
    split to balance engine load; ACT path reads a host-built 0/1
    one-hot in fp8 and scales by e in one Copy op)
  - psum[bucket][:,0:128] += P'.T @ V_t   (plain f32 matmul: hardware
    f32r is tf32-grade and fails the accuracy budget)
  - psum[bucket][:,128]   += P'.T @ ones  (denominator)
Flush per group: reciprocal of denominator, scale, write agg as bf16 in
slot-major layout; host unpermutes, casts to f32 and concats with x.
"""

import math
import numpy as np

import concourse.bass as bass
import concourse.mybir as mybir
import concourse.tile as tile
from concourse import bacc

F32 = mybir.dt.float32
F32R = mybir.dt.float32r
BF16 = mybir.dt.bfloat16
FP8 = mybir.dt.float8e4
I16 = mybir.dt.int16
AF = mybir.ActivationFunctionType
OP = mybir.AluOpType

# engine assignment for building P' (one-hot * e) per tile
ENG_DVE = 0
ENG_POOL = 1
ENG_ACT = 2
# shares (DVE, POOL, ACT) of P'-build work; ACT tiles need the p01 stream.
# Pool-engine P' builds stall the in-order Pool queue behind cross-engine
# waits (gathers share it), so the Pool share stays 0.
PP_SHARES = (0.28, 0.0, 0.72)
# fraction of score STTs routed to the Pool engine. Keep 0: any non-SWDGE
# work on the in-order Pool engine delays later groups' descriptor
# generation and starves the DMA engines.
SC_POOL = 0.0


class Config:
    def __init__(self, N=50000, E=640000, D=128, NC=8, GROUP=2):
        assert D == 128
        self.N, self.E, self.D, self.NC = N, E, D, NC
        self.NPC = N // NC          # nodes per core
        assert self.NPC * NC == N
        # overlapping lo/hi gather tables (int16 index limit 32768 rows);
        # sources in the overlap may be assigned to either run, letting the
        # host pad the lo run to a tile boundary with real edges
        self.LO_MAX = min(32768, N)
        self.HI_BASE = max(0, N - 32768)
        self.NBUK = math.ceil(self.NPC / 128)   # buckets per core
        self.TAIL = self.NPC - (self.NBUK - 1) * 128  # rows in last bucket
        self.GROUP = GROUP
        self.NG = math.ceil(self.NBUK / GROUP)


def assign_engines(TOT):
    """Per-tile engine for the P' build, interleaved to the target shares."""
    eng = np.zeros(TOT, np.int8)
    acc = [0.0, 0.0, 0.0]
    for t in range(TOT):
        # pick the engine furthest below its target share
        deficits = [PP_SHARES[k] * (t + 1) - acc[k] for k in range(3)]
        k = int(np.argmax(deficits))
        eng[t] = k
        acc[k] += 1.0
    return eng


def build_schedule(cfg, src, dst):
    """Host-side: sort/pad edges into a static per-tile schedule uniform
    across cores. Returns (sched, per_core) where sched is the static
    structure (identical across cores) and per_core has the data arrays."""
    N, NC, NPC, NBUK, GROUP = (
        cfg.N, cfg.NC, cfg.NPC, cfg.NBUK, cfg.GROUP)
    LO_MAX, HI_BASE = cfg.LO_MAX, cfg.HI_BASE

    src = np.asarray(src, np.int64)
    dst = np.asarray(dst, np.int64)
    c = dst // NPC
    r = dst % NPC
    lb = r // 128
    slot = r % 128

    order = np.lexsort((src, lb, c))
    src_s, lb_s, slot_s, c_s = (
        src[order], lb[order], slot[order], c[order])

    key = c_s * NBUK + lb_s
    cnt = np.bincount(key, minlength=NC * NBUK).reshape(NC, NBUK)
    starts = np.zeros(NC * NBUK + 1, np.int64)
    np.cumsum(cnt.reshape(-1), out=starts[1:])

    # within each (core, bucket) slice (sorted by src), edges below
    # HI_BASE must use the lo table, edges >= LO_MAX must use hi, and the
    # overlap is flexible: cut the slice to fill lo tiles exactly
    n_lo_min = np.zeros((NC, NBUK), np.int64)
    n_lo_cap = np.zeros((NC, NBUK), np.int64)
    for cc in range(NC):
        for b in range(NBUK):
            k = cc * NBUK + b
            sl = src_s[starts[k]:starts[k + 1]]
            n_lo_min[cc, b] = np.searchsorted(sl, HI_BASE)
            n_lo_cap[cc, b] = np.searchsorted(sl, LO_MAX)
    # per-core needs, then sort each core's buckets by total tiles
    # descending so position-wise maxima across cores are tight
    t_lo_c = np.ceil(n_lo_min / 128.0).astype(np.int64)       # [NC, NBUK]
    lo_cap_pos = n_lo_cap
    t_hi_c = np.ceil(np.maximum(cnt - np.minimum(128 * t_lo_c, lo_cap_pos),
                                0) / 128.0).astype(np.int64)
    tot_c = t_lo_c + t_hi_c
    perm = np.argsort(-tot_c, axis=1, kind="stable")          # [NC, NBUK]
    ar = np.arange(NC)[:, None]
    T_lo = t_lo_c[ar, perm].max(axis=0)                       # [NBUK] by pos
    lo_count_pos = np.minimum(128 * T_lo[None, :], n_lo_cap[ar, perm])
    T_hi = np.ceil((cnt[ar, perm] - lo_count_pos) / 128.0
                   ).astype(np.int64).max(axis=0)
    Th = np.stack([T_lo, T_hi], axis=1)  # [NBUK, 2] by position
    # scatter position-based lo counts back to per-(core,bucket)
    lo_count = np.zeros_like(cnt)
    np.put_along_axis(lo_count, perm, lo_count_pos, axis=1)
    # move the smallest bucket to position 0: paired with a single-bucket
    # first group below, it halves the ramp-critical first gather batch
    pi = np.r_[NBUK - 1, NBUK - 2, np.arange(NBUK - 2)]
    perm = perm[:, pi]
    Th = Th[pi]
    Tb = Th.sum(axis=1)

    # static tile stream: per group g: [lo tiles of buckets][hi tiles]
    # each entry: (bucket_local_index_in_group j, bucket b, first, last)
    sizes = [1, 1]
    left = NBUK - 2
    while left > 0:
        take = min(GROUP, left)
        sizes.append(take)
        left -= take
    bounds = np.cumsum([0] + sizes)
    groups = []
    for g in range(len(sizes)):
        b0, b1 = int(bounds[g]), int(bounds[g + 1])
        tiles = []
        for h in (0, 1):
            for b in range(b0, b1):
                nt = int(Th[b, h])
                for t in range(nt):
                    pos = t if h == 0 else int(Th[b, 0]) + t
                    first = pos == 0
                    last = pos == int(Tb[b]) - 1
                    tiles.append(dict(j=b - b0, b=b, first=first, last=last))
        lo_tiles = int(Th[b0:b1, 0].sum())
        hi_tiles = int(Th[b0:b1, 1].sum())
        groups.append(dict(b0=b0, b1=b1, lo=lo_tiles, hi=hi_tiles,
                           tiles=tiles))
    TOT = sum(g["lo"] + g["hi"] for g in groups)

    # engine assignment per global tile; ACT tiles get p01 stream slots
    eng = assign_engines(TOT)
    act_idx = np.nonzero(eng == ENG_ACT)[0]
    act_pos = np.full(TOT, -1, np.int64)    # tile -> index among ACT tiles
    for i, t in enumerate(act_idx):
        act_pos[t] = i
    NACT = len(act_idx)

    # pass A: per-tile slot-span unions across cores (edges re-sorted by
    # slot within each (bucket, half) run so a tile covers a narrow
    # contiguous window; matmul partition offsets must be 32-aligned with
    # legal widths {32 @0/32/64/96, 64 @0/64, 128 @0})
    span_lo = np.full(TOT, 255, np.int64)
    span_hi = np.full(TOT, -1, np.int64)
    for core in range(NC):
        t_glob = 0
        for g in groups:
            for h in (0, 1):
                for pos in range(g["b0"], g["b1"]):
                    b = int(perm[core, pos])
                    k = core * NBUK + b
                    s0, s1 = starts[k], starts[k + 1]
                    cut = s0 + lo_count[core, b]
                    e_slot = (slot_s[s0:cut] if h == 0
                              else slot_s[cut:s1])
                    nt = int(Th[pos, h])
                    if nt > 0 and len(e_slot) > 0:
                        sl = np.sort(e_slot)
                        for kk in range(nt):
                            seg = sl[kk * 128:(kk + 1) * 128]
                            if len(seg):
                                t = t_glob + kk
                                span_lo[t] = min(span_lo[t], int(seg[0]))
                                span_hi[t] = max(span_hi[t], int(seg[-1]))
                    t_glob += nt

    def legal_window(a, b):
        # base partitions allowed by the AP layer: {0, 32, 64} only
        if b < 0:
            return 0, 32
        p = 32 * (a // 32)
        if p <= 64 and b - p < 32:
            return p, 32
        p = 64 * (a // 64)
        if b - p < 64:
            return p, 64
        return 0, 128

    win = [legal_window(int(span_lo[t]), int(span_hi[t]))
           for t in range(TOT)]
    # the bucket-opening tile (start=True) must cover all 128 partitions:
    # hardware clears the full bank row only for partitions the matmul
    # writes (validated: narrow first tiles corrupt untouched partitions).
    # Swap a naturally-full-window slice to the front of the bucket's
    # first run when one exists; otherwise widen the first tile.
    slice_perm = {}  # (pos, h) -> permutation of the run's slice indices
    t_glob = 0
    for g in groups:
        for h in (0, 1):
            for pos in range(g["b0"], g["b1"]):
                nt = int(Th[pos, h])
                if nt == 0:
                    continue
                is_first_run = (h == 0) or int(Th[pos, 0]) == 0
                if is_first_run:
                    p = list(range(nt))
                    kfull = next((k for k in range(nt)
                                  if win[t_glob + k] == (0, 128)), None)
                    if kfull is not None and kfull != 0:
                        p[0], p[kfull] = p[kfull], p[0]
                        slice_perm[(pos, h)] = p
                        w2 = [win[t_glob + k] for k in p]
                        for k in range(nt):
                            win[t_glob + k] = w2[k]
                    elif kfull is None:
                        win[t_glob] = (0, 128)
                t_glob += nt
    wp0 = np.array([w[0] for w in win], np.int64)
    ww = np.array([w[1] for w in win], np.int64)
    # variable-width p01 offsets for ACT tiles
    act_woff = np.zeros(NACT + 1, np.int64)
    np.cumsum(ww[act_idx], out=act_woff[1:])
    P01W = int(act_woff[-1])

    # per-core data arrays
    per_core = []
    for core in range(NC):
        idx_stream = np.zeros(TOT * 128, np.int16)
        slot_stream = np.full((128, TOT), 255.0, np.float32)
        t_glob = 0
        for g in groups:
            for h in (0, 1):
                for pos in range(g["b0"], g["b1"]):
                    b = int(perm[core, pos])
                    k = core * NBUK + b
                    s0, s1 = starts[k], starts[k + 1]
                    cut = s0 + lo_count[core, b]
                    if h == 0:
                        e_src = src_s[s0:cut]
                        e_slot = slot_s[s0:cut]
                    else:
                        e_src = src_s[cut:s1] - HI_BASE
                        e_slot = slot_s[cut:s1]
                    o2 = np.argsort(e_slot, kind="stable")
                    e_src = e_src[o2]
                    e_slot = e_slot[o2]
                    sp = slice_perm.get((pos, h))
                    if sp is not None and len(e_src) > 0:
                        segs = [e_src[k * 128:(k + 1) * 128] for k in sp]
                        segt = [e_slot[k * 128:(k + 1) * 128] for k in sp]
                        e_src = np.concatenate(segs)
                        e_slot = np.concatenate(segt)
                    n = len(e_src)
                    nt = int(Th[pos, h])
                    base = t_glob * 128
                    if n > 0:
                        idx_stream[base:base + n] = e_src.astype(np.int16)
                        fl = np.full(nt * 128, 255.0, np.float32)
                        fl[:n] = e_slot.astype(np.float32)
                        slot_stream[:, t_glob:t_glob + nt] = (
                            fl.reshape(nt, 128).T)
                    t_glob += nt
        assert t_glob == TOT
        # window-relative slots (255 padding stays out of range for any W)
        real = slot_stream < 255.0
        slot_stream = slot_stream - wp0[None, :].astype(np.float32) * real
        # wrap-16 the index stream, replicate across the 8 16-part groups
        wrapped = idx_stream.reshape(-1, 16).T  # [16, TOT*8]
        idx_arr = np.tile(wrapped, (8, 1)).copy()  # [128, TOT*8]
        # host-built 0/1 one-hot for ACT tiles, window-relative and
        # variable-width (width ww[t] per tile, concatenated)
        f8 = mybir.dt.np(FP8)
        p01 = np.zeros((128, max(P01W, 1)), f8)
        for i, t in enumerate(act_idx):
            W = int(ww[t])
            off = int(act_woff[i])
            oh = (slot_stream[:, t][:, None] ==
                  np.arange(W, dtype=np.float32)[None, :])
            p01[:, off:off + W] = oh.astype(f8)
        per_core.append(dict(idx=idx_arr, slots=slot_stream, p01=p01))

    sched = dict(groups=groups, TOT=TOT, Th=Th, Tb=Tb, perm=perm,
                 eng=eng, act_pos=act_pos, NACT=NACT,
                 wp0=wp0, ww=ww, act_woff=act_woff, P01W=P01W)
    return sched, per_core


def build_consts(cfg, gate_w, gate_b):
    """[128, 258] f32: iota | W_bcast | ones | b"""
    C = np.zeros((128, 258), np.float32)
    C[:, 0:128] = np.arange(128, dtype=np.float32)[None, :]
    C[:, 128:256] = np.asarray(gate_w, np.float32).reshape(1, 128)
    C[:, 256] = 1.0
    # gate bias is a uniform score shift and cancels in the softmax, so
    # the device never applies it; column 257 is kept but unused
    C[:, 257] = np.float32(np.asarray(gate_b).reshape(-1)[0])
    return C


def build_program(cfg, sched, do_main=True, do_compute=True):
    nc = bacc.Bacc("TRN2", num_devices=cfg.NC,
                   dynamic_dma_scratch_size=49152)
    NPC, NBUK, GROUP = cfg.NPC, cfg.NBUK, cfg.GROUP
    TOT = sched["TOT"]
    groups = sched["groups"]
    eng_map = sched["eng"]
    act_pos = sched["act_pos"]
    NACT = sched["NACT"]
    wp0 = sched["wp0"]
    ww = sched["ww"]
    act_woff = sched["act_woff"]

    xlo = nc.dram_tensor("xlo", [cfg.LO_MAX, 128], F32,
                         kind="ExternalInput")
    xhi = nc.dram_tensor("xhi", [cfg.N - cfg.HI_BASE, 128], F32,
                         kind="ExternalInput")
    idx = nc.dram_tensor("idx", [128, TOT * 8], I16, kind="ExternalInput")
    slt = nc.dram_tensor("slt", [128, TOT], F32, kind="ExternalInput")
    p01 = nc.dram_tensor("p01", [128, max(sched["P01W"], 1)], FP8,
                         kind="ExternalInput")
    cst = nc.dram_tensor("cst", [128, 258], F32, kind="ExternalInput")
    # agg output, slot-major: out[p, b*128 + d] = agg[b*128 + p, d], bf16
    out = nc.dram_tensor("out", [128, NBUK * 128], BF16,
                         kind="ExternalOutput")

    with tile.TileContext(nc) as tc:
        with (
            tc.tile_pool(name="const", bufs=1) as cpool,
            tc.tile_pool(name="meta", bufs=1) as mpool,
            tc.tile_pool(name="gather", bufs=4) as gpool,
            tc.tile_pool(name="sc", bufs=6) as scpool,
            tc.tile_pool(name="grp", bufs=3) as grpool,
            tc.tile_pool(name="pp", bufs=8) as ppool,
            tc.tile_pool(name="fl", bufs=4) as flpool,
            tc.tile_pool(name="ag", bufs=3) as agpool,
            tc.tile_pool(name="pnum", bufs=8, space="PSUM") as pnum,
        ):
            C = cpool.tile([128, 258], F32)
            iota_ap = C[:, 0:128]
            w_ap = C[:, 128:256]
            ones_ap = C[:, 256:257]
            b_ap = C[:, 257:258]

            # the idx stream is uploaded in chunks emitted just ahead of
            # the gathers that read them: a single monolithic upload held
            # the first gather batch (and the whole pipeline) back ~10us
            idx_sb = mpool.tile([128, TOT * 8], I16)
            slt_sb = mpool.tile([128, TOT], F32)

            # ---- main loop over groups (gathers prefetched one group
            # ahead so the in-order Pool queue never parks later groups'
            # gathers behind P'-builds that wait on cross-engine deps) ----
            # each dma_gather is capped at GMAX tiles: the SWDGE ring
            # holds scratch/16 descriptors and one instruction's
            # descriptor set must fit entirely
            GMAX = 8
            work = [g for g in (groups if do_main else [])
                    if g["lo"] + g["hi"] > 0]
            tg_of = {}
            acc = 0
            for g in (groups if do_main else []):
                tg_of[id(g)] = acc
                acc += g["lo"] + g["hi"]

            gbufs = {}

            # idx chunks cover CHUNK_GROUPS consecutive work groups (kept
            # >=512B per partition to dodge the small-transfer penalty)
            CHUNK_GROUPS = 3
            chunk_of = {}
            chunk_rng = []
            for wi, g in enumerate(work):
                ci = wi // CHUNK_GROUPS
                chunk_of[id(g)] = ci
                tg0 = tg_of[id(g)]
                TGg = g["lo"] + g["hi"]
                if ci == len(chunk_rng):
                    chunk_rng.append([tg0, tg0 + TGg])
                else:
                    chunk_rng[ci][1] = tg0 + TGg
            idx_chunk_done = set()

            def emit_idx_chunk(ci):
                if ci in idx_chunk_done:
                    return
                idx_chunk_done.add(ci)
                r0, r1 = chunk_rng[ci]
                nc.sync.dma_start(idx_sb[:, r0 * 8:r1 * 8],
                                  idx[:, r0 * 8:r1 * 8])

            def emit_gathers(g, first_small=False):
                emit_idx_chunk(chunk_of[id(g)])
                TG = g["lo"] + g["hi"]
                tg0 = tg_of[id(g)]
                # extra zeroed tile at the end lets the last tile's matmul
                # read a 256-wide rhs (junk cols accumulate into psum cols
                # 128:256, never read)
                gbuf = gpool.tile([128, TG, 128], F32, tag="gbuf")
                for half, n_t, base in ((0, g["lo"], 0),
                                        (1, g["hi"], g["lo"])):
                    tbl = xlo if half == 0 else xhi
                    # a small leading batch lets the first scores start
                    # as soon as 4 tiles land instead of a full batch
                    cuts = list(range(0, n_t, GMAX))
                    if first_small and half == 0 and n_t > 4:
                        cuts = [0, 4] + [c for c in cuts if c > 4]
                    for ci, q0 in enumerate(cuts):
                        q1 = cuts[ci + 1] if ci + 1 < len(cuts) else min(
                            q0 + GMAX, n_t)
                        b0t = base + q0
                        g0 = tg0 + b0t
                        nc.gpsimd.dma_gather(
                            out_ap=gbuf[:, b0t:base + q1, :],
                            in_ap=tbl[:],
                            idxs_ap=idx_sb[:, g0 * 8:(g0 + q1 - q0) * 8],
                            num_idxs=(q1 - q0) * 128,
                            num_idxs_reg=(q1 - q0) * 128,
                            elem_size=128,
                            single_packet=False,
                        )
                gbufs[id(g)] = gbuf

            # zero-fill output for empty groups up front
            for g in (groups if do_main else []):
                if g["lo"] + g["hi"] == 0:
                    nbk = g["b1"] - g["b0"]
                    aggg = agpool.tile([128, nbk * 128], BF16, tag="aggg")
                    nc.vector.memset(aggg[:], 0.0)
                    nc.sync.dma_start(
                        out[:, g["b0"] * 128:(g["b0"] + nbk) * 128], aggg[:])

            PREFETCH = 1  # groups of gathers in flight ahead (gbuf bufs-1)
            for wi, g in enumerate(work[:PREFETCH]):
                emit_gathers(g)
            # consts and slot stream issue after the pipeline-critical
            # first gathers (each DMA issue serializes ~0.6us on HWDGE)
            nc.sync.dma_start(C[:], cst[:])
            nc.sync.dma_start(slt_sb[:], slt[:])
            for wi, g in enumerate(work):
                if wi + PREFETCH < len(work):
                    emit_gathers(work[wi + PREFETCH])
                TG = g["lo"] + g["hi"]
                nbk = g["b1"] - g["b0"]
                t_glob = tg_of[id(g)]
                gbuf = gbufs.pop(id(g))
                aggg = agpool.tile([128, nbk * 128], BF16, tag="aggg")

                # p01 stream for this group's ACT-assigned tiles
                # (variable widths, contiguous in the global ACT ordering)
                acts = [t for t in range(TG) if eng_map[t_glob + t] == ENG_ACT]
                nact = len(acts)
                if nact > 0:
                    ai0 = int(act_pos[t_glob + acts[0]])
                    w0 = int(act_woff[ai0])
                    w1 = int(act_woff[ai0 + nact])
                    p01_sb = grpool.tile([128, w1 - w0], FP8, tag="p01sb")
                    nc.sync.dma_start(p01_sb[:], p01[:, w0:w1])

                if not do_compute:
                    # still consume gbuf so Tile keeps the gathers
                    sink = flpool.tile([128, 1], F32, tag="sink")
                    nc.vector.tensor_scalar(
                        out=sink[:], in0=gbuf[:, 0, 0:1], scalar1=1.0,
                        scalar2=None, op0=OP.mult)
                    nc.sync.dma_start(out[0:128, 0:1], sink[:].bitcast(BF16)[:, 0:1])
                    continue
                # scores for all tiles of the group
                sg = grpool.tile([128, TG], F32, tag="sg")
                eg = grpool.tile([128, TG], F32, tag="eg")
                for t in range(TG):
                    scr = scpool.tile([128, 128], F32, tag="scr",
                                      name="scr_v")
                    sc_eng = (nc.gpsimd if (t_glob + t) % 100 <
                              SC_POOL * 100 else nc.vector)
                    sc_eng.scalar_tensor_tensor(
                        out=scr[:], in0=gbuf[:, t, :], scalar=1.0,
                        in1=w_ap, op0=OP.mult, op1=OP.mult,
                        accum_out=sg[:, t:t + 1])
                    # per-16-tile exp keeps the pipeline fine-grained
                    if t % 16 == 15 or t == TG - 1:
                        lo8 = (t // 16) * 16
                        nc.scalar.activation(out=eg[:, lo8:t + 1],
                                             in_=sg[:, lo8:t + 1],
                                             func=AF.Exp,
                                             bias=0.0, scale=1.0)

                # per-bucket psum tiles: cols 0:128 numerator, col 128 denom
                psums = {}
                for j in range(nbk):
                    if sched["Tb"][g["b0"] + j] > 0:
                        psums[j] = pnum.tile([128, 129], F32, tag="pn",
                                             name=f"pn_{g['b0']}_{j}")

                for t, ti in enumerate(g["tiles"]):
                    j = ti["j"]
                    tg = t_glob + t
                    p0 = int(wp0[tg])
                    W = int(ww[tg])
                    Pp = ppool.tile([128, W], F32, tag=f"pp{W}")
                    ek = eng_map[tg]
                    if ek == ENG_ACT:
                        # ACT path: P' = host-built 0/1 one-hot (fp8,
                        # exact, window-relative) scaled by e in one Copy
                        ai = int(act_pos[tg])
                        o0 = int(act_woff[ai]) - int(
                            act_woff[int(act_pos[t_glob + acts[0]])])
                        nc.scalar.activation(
                            out=Pp[:], in_=p01_sb[:, o0:o0 + W],
                            func=AF.Copy, scale=eg[:, t:t + 1])
                    else:
                        nc.vector.tensor_scalar(
                            out=Pp[:], in0=iota_ap[:, 0:W],
                            scalar1=slt_sb[:, tg:tg + 1],
                            scalar2=eg[:, t:t + 1],
                            op0=OP.is_equal, op1=OP.mult)
                    # slot-sorted tiles cover a narrow aligned window
                    # [p0, p0+W) of the bucket's 128 slots; the matmul
                    # writes just those psum partitions. The bucket's
                    # first tile is forced to the full window so its
                    # start=True clears every partition of the bank.
                    nc.tensor.matmul(
                        out=psums[j][p0:p0 + W, 0:128], lhsT=Pp[:],
                        rhs=gbuf[:, t, :],
                        start=ti["first"], stop=False)
                    nc.tensor.matmul(
                        out=psums[j][p0:p0 + W, 128:129], lhsT=Pp[:],
                        rhs=ones_ap,
                        start=False, stop=ti["last"])

                # flush group: per-bucket reciprocal + scale into the
                # group's bf16 slot-major agg tile, then one DMA out
                for j in range(nbk):
                    if j in psums:
                        # no epsilon guard: slots with zero in-degree give
                        # den=0 -> inf/NaN rows, which the host overwrites
                        # with zeros (it knows the in-degrees from
                        # edge_index); skipping the max() op saves ~6us DVE
                        rcp = flpool.tile([128, 1], F32, tag="rcp")
                        nc.vector.reciprocal(out=rcp[:],
                                             in_=psums[j][:, 128:129])
                        nc.scalar.activation(
                            out=aggg[:, j * 128:(j + 1) * 128],
                            in_=psums[j][:, 0:128],
                            func=AF.Copy, scale=rcp[:, 0:1])
                    else:
                        nc.vector.memset(aggg[:, j * 128:(j + 1) * 128], 0.0)
                nc.sync.dma_start(
                    out[:, g["b0"] * 128:(g["b0"] + nbk) * 128], aggg[:])

    nc.compile()
    return nc


def make_in_maps(cfg, sched, per_core, x, gate_w, gate_b):
    x = np.asarray(x, np.float32)
    consts = build_consts(cfg, gate_w, gate_b)
    in_maps = []
    for core in range(cfg.NC):
        in_maps.append({
            "xlo": x[:cfg.LO_MAX],
            "xhi": x[cfg.HI_BASE:],
            "idx": per_core[core]["idx"],
            "slt": per_core[core]["slots"],
            "p01": per_core[core]["p01"],
            "cst": consts,
        })
    return in_maps


def _kernel_impl(x, gate_w, gate_b, edge_index, cfg=None, return_nc=False):
    from concourse.bass_utils import run_bass_kernel_spmd
    if cfg is None:
        cfg = Config()
    sched, per_core = build_schedule(cfg, edge_index[0], edge_index[1])
    nc = build_program(cfg, sched)
    in_maps = make_in_maps(cfg, sched, per_core, x, gate_w, gate_b)
    res = run_bass_kernel_spmd(nc, in_maps, core_ids=list(range(cfg.NC)))
    perm = sched["perm"]
    outp = np.zeros((cfg.N, 256), np.float32)
    outp[:, 0:128] = x
    indeg = np.bincount(np.asarray(edge_index[1], np.int64),
                        minlength=cfg.N)
    for core in range(cfg.NC):
        # o: [128, NBUK*128] bf16 slot-major -> [NBUK, 128, 128] agg
        o = np.asarray(res.results[core]["out"], dtype=np.float32)
        o = o.reshape(128, cfg.NBUK, 128).transpose(1, 0, 2)
        base = core * cfg.NPC
        for k in range(cfg.NBUK):
            b = int(perm[core, k])
            v = min(128, cfg.NPC - b * 128)
            outp[base + b * 128:base + b * 128 + v, 128:256] = o[k, :v]
    outp[indeg == 0, 128:256] = 0.0
    if return_nc:
        return outp, nc
    return outp


def kernel(**inputs):
    """Harness entry: full unsharded inputs -> full [50000, 256] f32 output.

    Shards edges by destination-node range across the 8 NeuronCores
    (each core computes its 6250-node output slice fully locally),
    compiles the Bass program, and runs it via run_bass_kernel_spmd.
    """
    x = np.ascontiguousarray(np.asarray(inputs["x"], np.float32))
    gate_w = np.asarray(inputs["gate_w"], np.float32)
    gate_b = np.asarray(inputs["gate_b"], np.float32)
    edge_index = np.asarray(inputs["edge_index"])
    return _kernel_impl(x, gate_w, gate_b, edge_index)


# revision 56
# speedup vs baseline: 1.0316x; 1.0014x over previous
"""AttentionalAggregation GNN kernel for 8 TRN2 NeuronCores.

Strategy: edges sorted by destination bucket on host; core m owns nodes
[m*NPC, (m+1)*NPC) and computes its output slice fully locally (no
collectives). Per 128-edge tile:
  - dma_gather x[src] rows (512B each) from lo/hi half tables (int16 idx)
  - score_e = sum_d V[e,d]*w[d]  (scalar_tensor_tensor accum)
  - e = exp(score + b)           (ACT)
  - P'[e,s] = (iota_s == slot_e) * e_e   (built on DVE, Pool or ACT,
    split to balance engine load; ACT path reads a host-built 0/1
    one-hot in fp8 and scales by e in one Copy op)
  - psum[bucket][:,0:128] += P'.T @ V_t   (plain f32 matmul: hardware
    f32r is tf32-grade and fails the accuracy budget)
  - psum[bucket][:,128]   += P'.T @ ones  (denominator)
Flush per group: reciprocal of denominator, scale, write agg as bf16 in
slot-major layout; host unpermutes, casts to f32 and concats with x.
"""

import math
import numpy as np

import concourse.bass as bass
import concourse.mybir as mybir
import concourse.tile as tile
from concourse import bacc

F32 = mybir.dt.float32
F32R = mybir.dt.float32r
BF16 = mybir.dt.bfloat16
FP8 = mybir.dt.float8e4
I16 = mybir.dt.int16
AF = mybir.ActivationFunctionType
OP = mybir.AluOpType

# engine assignment for building P' (one-hot * e) per tile
ENG_DVE = 0
ENG_POOL = 1
ENG_ACT = 2
# shares (DVE, POOL, ACT) of P'-build work; ACT tiles need the p01 stream.
# Pool-engine P' builds stall the in-order Pool queue behind cross-engine
# waits (gathers share it), so the Pool share stays 0.
PP_SHARES = (0.28, 0.0, 0.72)
# fraction of score STTs routed to the Pool engine. Keep 0: any non-SWDGE
# work on the in-order Pool engine delays later groups' descriptor
# generation and starves the DMA engines.
SC_POOL = 0.0


class Config:
    def __init__(self, N=50000, E=640000, D=128, NC=8, GROUP=2):
        assert D == 128
        self.N, self.E, self.D, self.NC = N, E, D, NC
        self.NPC = N // NC          # nodes per core
        assert self.NPC * NC == N
        # overlapping lo/hi gather tables (int16 index limit 32768 rows);
        # sources in the overlap may be assigned to either run, letting the
        # host pad the lo run to a tile boundary with real edges
        self.LO_MAX = min(32768, N)
        self.HI_BASE = max(0, N - 32768)
        self.NBUK = math.ceil(self.NPC / 128)   # buckets per core
        self.TAIL = self.NPC - (self.NBUK - 1) * 128  # rows in last bucket
        self.GROUP = GROUP
        self.NG = math.ceil(self.NBUK / GROUP)


def assign_engines(TOT):
    """Per-tile engine for the P' build, interleaved to the target shares."""
    eng = np.zeros(TOT, np.int8)
    acc = [0.0, 0.0, 0.0]
    for t in range(TOT):
        # pick the engine furthest below its target share
        deficits = [PP_SHARES[k] * (t + 1) - acc[k] for k in range(3)]
        k = int(np.argmax(deficits))
        eng[t] = k
        acc[k] += 1.0
    return eng


def build_schedule(cfg, src, dst):
    """Host-side: sort/pad edges into a static per-tile schedule uniform
    across cores. Returns (sched, per_core) where sched is the static
    structure (identical across cores) and per_core has the data arrays."""
    N, NC, NPC, NBUK, GROUP = (
        cfg.N, cfg.NC, cfg.NPC, cfg.NBUK, cfg.GROUP)
    LO_MAX, HI_BASE = cfg.LO_MAX, cfg.HI_BASE

    src = np.asarray(src, np.int64)
    dst = np.asarray(dst, np.int64)
    c = dst // NPC
    r = dst % NPC
    lb = r // 128
    slot = r % 128

    order = np.lexsort((src, lb, c))
    src_s, lb_s, slot_s, c_s = (
        src[order], lb[order], slot[order], c[order])

    key = c_s * NBUK + lb_s
    cnt = np.bincount(key, minlength=NC * NBUK).reshape(NC, NBUK)
    starts = np.zeros(NC * NBUK + 1, np.int64)
    np.cumsum(cnt.reshape(-1), out=starts[1:])

    # within each (core, bucket) slice (sorted by src), edges below
    # HI_BASE must use the lo table, edges >= LO_MAX must use hi, and the
    # overlap is flexible: cut the slice to fill lo tiles exactly
    n_lo_min = np.zeros((NC, NBUK), np.int64)
    n_lo_cap = np.zeros((NC, NBUK), np.int64)
    for cc in range(NC):
        for b in range(NBUK):
            k = cc * NBUK + b
            sl = src_s[starts[k]:starts[k + 1]]
            n_lo_min[cc, b] = np.searchsorted(sl, HI_BASE)
            n_lo_cap[cc, b] = np.searchsorted(sl, LO_MAX)
    # per-core needs, then sort each core's buckets by total tiles
    # descending so position-wise maxima across cores are tight
    t_lo_c = np.ceil(n_lo_min / 128.0).astype(np.int64)       # [NC, NBUK]
    lo_cap_pos = n_lo_cap
    t_hi_c = np.ceil(np.maximum(cnt - np.minimum(128 * t_lo_c, lo_cap_pos),
                                0) / 128.0).astype(np.int64)
    tot_c = t_lo_c + t_hi_c
    perm = np.argsort(-tot_c, axis=1, kind="stable")          # [NC, NBUK]
    ar = np.arange(NC)[:, None]
    T_lo = t_lo_c[ar, perm].max(axis=0)                       # [NBUK] by pos
    lo_count_pos = np.minimum(128 * T_lo[None, :], n_lo_cap[ar, perm])
    T_hi = np.ceil((cnt[ar, perm] - lo_count_pos) / 128.0
                   ).astype(np.int64).max(axis=0)
    Th = np.stack([T_lo, T_hi], axis=1)  # [NBUK, 2] by position
    # scatter position-based lo counts back to per-(core,bucket)
    lo_count = np.zeros_like(cnt)
    np.put_along_axis(lo_count, perm, lo_count_pos, axis=1)
    # move the smallest bucket to position 0: paired with a single-bucket
    # first group below, it halves the ramp-critical first gather batch
    pi = np.r_[NBUK - 1, NBUK - 2, np.arange(NBUK - 2)]
    perm = perm[:, pi]
    Th = Th[pi]
    Tb = Th.sum(axis=1)

    # static tile stream: per group g: [lo tiles of buckets][hi tiles]
    # each entry: (bucket_local_index_in_group j, bucket b, first, last)
    sizes = [1, 1]
    left = NBUK - 2
    while left > 0:
        take = min(GROUP, left)
        sizes.append(take)
        left -= take
    bounds = np.cumsum([0] + sizes)
    groups = []
    for g in range(len(sizes)):
        b0, b1 = int(bounds[g]), int(bounds[g + 1])
        tiles = []
        for h in (0, 1):
            for b in range(b0, b1):
                nt = int(Th[b, h])
                for t in range(nt):
                    pos = t if h == 0 else int(Th[b, 0]) + t
                    first = pos == 0
                    last = pos == int(Tb[b]) - 1
                    tiles.append(dict(j=b - b0, b=b, first=first, last=last))
        lo_tiles = int(Th[b0:b1, 0].sum())
        hi_tiles = int(Th[b0:b1, 1].sum())
        groups.append(dict(b0=b0, b1=b1, lo=lo_tiles, hi=hi_tiles,
                           tiles=tiles))
    TOT = sum(g["lo"] + g["hi"] for g in groups)

    # engine assignment per global tile; ACT tiles get p01 stream slots
    eng = assign_engines(TOT)
    act_idx = np.nonzero(eng == ENG_ACT)[0]
    act_pos = np.full(TOT, -1, np.int64)    # tile -> index among ACT tiles
    for i, t in enumerate(act_idx):
        act_pos[t] = i
    NACT = len(act_idx)

    # pass A: per-tile slot-span unions across cores (edges re-sorted by
    # slot within each (bucket, half) run so a tile covers a narrow
    # contiguous window; matmul partition offsets must be 32-aligned with
    # legal widths {32 @0/32/64/96, 64 @0/64, 128 @0})
    span_lo = np.full(TOT, 255, np.int64)
    span_hi = np.full(TOT, -1, np.int64)
    for core in range(NC):
        t_glob = 0
        for g in groups:
            for h in (0, 1):
                for pos in range(g["b0"], g["b1"]):
                    b = int(perm[core, pos])
                    k = core * NBUK + b
                    s0, s1 = starts[k], starts[k + 1]
                    cut = s0 + lo_count[core, b]
                    e_slot = (slot_s[s0:cut] if h == 0
                              else slot_s[cut:s1])
                    nt = int(Th[pos, h])
                    if nt > 0 and len(e_slot) > 0:
                        sl = np.sort(e_slot)
                        for kk in range(nt):
                            seg = sl[kk * 128:(kk + 1) * 128]
                            if len(seg):
                                t = t_glob + kk
                                span_lo[t] = min(span_lo[t], int(seg[0]))
                                span_hi[t] = max(span_hi[t], int(seg[-1]))
                    t_glob += nt

    def legal_window(a, b):
        # base partitions allowed by the AP layer: {0, 32, 64} only
        if b < 0:
            return 0, 32
        p = 32 * (a // 32)
        if p <= 64 and b - p < 32:
            return p, 32
        p = 64 * (a // 64)
        if b - p < 64:
            return p, 64
        return 0, 128

    win = [legal_window(int(span_lo[t]), int(span_hi[t]))
           for t in range(TOT)]
    # the bucket-opening tile (start=True) must cover all 128 partitions:
    # hardware clears the full bank row only for partitions the matmul
    # writes (validated: narrow first tiles corrupt untouched partitions).
    # Swap a naturally-full-window slice to the front of the bucket's
    # first run when one exists; otherwise widen the first tile.
    slice_perm = {}  # (pos, h) -> permutation of the run's slice indices
    t_glob = 0
    for g in groups:
        for h in (0, 1):
            for pos in range(g["b0"], g["b1"]):
                nt = int(Th[pos, h])
                if nt == 0:
                    continue
                is_first_run = (h == 0) or int(Th[pos, 0]) == 0
                if is_first_run:
                    p = list(range(nt))
                    kfull = next((k for k in range(nt)
                                  if win[t_glob + k] == (0, 128)), None)
                    if kfull is not None and kfull != 0:
                        p[0], p[kfull] = p[kfull], p[0]
                        slice_perm[(pos, h)] = p
                        w2 = [win[t_glob + k] for k in p]
                        for k in range(nt):
                            win[t_glob + k] = w2[k]
                    elif kfull is None:
                        win[t_glob] = (0, 128)
                t_glob += nt
    wp0 = np.array([w[0] for w in win], np.int64)
    ww = np.array([w[1] for w in win], np.int64)
    # variable-width p01 offsets for ACT tiles
    act_woff = np.zeros(NACT + 1, np.int64)
    np.cumsum(ww[act_idx], out=act_woff[1:])
    P01W = int(act_woff[-1])

    # per-core data arrays
    per_core = []
    for core in range(NC):
        idx_stream = np.zeros(TOT * 128, np.int16)
        slot_stream = np.full((128, TOT), 255.0, np.float32)
        t_glob = 0
        for g in groups:
            for h in (0, 1):
                for pos in range(g["b0"], g["b1"]):
                    b = int(perm[core, pos])
                    k = core * NBUK + b
                    s0, s1 = starts[k], starts[k + 1]
                    cut = s0 + lo_count[core, b]
                    if h == 0:
                        e_src = src_s[s0:cut]
                        e_slot = slot_s[s0:cut]
                    else:
                        e_src = src_s[cut:s1] - HI_BASE
                        e_slot = slot_s[cut:s1]
                    o2 = np.argsort(e_slot, kind="stable")
                    e_src = e_src[o2]
                    e_slot = e_slot[o2]
                    sp = slice_perm.get((pos, h))
                    if sp is not None and len(e_src) > 0:
                        segs = [e_src[k * 128:(k + 1) * 128] for k in sp]
                        segt = [e_slot[k * 128:(k + 1) * 128] for k in sp]
                        e_src = np.concatenate(segs)
                        e_slot = np.concatenate(segt)
                    n = len(e_src)
                    nt = int(Th[pos, h])
                    base = t_glob * 128
                    if n > 0:
                        idx_stream[base:base + n] = e_src.astype(np.int16)
                        fl = np.full(nt * 128, 255.0, np.float32)
                        fl[:n] = e_slot.astype(np.float32)
                        slot_stream[:, t_glob:t_glob + nt] = (
                            fl.reshape(nt, 128).T)
                    t_glob += nt
        assert t_glob == TOT
        # window-relative slots (255 padding stays out of range for any W)
        real = slot_stream < 255.0
        slot_stream = slot_stream - wp0[None, :].astype(np.float32) * real
        # wrap-16 the index stream, replicate across the 8 16-part groups
        wrapped = idx_stream.reshape(-1, 16).T  # [16, TOT*8]
        idx_arr = np.tile(wrapped, (8, 1)).copy()  # [128, TOT*8]
        # host-built 0/1 one-hot for ACT tiles, window-relative and
        # variable-width (width ww[t] per tile, concatenated)
        f8 = mybir.dt.np(FP8)
        p01 = np.zeros((128, max(P01W, 1)), f8)
        for i, t in enumerate(act_idx):
            W = int(ww[t])
            off = int(act_woff[i])
            oh = (slot_stream[:, t][:, None] ==
                  np.arange(W, dtype=np.float32)[None, :])
            p01[:, off:off + W] = oh.astype(f8)
        per_core.append(dict(idx=idx_arr, slots=slot_stream, p01=p01))

    sched = dict(groups=groups, TOT=TOT, Th=Th, Tb=Tb, perm=perm,
                 eng=eng, act_pos=act_pos, NACT=NACT,
                 wp0=wp0, ww=ww, act_woff=act_woff, P01W=P01W)
    return sched, per_core


def build_consts(cfg, gate_w, gate_b):
    """[128, 258] f32: iota | W_bcast | ones | b"""
    C = np.zeros((128, 258), np.float32)
    C[:, 0:128] = np.arange(128, dtype=np.float32)[None, :]
    C[:, 128:256] = np.asarray(gate_w, np.float32).reshape(1, 128)
    C[:, 256] = 1.0
    # gate bias is a uniform score shift and cancels in the softmax, so
    # the device never applies it; column 257 is kept but unused
    C[:, 257] = np.float32(np.asarray(gate_b).reshape(-1)[0])
    return C


def build_program(cfg, sched, do_main=True, do_compute=True):
    nc = bacc.Bacc("TRN2", num_devices=cfg.NC,
                   dynamic_dma_scratch_size=49152)
    NPC, NBUK, GROUP = cfg.NPC, cfg.NBUK, cfg.GROUP
    TOT = sched["TOT"]
    groups = sched["groups"]
    eng_map = sched["eng"]
    act_pos = sched["act_pos"]
    NACT = sched["NACT"]
    wp0 = sched["wp0"]
    ww = sched["ww"]
    act_woff = sched["act_woff"]

    xlo = nc.dram_tensor("xlo", [cfg.LO_MAX, 128], F32,
                         kind="ExternalInput")
    xhi = nc.dram_tensor("xhi", [cfg.N - cfg.HI_BASE, 128], F32,
                         kind="ExternalInput")
    idx = nc.dram_tensor("idx", [128, TOT * 8], I16, kind="ExternalInput")
    slt = nc.dram_tensor("slt", [128, TOT], F32, kind="ExternalInput")
    p01 = nc.dram_tensor("p01", [128, max(sched["P01W"], 1)], FP8,
                         kind="ExternalInput")
    cst = nc.dram_tensor("cst", [128, 258], F32, kind="ExternalInput")
    # agg output, slot-major: out[p, b*128 + d] = agg[b*128 + p, d], bf16
    out = nc.dram_tensor("out", [128, NBUK * 128], BF16,
                         kind="ExternalOutput")

    with tile.TileContext(nc) as tc:
        with (
            tc.tile_pool(name="const", bufs=1) as cpool,
            tc.tile_pool(name="meta", bufs=1) as mpool,
            tc.tile_pool(name="gather", bufs=4) as gpool,
            tc.tile_pool(name="sc", bufs=6) as scpool,
            tc.tile_pool(name="grp", bufs=3) as grpool,
            tc.tile_pool(name="pp", bufs=8) as ppool,
            tc.tile_pool(name="fl", bufs=4) as flpool,
            tc.tile_pool(name="ag", bufs=3) as agpool,
            tc.tile_pool(name="pnum", bufs=8, space="PSUM") as pnum,
        ):
            C = cpool.tile([128, 258], F32)
            iota_ap = C[:, 0:128]
            w_ap = C[:, 128:256]
            ones_ap = C[:, 256:257]
            b_ap = C[:, 257:258]

            # the idx stream is uploaded in chunks emitted just ahead of
            # the gathers that read them: a single monolithic upload held
            # the first gather batch (and the whole pipeline) back ~10us
            idx_sb = mpool.tile([128, TOT * 8], I16)
            slt_sb = mpool.tile([128, TOT], F32)

            # ---- main loop over groups (gathers prefetched one group
            # ahead so the in-order Pool queue never parks later groups'
            # gathers behind P'-builds that wait on cross-engine deps) ----
            # each dma_gather is capped at GMAX tiles: the SWDGE ring
            # holds scratch/16 descriptors and one instruction's
            # descriptor set must fit entirely
            GMAX = 8
            work = [g for g in (groups if do_main else [])
                    if g["lo"] + g["hi"] > 0]
            tg_of = {}
            acc = 0
            for g in (groups if do_main else []):
                tg_of[id(g)] = acc
                acc += g["lo"] + g["hi"]

            gbufs = {}

            # idx chunks cover CHUNK_GROUPS consecutive work groups (kept
            # >=512B per partition to dodge the small-transfer penalty)
            CHUNK_GROUPS = 3
            chunk_of = {}
            chunk_rng = []
            for wi, g in enumerate(work):
                ci = wi // CHUNK_GROUPS
                chunk_of[id(g)] = ci
                tg0 = tg_of[id(g)]
                TGg = g["lo"] + g["hi"]
                if ci == len(chunk_rng):
                    chunk_rng.append([tg0, tg0 + TGg])
                else:
                    chunk_rng[ci][1] = tg0 + TGg
            idx_chunk_done = set()

            def emit_idx_chunk(ci):
                if ci in idx_chunk_done:
                    return
                idx_chunk_done.add(ci)
                r0, r1 = chunk_rng[ci]
                nc.sync.dma_start(idx_sb[:, r0 * 8:r1 * 8],
                                  idx[:, r0 * 8:r1 * 8])

            def emit_gathers(g, first_small=False):
                emit_idx_chunk(chunk_of[id(g)])
                TG = g["lo"] + g["hi"]
                tg0 = tg_of[id(g)]
                # extra zeroed tile at the end lets the last tile's matmul
                # read a 256-wide rhs (junk cols accumulate into psum cols
                # 128:256, never read)
                gbuf = gpool.tile([128, TG, 128], F32, tag="gbuf")
                for half, n_t, base in ((0, g["lo"], 0),
                                        (1, g["hi"], g["lo"])):
                    tbl = xlo if half == 0 else xhi
                    # a small leading batch lets the first scores start
                    # as soon as 4 tiles land instead of a full batch
                    cuts = list(range(0, n_t, GMAX))
                    if first_small and half == 0 and n_t > 4:
                        cuts = [0, 4] + [c for c in cuts if c > 4]
                    for ci, q0 in enumerate(cuts):
                        q1 = cuts[ci + 1] if ci + 1 < len(cuts) else min(
                            q0 + GMAX, n_t)
                        b0t = base + q0
                        g0 = tg0 + b0t
                        nc.gpsimd.dma_gather(
                            out_ap=gbuf[:, b0t:base + q1, :],
                            in_ap=tbl[:],
                            idxs_ap=idx_sb[:, g0 * 8:(g0 + q1 - q0) * 8],
                            num_idxs=(q1 - q0) * 128,
                            num_idxs_reg=(q1 - q0) * 128,
                            elem_size=128,
                            single_packet=False,
                        )
                gbufs[id(g)] = gbuf

            # zero-fill output for empty groups up front
            for g in (groups if do_main else []):
                if g["lo"] + g["hi"] == 0:
                    nbk = g["b1"] - g["b0"]
                    aggg = agpool.tile([128, nbk * 128], BF16, tag="aggg")
                    nc.vector.memset(aggg[:], 0.0)
                    nc.sync.dma_start(
                        out[:, g["b0"] * 128:(g["b0"] + nbk) * 128], aggg[:])

            PREFETCH = 2  # groups of gathers in flight ahead (gbuf bufs-1)
            for wi, g in enumerate(work[:PREFETCH]):
                emit_gathers(g)
            # consts and slot stream issue after the pipeline-critical
            # first gathers (each DMA issue serializes ~0.6us on HWDGE)
            nc.sync.dma_start(C[:], cst[:])
            nc.sync.dma_start(slt_sb[:], slt[:])
            for wi, g in enumerate(work):
                if wi + PREFETCH < len(work):
                    emit_gathers(work[wi + PREFETCH])
                TG = g["lo"] + g["hi"]
                nbk = g["b1"] - g["b0"]
                t_glob = tg_of[id(g)]
                gbuf = gbufs.pop(id(g))
                aggg = agpool.tile([128, nbk * 128], BF16, tag="aggg")

                # p01 stream for this group's ACT-assigned tiles
                # (variable widths, contiguous in the global ACT ordering)
                acts = [t for t in range(TG) if eng_map[t_glob + t] == ENG_ACT]
                nact = len(acts)
                if nact > 0:
                    ai0 = int(act_pos[t_glob + acts[0]])
                    w0 = int(act_woff[ai0])
                    w1 = int(act_woff[ai0 + nact])
                    p01_sb = grpool.tile([128, w1 - w0], FP8, tag="p01sb")
                    nc.sync.dma_start(p01_sb[:], p01[:, w0:w1])

                if not do_compute:
                    # still consume gbuf so Tile keeps the gathers
                    sink = flpool.tile([128, 1], F32, tag="sink")
                    nc.vector.tensor_scalar(
                        out=sink[:], in0=gbuf[:, 0, 0:1], scalar1=1.0,
                        scalar2=None, op0=OP.mult)
                    nc.sync.dma_start(out[0:128, 0:1], sink[:].bitcast(BF16)[:, 0:1])
                    continue
                # scores for all tiles of the group
                sg = grpool.tile([128, TG], F32, tag="sg")
                eg = grpool.tile([128, TG], F32, tag="eg")
                for t in range(TG):
                    scr = scpool.tile([128, 128], F32, tag="scr",
                                      name="scr_v")
                    sc_eng = (nc.gpsimd if (t_glob + t) % 100 <
                              SC_POOL * 100 else nc.vector)
                    sc_eng.scalar_tensor_tensor(
                        out=scr[:], in0=gbuf[:, t, :], scalar=1.0,
                        in1=w_ap, op0=OP.mult, op1=OP.mult,
                        accum_out=sg[:, t:t + 1])
                    # per-16-tile exp keeps the pipeline fine-grained
                    if t % 16 == 15 or t == TG - 1:
                        lo8 = (t // 16) * 16
                        nc.scalar.activation(out=eg[:, lo8:t + 1],
                                             in_=sg[:, lo8:t + 1],
                                             func=AF.Exp,
                                             bias=0.0, scale=1.0)

                # per-bucket psum tiles: cols 0:128 numerator, col 128 denom
                psums = {}
                for j in range(nbk):
                    if sched["Tb"][g["b0"] + j] > 0:
                        psums[j] = pnum.tile([128, 129], F32, tag="pn",
                                             name=f"pn_{g['b0']}_{j}")

                for t, ti in enumerate(g["tiles"]):
                    j = ti["j"]
                    tg = t_glob + t
                    p0 = int(wp0[tg])
                    W = int(ww[tg])
                    Pp = ppool.tile([128, W], F32, tag=f"pp{W}")
                    ek = eng_map[tg]
                    if ek == ENG_ACT:
                        # ACT path: P' = host-built 0/1 one-hot (fp8,
                        # exact, window-relative) scaled by e in one Copy
                        ai = int(act_pos[tg])
                        o0 = int(act_woff[ai]) - int(
                            act_woff[int(act_pos[t_glob + acts[0]])])
                        nc.scalar.activation(
                            out=Pp[:], in_=p01_sb[:, o0:o0 + W],
                            func=AF.Copy, scale=eg[:, t:t + 1])
                    else:
                        nc.vector.tensor_scalar(
                            out=Pp[:], in0=iota_ap[:, 0:W],
                            scalar1=slt_sb[:, tg:tg + 1],
                            scalar2=eg[:, t:t + 1],
                            op0=OP.is_equal, op1=OP.mult)
                    # slot-sorted tiles cover a narrow aligned window
                    # [p0, p0+W) of the bucket's 128 slots; the matmul
                    # writes just those psum partitions. The bucket's
                    # first tile is forced to the full window so its
                    # start=True clears every partition of the bank.
                    nc.tensor.matmul(
                        out=psums[j][p0:p0 + W, 0:128], lhsT=Pp[:],
                        rhs=gbuf[:, t, :],
                        start=ti["first"], stop=False)
                    nc.tensor.matmul(
                        out=psums[j][p0:p0 + W, 128:129], lhsT=Pp[:],
                        rhs=ones_ap,
                        start=False, stop=ti["last"])

                # flush group: per-bucket reciprocal + scale into the
                # group's bf16 slot-major agg tile, then one DMA out
                for j in range(nbk):
                    if j in psums:
                        # no epsilon guard: slots with zero in-degree give
                        # den=0 -> inf/NaN rows, which the host overwrites
                        # with zeros (it knows the in-degrees from
                        # edge_index); skipping the max() op saves ~6us DVE
                        rcp = flpool.tile([128, 1], F32, tag="rcp")
                        nc.vector.reciprocal(out=rcp[:],
                                             in_=psums[j][:, 128:129])
                        nc.scalar.activation(
                            out=aggg[:, j * 128:(j + 1) * 128],
                            in_=psums[j][:, 0:128],
                            func=AF.Copy, scale=rcp[:, 0:1])
                    else:
                        nc.vector.memset(aggg[:, j * 128:(j + 1) * 128], 0.0)
                nc.sync.dma_start(
                    out[:, g["b0"] * 128:(g["b0"] + nbk) * 128], aggg[:])

    nc.compile()
    return nc


def make_in_maps(cfg, sched, per_core, x, gate_w, gate_b):
    x = np.asarray(x, np.float32)
    consts = build_consts(cfg, gate_w, gate_b)
    in_maps = []
    for core in range(cfg.NC):
        in_maps.append({
            "xlo": x[:cfg.LO_MAX],
            "xhi": x[cfg.HI_BASE:],
            "idx": per_core[core]["idx"],
            "slt": per_core[core]["slots"],
            "p01": per_core[core]["p01"],
            "cst": consts,
        })
    return in_maps


def _kernel_impl(x, gate_w, gate_b, edge_index, cfg=None, return_nc=False):
    from concourse.bass_utils import run_bass_kernel_spmd
    if cfg is None:
        cfg = Config()
    sched, per_core = build_schedule(cfg, edge_index[0], edge_index[1])
    nc = build_program(cfg, sched)
    in_maps = make_in_maps(cfg, sched, per_core, x, gate_w, gate_b)
    res = run_bass_kernel_spmd(nc, in_maps, core_ids=list(range(cfg.NC)))
    perm = sched["perm"]
    outp = np.zeros((cfg.N, 256), np.float32)
    outp[:, 0:128] = x
    indeg = np.bincount(np.asarray(edge_index[1], np.int64),
                        minlength=cfg.N)
    for core in range(cfg.NC):
        # o: [128, NBUK*128] bf16 slot-major -> [NBUK, 128, 128] agg
        o = np.asarray(res.results[core]["out"], dtype=np.float32)
        o = o.reshape(128, cfg.NBUK, 128).transpose(1, 0, 2)
        base = core * cfg.NPC
        for k in range(cfg.NBUK):
            b = int(perm[core, k])
            v = min(128, cfg.NPC - b * 128)
            outp[base + b * 128:base + b * 128 + v, 128:256] = o[k, :v]
    outp[indeg == 0, 128:256] = 0.0
    if return_nc:
        return outp, nc
    return outp


def kernel(**inputs):
    """Harness entry: full unsharded inputs -> full [50000, 256] f32 output.

    Shards edges by destination-node range across the 8 NeuronCores
    (each core computes its 6250-node output slice fully locally),
    compiles the Bass program, and runs it via run_bass_kernel_spmd.
    """
    x = np.ascontiguousarray(np.asarray(inputs["x"], np.float32))
    gate_w = np.asarray(inputs["gate_w"], np.float32)
    gate_b = np.asarray(inputs["gate_b"], np.float32)
    edge_index = np.asarray(inputs["edge_index"])
    return _kernel_impl(x, gate_w, gate_b, edge_index)


# revision 57
# speedup vs baseline: 1.0317x; 1.0002x over previous
"""AttentionalAggregation GNN kernel for 8 TRN2 NeuronCores.

Strategy: edges sorted by destination bucket on host; core m owns nodes
[m*NPC, (m+1)*NPC) and computes its output slice fully locally (no
collectives). Per 128-edge tile:
  - dma_gather x[src] rows (512B each) from lo/hi half tables (int16 idx)
  - score_e = sum_d V[e,d]*w[d]  (scalar_tensor_tensor accum)
  - e = exp(score + b)           (ACT)
  - P'[e,s] = (iota_s == slot_e) * e_e   (built on DVE, Pool or ACT,
    split to balance engine load; ACT path reads a host-built 0/1
    one-hot in fp8 and scales by e in one Copy op)
  - psum[bucket][:,0:128] += P'.T @ V_t   (plain f32 matmul: hardware
    f32r is tf32-grade and fails the accuracy budget)
  - psum[bucket][:,128]   += P'.T @ ones  (denominator)
Flush per group: reciprocal of denominator, scale, write agg as bf16 in
slot-major layout; host unpermutes, casts to f32 and concats with x.
"""

import math
import numpy as np

import concourse.bass as bass
import concourse.mybir as mybir
import concourse.tile as tile
from concourse import bacc

F32 = mybir.dt.float32
F32R = mybir.dt.float32r
BF16 = mybir.dt.bfloat16
FP8 = mybir.dt.float8e4
I16 = mybir.dt.int16
AF = mybir.ActivationFunctionType
OP = mybir.AluOpType

# engine assignment for building P' (one-hot * e) per tile
ENG_DVE = 0
ENG_POOL = 1
ENG_ACT = 2
# shares (DVE, POOL, ACT) of P'-build work; ACT tiles need the p01 stream.
# Pool-engine P' builds stall the in-order Pool queue behind cross-engine
# waits (gathers share it), so the Pool share stays 0.
PP_SHARES = (0.28, 0.0, 0.72)
# fraction of score STTs routed to the Pool engine. Keep 0: any non-SWDGE
# work on the in-order Pool engine delays later groups' descriptor
# generation and starves the DMA engines.
SC_POOL = 0.0


class Config:
    def __init__(self, N=50000, E=640000, D=128, NC=8, GROUP=2):
        assert D == 128
        self.N, self.E, self.D, self.NC = N, E, D, NC
        self.NPC = N // NC          # nodes per core
        assert self.NPC * NC == N
        # overlapping lo/hi gather tables (int16 index limit 32768 rows);
        # sources in the overlap may be assigned to either run, letting the
        # host pad the lo run to a tile boundary with real edges
        self.LO_MAX = min(32768, N)
        self.HI_BASE = max(0, N - 32768)
        self.NBUK = math.ceil(self.NPC / 128)   # buckets per core
        self.TAIL = self.NPC - (self.NBUK - 1) * 128  # rows in last bucket
        self.GROUP = GROUP
        self.NG = math.ceil(self.NBUK / GROUP)


def assign_engines(TOT):
    """Per-tile engine for the P' build, interleaved to the target shares."""
    eng = np.zeros(TOT, np.int8)
    acc = [0.0, 0.0, 0.0]
    for t in range(TOT):
        # pick the engine furthest below its target share
        deficits = [PP_SHARES[k] * (t + 1) - acc[k] for k in range(3)]
        k = int(np.argmax(deficits))
        eng[t] = k
        acc[k] += 1.0
    return eng


def build_schedule(cfg, src, dst):
    """Host-side: sort/pad edges into a static per-tile schedule uniform
    across cores. Returns (sched, per_core) where sched is the static
    structure (identical across cores) and per_core has the data arrays."""
    N, NC, NPC, NBUK, GROUP = (
        cfg.N, cfg.NC, cfg.NPC, cfg.NBUK, cfg.GROUP)
    LO_MAX, HI_BASE = cfg.LO_MAX, cfg.HI_BASE

    src = np.asarray(src, np.int64)
    dst = np.asarray(dst, np.int64)
    c = dst // NPC
    r = dst % NPC
    lb = r // 128
    slot = r % 128

    order = np.lexsort((src, lb, c))
    src_s, lb_s, slot_s, c_s = (
        src[order], lb[order], slot[order], c[order])

    key = c_s * NBUK + lb_s
    cnt = np.bincount(key, minlength=NC * NBUK).reshape(NC, NBUK)
    starts = np.zeros(NC * NBUK + 1, np.int64)
    np.cumsum(cnt.reshape(-1), out=starts[1:])

    # within each (core, bucket) slice (sorted by src), edges below
    # HI_BASE must use the lo table, edges >= LO_MAX must use hi, and the
    # overlap is flexible: cut the slice to fill lo tiles exactly
    n_lo_min = np.zeros((NC, NBUK), np.int64)
    n_lo_cap = np.zeros((NC, NBUK), np.int64)
    for cc in range(NC):
        for b in range(NBUK):
            k = cc * NBUK + b
            sl = src_s[starts[k]:starts[k + 1]]
            n_lo_min[cc, b] = np.searchsorted(sl, HI_BASE)
            n_lo_cap[cc, b] = np.searchsorted(sl, LO_MAX)
    # per-core needs, then sort each core's buckets by total tiles
    # descending so position-wise maxima across cores are tight
    t_lo_c = np.ceil(n_lo_min / 128.0).astype(np.int64)       # [NC, NBUK]
    lo_cap_pos = n_lo_cap
    t_hi_c = np.ceil(np.maximum(cnt - np.minimum(128 * t_lo_c, lo_cap_pos),
                                0) / 128.0).astype(np.int64)
    tot_c = t_lo_c + t_hi_c
    perm = np.argsort(-tot_c, axis=1, kind="stable")          # [NC, NBUK]
    ar = np.arange(NC)[:, None]
    T_lo = t_lo_c[ar, perm].max(axis=0)                       # [NBUK] by pos
    lo_count_pos = np.minimum(128 * T_lo[None, :], n_lo_cap[ar, perm])
    T_hi = np.ceil((cnt[ar, perm] - lo_count_pos) / 128.0
                   ).astype(np.int64).max(axis=0)
    Th = np.stack([T_lo, T_hi], axis=1)  # [NBUK, 2] by position
    # scatter position-based lo counts back to per-(core,bucket)
    lo_count = np.zeros_like(cnt)
    np.put_along_axis(lo_count, perm, lo_count_pos, axis=1)
    # move the smallest bucket to position 0: paired with a single-bucket
    # first group below, it halves the ramp-critical first gather batch
    pi = np.r_[NBUK - 1, NBUK - 2, np.arange(NBUK - 2)]
    perm = perm[:, pi]
    Th = Th[pi]
    Tb = Th.sum(axis=1)

    # static tile stream: per group g: [lo tiles of buckets][hi tiles]
    # each entry: (bucket_local_index_in_group j, bucket b, first, last)
    sizes = [1, 1]
    left = NBUK - 2
    while left > 0:
        take = min(GROUP, left)
        sizes.append(take)
        left -= take
    bounds = np.cumsum([0] + sizes)
    groups = []
    for g in range(len(sizes)):
        b0, b1 = int(bounds[g]), int(bounds[g + 1])
        tiles = []
        for h in (0, 1):
            for b in range(b0, b1):
                nt = int(Th[b, h])
                for t in range(nt):
                    pos = t if h == 0 else int(Th[b, 0]) + t
                    first = pos == 0
                    last = pos == int(Tb[b]) - 1
                    tiles.append(dict(j=b - b0, b=b, first=first, last=last))
        lo_tiles = int(Th[b0:b1, 0].sum())
        hi_tiles = int(Th[b0:b1, 1].sum())
        groups.append(dict(b0=b0, b1=b1, lo=lo_tiles, hi=hi_tiles,
                           tiles=tiles))
    TOT = sum(g["lo"] + g["hi"] for g in groups)

    # engine assignment per global tile; ACT tiles get p01 stream slots
    eng = assign_engines(TOT)
    act_idx = np.nonzero(eng == ENG_ACT)[0]
    act_pos = np.full(TOT, -1, np.int64)    # tile -> index among ACT tiles
    for i, t in enumerate(act_idx):
        act_pos[t] = i
    NACT = len(act_idx)

    # pass A: per-tile slot-span unions across cores (edges re-sorted by
    # slot within each (bucket, half) run so a tile covers a narrow
    # contiguous window; matmul partition offsets must be 32-aligned with
    # legal widths {32 @0/32/64/96, 64 @0/64, 128 @0})
    span_lo = np.full(TOT, 255, np.int64)
    span_hi = np.full(TOT, -1, np.int64)
    for core in range(NC):
        t_glob = 0
        for g in groups:
            for h in (0, 1):
                for pos in range(g["b0"], g["b1"]):
                    b = int(perm[core, pos])
                    k = core * NBUK + b
                    s0, s1 = starts[k], starts[k + 1]
                    cut = s0 + lo_count[core, b]
                    e_slot = (slot_s[s0:cut] if h == 0
                              else slot_s[cut:s1])
                    nt = int(Th[pos, h])
                    if nt > 0 and len(e_slot) > 0:
                        sl = np.sort(e_slot)
                        for kk in range(nt):
                            seg = sl[kk * 128:(kk + 1) * 128]
                            if len(seg):
                                t = t_glob + kk
                                span_lo[t] = min(span_lo[t], int(seg[0]))
                                span_hi[t] = max(span_hi[t], int(seg[-1]))
                    t_glob += nt

    def legal_window(a, b):
        # base partitions allowed by the AP layer: {0, 32, 64} only
        if b < 0:
            return 0, 32
        p = 32 * (a // 32)
        if p <= 64 and b - p < 32:
            return p, 32
        p = 64 * (a // 64)
        if b - p < 64:
            return p, 64
        return 0, 128

    win = [legal_window(int(span_lo[t]), int(span_hi[t]))
           for t in range(TOT)]
    # the bucket-opening tile (start=True) must cover all 128 partitions:
    # hardware clears the full bank row only for partitions the matmul
    # writes (validated: narrow first tiles corrupt untouched partitions).
    # Swap a naturally-full-window slice to the front of the bucket's
    # first run when one exists; otherwise widen the first tile.
    slice_perm = {}  # (pos, h) -> permutation of the run's slice indices
    t_glob = 0
    for g in groups:
        for h in (0, 1):
            for pos in range(g["b0"], g["b1"]):
                nt = int(Th[pos, h])
                if nt == 0:
                    continue
                is_first_run = (h == 0) or int(Th[pos, 0]) == 0
                if is_first_run:
                    p = list(range(nt))
                    kfull = next((k for k in range(nt)
                                  if win[t_glob + k] == (0, 128)), None)
                    if kfull is not None and kfull != 0:
                        p[0], p[kfull] = p[kfull], p[0]
                        slice_perm[(pos, h)] = p
                        w2 = [win[t_glob + k] for k in p]
                        for k in range(nt):
                            win[t_glob + k] = w2[k]
                    elif kfull is None:
                        win[t_glob] = (0, 128)
                t_glob += nt
    wp0 = np.array([w[0] for w in win], np.int64)
    ww = np.array([w[1] for w in win], np.int64)
    # variable-width p01 offsets for ACT tiles
    act_woff = np.zeros(NACT + 1, np.int64)
    np.cumsum(ww[act_idx], out=act_woff[1:])
    P01W = int(act_woff[-1])

    # per-core data arrays
    per_core = []
    for core in range(NC):
        idx_stream = np.zeros(TOT * 128, np.int16)
        slot_stream = np.full((128, TOT), 255.0, np.float32)
        t_glob = 0
        for g in groups:
            for h in (0, 1):
                for pos in range(g["b0"], g["b1"]):
                    b = int(perm[core, pos])
                    k = core * NBUK + b
                    s0, s1 = starts[k], starts[k + 1]
                    cut = s0 + lo_count[core, b]
                    if h == 0:
                        e_src = src_s[s0:cut]
                        e_slot = slot_s[s0:cut]
                    else:
                        e_src = src_s[cut:s1] - HI_BASE
                        e_slot = slot_s[cut:s1]
                    o2 = np.argsort(e_slot, kind="stable")
                    e_src = e_src[o2]
                    e_slot = e_slot[o2]
                    sp = slice_perm.get((pos, h))
                    if sp is not None and len(e_src) > 0:
                        segs = [e_src[k * 128:(k + 1) * 128] for k in sp]
                        segt = [e_slot[k * 128:(k + 1) * 128] for k in sp]
                        e_src = np.concatenate(segs)
                        e_slot = np.concatenate(segt)
                    n = len(e_src)
                    nt = int(Th[pos, h])
                    base = t_glob * 128
                    if n > 0:
                        idx_stream[base:base + n] = e_src.astype(np.int16)
                        fl = np.full(nt * 128, 255.0, np.float32)
                        fl[:n] = e_slot.astype(np.float32)
                        slot_stream[:, t_glob:t_glob + nt] = (
                            fl.reshape(nt, 128).T)
                    t_glob += nt
        assert t_glob == TOT
        # window-relative slots (255 padding stays out of range for any W)
        real = slot_stream < 255.0
        slot_stream = slot_stream - wp0[None, :].astype(np.float32) * real
        # wrap-16 the index stream, replicate across the 8 16-part groups
        wrapped = idx_stream.reshape(-1, 16).T  # [16, TOT*8]
        idx_arr = np.tile(wrapped, (8, 1)).copy()  # [128, TOT*8]
        # host-built 0/1 one-hot for ACT tiles, window-relative and
        # variable-width (width ww[t] per tile, concatenated)
        f8 = mybir.dt.np(FP8)
        p01 = np.zeros((128, max(P01W, 1)), f8)
        for i, t in enumerate(act_idx):
            W = int(ww[t])
            off = int(act_woff[i])
            oh = (slot_stream[:, t][:, None] ==
                  np.arange(W, dtype=np.float32)[None, :])
            p01[:, off:off + W] = oh.astype(f8)
        per_core.append(dict(idx=idx_arr, slots=slot_stream, p01=p01))

    sched = dict(groups=groups, TOT=TOT, Th=Th, Tb=Tb, perm=perm,
                 eng=eng, act_pos=act_pos, NACT=NACT,
                 wp0=wp0, ww=ww, act_woff=act_woff, P01W=P01W)
    return sched, per_core


def build_consts(cfg, gate_w, gate_b):
    """[128, 258] f32: iota | W_bcast | ones | b"""
    C = np.zeros((128, 258), np.float32)
    C[:, 0:128] = np.arange(128, dtype=np.float32)[None, :]
    C[:, 128:256] = np.asarray(gate_w, np.float32).reshape(1, 128)
    C[:, 256] = 1.0
    # gate bias is a uniform score shift and cancels in the softmax, so
    # the device never applies it; column 257 is kept but unused
    C[:, 257] = np.float32(np.asarray(gate_b).reshape(-1)[0])
    return C


def build_program(cfg, sched, do_main=True, do_compute=True):
    nc = bacc.Bacc("TRN2", num_devices=cfg.NC,
                   dynamic_dma_scratch_size=49152)
    NPC, NBUK, GROUP = cfg.NPC, cfg.NBUK, cfg.GROUP
    TOT = sched["TOT"]
    groups = sched["groups"]
    eng_map = sched["eng"]
    act_pos = sched["act_pos"]
    NACT = sched["NACT"]
    wp0 = sched["wp0"]
    ww = sched["ww"]
    act_woff = sched["act_woff"]

    xlo = nc.dram_tensor("xlo", [cfg.LO_MAX, 128], F32,
                         kind="ExternalInput")
    xhi = nc.dram_tensor("xhi", [cfg.N - cfg.HI_BASE, 128], F32,
                         kind="ExternalInput")
    idx = nc.dram_tensor("idx", [128, TOT * 8], I16, kind="ExternalInput")
    slt = nc.dram_tensor("slt", [128, TOT], F32, kind="ExternalInput")
    p01 = nc.dram_tensor("p01", [128, max(sched["P01W"], 1)], FP8,
                         kind="ExternalInput")
    cst = nc.dram_tensor("cst", [128, 258], F32, kind="ExternalInput")
    # agg output, slot-major: out[p, b*128 + d] = agg[b*128 + p, d], bf16
    out = nc.dram_tensor("out", [128, NBUK * 128], BF16,
                         kind="ExternalOutput")

    with tile.TileContext(nc) as tc:
        with (
            tc.tile_pool(name="const", bufs=1) as cpool,
            tc.tile_pool(name="meta", bufs=1) as mpool,
            tc.tile_pool(name="gather", bufs=5) as gpool,
            tc.tile_pool(name="sc", bufs=6) as scpool,
            tc.tile_pool(name="grp", bufs=3) as grpool,
            tc.tile_pool(name="pp", bufs=8) as ppool,
            tc.tile_pool(name="fl", bufs=4) as flpool,
            tc.tile_pool(name="ag", bufs=3) as agpool,
            tc.tile_pool(name="pnum", bufs=8, space="PSUM") as pnum,
        ):
            C = cpool.tile([128, 258], F32)
            iota_ap = C[:, 0:128]
            w_ap = C[:, 128:256]
            ones_ap = C[:, 256:257]
            b_ap = C[:, 257:258]

            # the idx stream is uploaded in chunks emitted just ahead of
            # the gathers that read them: a single monolithic upload held
            # the first gather batch (and the whole pipeline) back ~10us
            idx_sb = mpool.tile([128, TOT * 8], I16)
            slt_sb = mpool.tile([128, TOT], F32)

            # ---- main loop over groups (gathers prefetched one group
            # ahead so the in-order Pool queue never parks later groups'
            # gathers behind P'-builds that wait on cross-engine deps) ----
            # each dma_gather is capped at GMAX tiles: the SWDGE ring
            # holds scratch/16 descriptors and one instruction's
            # descriptor set must fit entirely
            GMAX = 8
            work = [g for g in (groups if do_main else [])
                    if g["lo"] + g["hi"] > 0]
            tg_of = {}
            acc = 0
            for g in (groups if do_main else []):
                tg_of[id(g)] = acc
                acc += g["lo"] + g["hi"]

            gbufs = {}

            # idx chunks cover CHUNK_GROUPS consecutive work groups (kept
            # >=512B per partition to dodge the small-transfer penalty)
            CHUNK_GROUPS = 3
            chunk_of = {}
            chunk_rng = []
            for wi, g in enumerate(work):
                ci = wi // CHUNK_GROUPS
                chunk_of[id(g)] = ci
                tg0 = tg_of[id(g)]
                TGg = g["lo"] + g["hi"]
                if ci == len(chunk_rng):
                    chunk_rng.append([tg0, tg0 + TGg])
                else:
                    chunk_rng[ci][1] = tg0 + TGg
            idx_chunk_done = set()

            def emit_idx_chunk(ci):
                if ci in idx_chunk_done:
                    return
                idx_chunk_done.add(ci)
                r0, r1 = chunk_rng[ci]
                nc.sync.dma_start(idx_sb[:, r0 * 8:r1 * 8],
                                  idx[:, r0 * 8:r1 * 8])

            def emit_gathers(g, first_small=False):
                emit_idx_chunk(chunk_of[id(g)])
                TG = g["lo"] + g["hi"]
                tg0 = tg_of[id(g)]
                # extra zeroed tile at the end lets the last tile's matmul
                # read a 256-wide rhs (junk cols accumulate into psum cols
                # 128:256, never read)
                gbuf = gpool.tile([128, TG, 128], F32, tag="gbuf")
                for half, n_t, base in ((0, g["lo"], 0),
                                        (1, g["hi"], g["lo"])):
                    tbl = xlo if half == 0 else xhi
                    # a small leading batch lets the first scores start
                    # as soon as 4 tiles land instead of a full batch
                    cuts = list(range(0, n_t, GMAX))
                    if first_small and half == 0 and n_t > 4:
                        cuts = [0, 4] + [c for c in cuts if c > 4]
                    for ci, q0 in enumerate(cuts):
                        q1 = cuts[ci + 1] if ci + 1 < len(cuts) else min(
                            q0 + GMAX, n_t)
                        b0t = base + q0
                        g0 = tg0 + b0t
                        nc.gpsimd.dma_gather(
                            out_ap=gbuf[:, b0t:base + q1, :],
                            in_ap=tbl[:],
                            idxs_ap=idx_sb[:, g0 * 8:(g0 + q1 - q0) * 8],
                            num_idxs=(q1 - q0) * 128,
                            num_idxs_reg=(q1 - q0) * 128,
                            elem_size=128,
                            single_packet=False,
                        )
                gbufs[id(g)] = gbuf

            # zero-fill output for empty groups up front
            for g in (groups if do_main else []):
                if g["lo"] + g["hi"] == 0:
                    nbk = g["b1"] - g["b0"]
                    aggg = agpool.tile([128, nbk * 128], BF16, tag="aggg")
                    nc.vector.memset(aggg[:], 0.0)
                    nc.sync.dma_start(
                        out[:, g["b0"] * 128:(g["b0"] + nbk) * 128], aggg[:])

            PREFETCH = 3  # groups of gathers in flight ahead (gbuf bufs-1)
            for wi, g in enumerate(work[:PREFETCH]):
                emit_gathers(g)
            # consts and slot stream issue after the pipeline-critical
            # first gathers (each DMA issue serializes ~0.6us on HWDGE)
            nc.sync.dma_start(C[:], cst[:])
            nc.sync.dma_start(slt_sb[:], slt[:])
            for wi, g in enumerate(work):
                if wi + PREFETCH < len(work):
                    emit_gathers(work[wi + PREFETCH])
                TG = g["lo"] + g["hi"]
                nbk = g["b1"] - g["b0"]
                t_glob = tg_of[id(g)]
                gbuf = gbufs.pop(id(g))
                aggg = agpool.tile([128, nbk * 128], BF16, tag="aggg")

                # p01 stream for this group's ACT-assigned tiles
                # (variable widths, contiguous in the global ACT ordering)
                acts = [t for t in range(TG) if eng_map[t_glob + t] == ENG_ACT]
                nact = len(acts)
                if nact > 0:
                    ai0 = int(act_pos[t_glob + acts[0]])
                    w0 = int(act_woff[ai0])
                    w1 = int(act_woff[ai0 + nact])
                    p01_sb = grpool.tile([128, w1 - w0], FP8, tag="p01sb")
                    nc.sync.dma_start(p01_sb[:], p01[:, w0:w1])

                if not do_compute:
                    # still consume gbuf so Tile keeps the gathers
                    sink = flpool.tile([128, 1], F32, tag="sink")
                    nc.vector.tensor_scalar(
                        out=sink[:], in0=gbuf[:, 0, 0:1], scalar1=1.0,
                        scalar2=None, op0=OP.mult)
                    nc.sync.dma_start(out[0:128, 0:1], sink[:].bitcast(BF16)[:, 0:1])
                    continue
                # scores for all tiles of the group
                sg = grpool.tile([128, TG], F32, tag="sg")
                eg = grpool.tile([128, TG], F32, tag="eg")
                for t in range(TG):
                    scr = scpool.tile([128, 128], F32, tag="scr",
                                      name="scr_v")
                    sc_eng = (nc.gpsimd if (t_glob + t) % 100 <
                              SC_POOL * 100 else nc.vector)
                    sc_eng.scalar_tensor_tensor(
                        out=scr[:], in0=gbuf[:, t, :], scalar=1.0,
                        in1=w_ap, op0=OP.mult, op1=OP.mult,
                        accum_out=sg[:, t:t + 1])
                    # per-16-tile exp keeps the pipeline fine-grained
                    if t % 16 == 15 or t == TG - 1:
                        lo8 = (t // 16) * 16
                        nc.scalar.activation(out=eg[:, lo8:t + 1],
                                             in_=sg[:, lo8:t + 1],
                                             func=AF.Exp,
                                             bias=0.0, scale=1.0)

                # per-bucket psum tiles: cols 0:128 numerator, col 128 denom
                psums = {}
                for j in range(nbk):
                    if sched["Tb"][g["b0"] + j] > 0:
                        psums[j] = pnum.tile([128, 129], F32, tag="pn",
                                             name=f"pn_{g['b0']}_{j}")

                for t, ti in enumerate(g["tiles"]):
                    j = ti["j"]
                    tg = t_glob + t
                    p0 = int(wp0[tg])
                    W = int(ww[tg])
                    Pp = ppool.tile([128, W], F32, tag=f"pp{W}")
                    ek = eng_map[tg]
                    if ek == ENG_ACT:
                        # ACT path: P' = host-built 0/1 one-hot (fp8,
                        # exact, window-relative) scaled by e in one Copy
                        ai = int(act_pos[tg])
                        o0 = int(act_woff[ai]) - int(
                            act_woff[int(act_pos[t_glob + acts[0]])])
                        nc.scalar.activation(
                            out=Pp[:], in_=p01_sb[:, o0:o0 + W],
                            func=AF.Copy, scale=eg[:, t:t + 1])
                    else:
                        nc.vector.tensor_scalar(
                            out=Pp[:], in0=iota_ap[:, 0:W],
                            scalar1=slt_sb[:, tg:tg + 1],
                            scalar2=eg[:, t:t + 1],
                            op0=OP.is_equal, op1=OP.mult)
                    # slot-sorted tiles cover a narrow aligned window
                    # [p0, p0+W) of the bucket's 128 slots; the matmul
                    # writes just those psum partitions. The bucket's
                    # first tile is forced to the full window so its
                    # start=True clears every partition of the bank.
                    nc.tensor.matmul(
                        out=psums[j][p0:p0 + W, 0:128], lhsT=Pp[:],
                        rhs=gbuf[:, t, :],
                        start=ti["first"], stop=False)
                    nc.tensor.matmul(
                        out=psums[j][p0:p0 + W, 128:129], lhsT=Pp[:],
                        rhs=ones_ap,
                        start=False, stop=ti["last"])

                # flush group: per-bucket reciprocal + scale into the
                # group's bf16 slot-major agg tile, then one DMA out
                for j in range(nbk):
                    if j in psums:
                        # no epsilon guard: slots with zero in-degree give
                        # den=0 -> inf/NaN rows, which the host overwrites
                        # with zeros (it knows the in-degrees from
                        # edge_index); skipping the max() op saves ~6us DVE
                        rcp = flpool.tile([128, 1], F32, tag="rcp")
                        nc.vector.reciprocal(out=rcp[:],
                                             in_=psums[j][:, 128:129])
                        nc.scalar.activation(
                            out=aggg[:, j * 128:(j + 1) * 128],
                            in_=psums[j][:, 0:128],
                            func=AF.Copy, scale=rcp[:, 0:1])
                    else:
                        nc.vector.memset(aggg[:, j * 128:(j + 1) * 128], 0.0)
                nc.sync.dma_start(
                    out[:, g["b0"] * 128:(g["b0"] + nbk) * 128], aggg[:])

    nc.compile()
    return nc


def make_in_maps(cfg, sched, per_core, x, gate_w, gate_b):
    x = np.asarray(x, np.float32)
    consts = build_consts(cfg, gate_w, gate_b)
    in_maps = []
    for core in range(cfg.NC):
        in_maps.append({
            "xlo": x[:cfg.LO_MAX],
            "xhi": x[cfg.HI_BASE:],
            "idx": per_core[core]["idx"],
            "slt": per_core[core]["slots"],
            "p01": per_core[core]["p01"],
            "cst": consts,
        })
    return in_maps


def _kernel_impl(x, gate_w, gate_b, edge_index, cfg=None, return_nc=False):
    from concourse.bass_utils import run_bass_kernel_spmd
    if cfg is None:
        cfg = Config()
    sched, per_core = build_schedule(cfg, edge_index[0], edge_index[1])
    nc = build_program(cfg, sched)
    in_maps = make_in_maps(cfg, sched, per_core, x, gate_w, gate_b)
    res = run_bass_kernel_spmd(nc, in_maps, core_ids=list(range(cfg.NC)))
    perm = sched["perm"]
    outp = np.zeros((cfg.N, 256), np.float32)
    outp[:, 0:128] = x
    indeg = np.bincount(np.asarray(edge_index[1], np.int64),
                        minlength=cfg.N)
    for core in range(cfg.NC):
        # o: [128, NBUK*128] bf16 slot-major -> [NBUK, 128, 128] agg
        o = np.asarray(res.results[core]["out"], dtype=np.float32)
        o = o.reshape(128, cfg.NBUK, 128).transpose(1, 0, 2)
        base = core * cfg.NPC
        for k in range(cfg.NBUK):
            b = int(perm[core, k])
            v = min(128, cfg.NPC - b * 128)
            outp[base + b * 128:base + b * 128 + v, 128:256] = o[k, :v]
    outp[indeg == 0, 128:256] = 0.0
    if return_nc:
        return outp, nc
    return outp


def kernel(**inputs):
    """Harness entry: full unsharded inputs -> full [50000, 256] f32 output.

    Shards edges by destination-node range across the 8 NeuronCores
    (each core computes its 6250-node output slice fully locally),
    compiles the Bass program, and runs it via run_bass_kernel_spmd.
    """
    x = np.ascontiguousarray(np.asarray(inputs["x"], np.float32))
    gate_w = np.asarray(inputs["gate_w"], np.float32)
    gate_b = np.asarray(inputs["gate_b"], np.float32)
    edge_index = np.asarray(inputs["edge_index"])
    return _kernel_impl(x, gate_w, gate_b, edge_index)


# revision 58
# speedup vs baseline: 1.0355x; 1.0037x over previous
"""AttentionalAggregation GNN kernel for 8 TRN2 NeuronCores.

Strategy: edges sorted by destination bucket on host; core m owns nodes
[m*NPC, (m+1)*NPC) and computes its output slice fully locally (no
collectives). Per 128-edge tile:
  - dma_gather x[src] rows (512B each) from lo/hi half tables (int16 idx)
  - score_e = sum_d V[e,d]*w[d]  (scalar_tensor_tensor accum)
  - e = exp(score + b)           (ACT)
  - P'[e,s] = (iota_s == slot_e) * e_e   (built on DVE, Pool or ACT,
    split to balance engine load; ACT path reads a host-built 0/1
    one-hot in fp8 and scales by e in one Copy op)
  - psum[bucket][:,0:128] += P'.T @ V_t   (plain f32 matmul: hardware
    f32r is tf32-grade and fails the accuracy budget)
  - psum[bucket][:,128]   += P'.T @ ones  (denominator)
Flush per group: reciprocal of denominator, scale, write agg as bf16 in
slot-major layout; host unpermutes, casts to f32 and concats with x.
"""

import math
import numpy as np

import concourse.bass as bass
import concourse.mybir as mybir
import concourse.tile as tile
from concourse import bacc

F32 = mybir.dt.float32
F32R = mybir.dt.float32r
BF16 = mybir.dt.bfloat16
FP8 = mybir.dt.float8e4
I16 = mybir.dt.int16
AF = mybir.ActivationFunctionType
OP = mybir.AluOpType

# engine assignment for building P' (one-hot * e) per tile
ENG_DVE = 0
ENG_POOL = 1
ENG_ACT = 2
# shares (DVE, POOL, ACT) of P'-build work; ACT tiles need the p01 stream.
# Pool-engine P' builds stall the in-order Pool queue behind cross-engine
# waits (gathers share it), so the Pool share stays 0.
PP_SHARES = (0.27, 0.0, 0.73)
# fraction of score STTs routed to the Pool engine. Keep 0: any non-SWDGE
# work on the in-order Pool engine delays later groups' descriptor
# generation and starves the DMA engines.
SC_POOL = 0.0


class Config:
    def __init__(self, N=50000, E=640000, D=128, NC=8, GROUP=2):
        assert D == 128
        self.N, self.E, self.D, self.NC = N, E, D, NC
        self.NPC = N // NC          # nodes per core
        assert self.NPC * NC == N
        # overlapping lo/hi gather tables (int16 index limit 32768 rows);
        # sources in the overlap may be assigned to either run, letting the
        # host pad the lo run to a tile boundary with real edges
        self.LO_MAX = min(32768, N)
        self.HI_BASE = max(0, N - 32768)
        self.NBUK = math.ceil(self.NPC / 128)   # buckets per core
        self.TAIL = self.NPC - (self.NBUK - 1) * 128  # rows in last bucket
        self.GROUP = GROUP
        self.NG = math.ceil(self.NBUK / GROUP)


def assign_engines(TOT):
    """Per-tile engine for the P' build, interleaved to the target shares."""
    eng = np.zeros(TOT, np.int8)
    acc = [0.0, 0.0, 0.0]
    for t in range(TOT):
        # pick the engine furthest below its target share
        deficits = [PP_SHARES[k] * (t + 1) - acc[k] for k in range(3)]
        k = int(np.argmax(deficits))
        eng[t] = k
        acc[k] += 1.0
    return eng


def build_schedule(cfg, src, dst):
    """Host-side: sort/pad edges into a static per-tile schedule uniform
    across cores. Returns (sched, per_core) where sched is the static
    structure (identical across cores) and per_core has the data arrays."""
    N, NC, NPC, NBUK, GROUP = (
        cfg.N, cfg.NC, cfg.NPC, cfg.NBUK, cfg.GROUP)
    LO_MAX, HI_BASE = cfg.LO_MAX, cfg.HI_BASE

    src = np.asarray(src, np.int64)
    dst = np.asarray(dst, np.int64)
    c = dst // NPC
    r = dst % NPC
    lb = r // 128
    slot = r % 128

    order = np.lexsort((src, lb, c))
    src_s, lb_s, slot_s, c_s = (
        src[order], lb[order], slot[order], c[order])

    key = c_s * NBUK + lb_s
    cnt = np.bincount(key, minlength=NC * NBUK).reshape(NC, NBUK)
    starts = np.zeros(NC * NBUK + 1, np.int64)
    np.cumsum(cnt.reshape(-1), out=starts[1:])

    # within each (core, bucket) slice (sorted by src), edges below
    # HI_BASE must use the lo table, edges >= LO_MAX must use hi, and the
    # overlap is flexible: cut the slice to fill lo tiles exactly
    n_lo_min = np.zeros((NC, NBUK), np.int64)
    n_lo_cap = np.zeros((NC, NBUK), np.int64)
    for cc in range(NC):
        for b in range(NBUK):
            k = cc * NBUK + b
            sl = src_s[starts[k]:starts[k + 1]]
            n_lo_min[cc, b] = np.searchsorted(sl, HI_BASE)
            n_lo_cap[cc, b] = np.searchsorted(sl, LO_MAX)
    # per-core needs, then sort each core's buckets by total tiles
    # descending so position-wise maxima across cores are tight
    t_lo_c = np.ceil(n_lo_min / 128.0).astype(np.int64)       # [NC, NBUK]
    lo_cap_pos = n_lo_cap
    t_hi_c = np.ceil(np.maximum(cnt - np.minimum(128 * t_lo_c, lo_cap_pos),
                                0) / 128.0).astype(np.int64)
    tot_c = t_lo_c + t_hi_c
    perm = np.argsort(-tot_c, axis=1, kind="stable")          # [NC, NBUK]
    ar = np.arange(NC)[:, None]
    T_lo = t_lo_c[ar, perm].max(axis=0)                       # [NBUK] by pos
    lo_count_pos = np.minimum(128 * T_lo[None, :], n_lo_cap[ar, perm])
    T_hi = np.ceil((cnt[ar, perm] - lo_count_pos) / 128.0
                   ).astype(np.int64).max(axis=0)
    Th = np.stack([T_lo, T_hi], axis=1)  # [NBUK, 2] by position
    # scatter position-based lo counts back to per-(core,bucket)
    lo_count = np.zeros_like(cnt)
    np.put_along_axis(lo_count, perm, lo_count_pos, axis=1)
    # move the smallest bucket to position 0: paired with a single-bucket
    # first group below, it halves the ramp-critical first gather batch
    pi = np.r_[NBUK - 1, NBUK - 2, np.arange(NBUK - 2)]
    perm = perm[:, pi]
    Th = Th[pi]
    Tb = Th.sum(axis=1)

    # static tile stream: per group g: [lo tiles of buckets][hi tiles]
    # each entry: (bucket_local_index_in_group j, bucket b, first, last)
    sizes = [1, 1]
    left = NBUK - 2
    while left > 0:
        take = min(GROUP, left)
        sizes.append(take)
        left -= take
    bounds = np.cumsum([0] + sizes)
    groups = []
    for g in range(len(sizes)):
        b0, b1 = int(bounds[g]), int(bounds[g + 1])
        tiles = []
        for h in (0, 1):
            for b in range(b0, b1):
                nt = int(Th[b, h])
                for t in range(nt):
                    pos = t if h == 0 else int(Th[b, 0]) + t
                    first = pos == 0
                    last = pos == int(Tb[b]) - 1
                    tiles.append(dict(j=b - b0, b=b, first=first, last=last))
        lo_tiles = int(Th[b0:b1, 0].sum())
        hi_tiles = int(Th[b0:b1, 1].sum())
        groups.append(dict(b0=b0, b1=b1, lo=lo_tiles, hi=hi_tiles,
                           tiles=tiles))
    TOT = sum(g["lo"] + g["hi"] for g in groups)

    # engine assignment per global tile; ACT tiles get p01 stream slots
    eng = assign_engines(TOT)
    act_idx = np.nonzero(eng == ENG_ACT)[0]
    act_pos = np.full(TOT, -1, np.int64)    # tile -> index among ACT tiles
    for i, t in enumerate(act_idx):
        act_pos[t] = i
    NACT = len(act_idx)

    # pass A: per-tile slot-span unions across cores (edges re-sorted by
    # slot within each (bucket, half) run so a tile covers a narrow
    # contiguous window; matmul partition offsets must be 32-aligned with
    # legal widths {32 @0/32/64/96, 64 @0/64, 128 @0})
    span_lo = np.full(TOT, 255, np.int64)
    span_hi = np.full(TOT, -1, np.int64)
    for core in range(NC):
        t_glob = 0
        for g in groups:
            for h in (0, 1):
                for pos in range(g["b0"], g["b1"]):
                    b = int(perm[core, pos])
                    k = core * NBUK + b
                    s0, s1 = starts[k], starts[k + 1]
                    cut = s0 + lo_count[core, b]
                    e_slot = (slot_s[s0:cut] if h == 0
                              else slot_s[cut:s1])
                    nt = int(Th[pos, h])
                    if nt > 0 and len(e_slot) > 0:
                        sl = np.sort(e_slot)
                        for kk in range(nt):
                            seg = sl[kk * 128:(kk + 1) * 128]
                            if len(seg):
                                t = t_glob + kk
                                span_lo[t] = min(span_lo[t], int(seg[0]))
                                span_hi[t] = max(span_hi[t], int(seg[-1]))
                    t_glob += nt

    def legal_window(a, b):
        # base partitions allowed by the AP layer: {0, 32, 64} only
        if b < 0:
            return 0, 32
        p = 32 * (a // 32)
        if p <= 64 and b - p < 32:
            return p, 32
        p = 64 * (a // 64)
        if b - p < 64:
            return p, 64
        return 0, 128

    win = [legal_window(int(span_lo[t]), int(span_hi[t]))
           for t in range(TOT)]
    # the bucket-opening tile (start=True) must cover all 128 partitions:
    # hardware clears the full bank row only for partitions the matmul
    # writes (validated: narrow first tiles corrupt untouched partitions).
    # Swap a naturally-full-window slice to the front of the bucket's
    # first run when one exists; otherwise widen the first tile.
    slice_perm = {}  # (pos, h) -> permutation of the run's slice indices
    t_glob = 0
    for g in groups:
        for h in (0, 1):
            for pos in range(g["b0"], g["b1"]):
                nt = int(Th[pos, h])
                if nt == 0:
                    continue
                is_first_run = (h == 0) or int(Th[pos, 0]) == 0
                if is_first_run:
                    p = list(range(nt))
                    kfull = next((k for k in range(nt)
                                  if win[t_glob + k] == (0, 128)), None)
                    if kfull is not None and kfull != 0:
                        p[0], p[kfull] = p[kfull], p[0]
                        slice_perm[(pos, h)] = p
                        w2 = [win[t_glob + k] for k in p]
                        for k in range(nt):
                            win[t_glob + k] = w2[k]
                    elif kfull is None:
                        win[t_glob] = (0, 128)
                t_glob += nt
    wp0 = np.array([w[0] for w in win], np.int64)
    ww = np.array([w[1] for w in win], np.int64)
    # variable-width p01 offsets for ACT tiles
    act_woff = np.zeros(NACT + 1, np.int64)
    np.cumsum(ww[act_idx], out=act_woff[1:])
    P01W = int(act_woff[-1])

    # per-core data arrays
    per_core = []
    for core in range(NC):
        idx_stream = np.zeros(TOT * 128, np.int16)
        slot_stream = np.full((128, TOT), 255.0, np.float32)
        t_glob = 0
        for g in groups:
            for h in (0, 1):
                for pos in range(g["b0"], g["b1"]):
                    b = int(perm[core, pos])
                    k = core * NBUK + b
                    s0, s1 = starts[k], starts[k + 1]
                    cut = s0 + lo_count[core, b]
                    if h == 0:
                        e_src = src_s[s0:cut]
                        e_slot = slot_s[s0:cut]
                    else:
                        e_src = src_s[cut:s1] - HI_BASE
                        e_slot = slot_s[cut:s1]
                    o2 = np.argsort(e_slot, kind="stable")
                    e_src = e_src[o2]
                    e_slot = e_slot[o2]
                    sp = slice_perm.get((pos, h))
                    if sp is not None and len(e_src) > 0:
                        segs = [e_src[k * 128:(k + 1) * 128] for k in sp]
                        segt = [e_slot[k * 128:(k + 1) * 128] for k in sp]
                        e_src = np.concatenate(segs)
                        e_slot = np.concatenate(segt)
                    n = len(e_src)
                    nt = int(Th[pos, h])
                    base = t_glob * 128
                    if n > 0:
                        idx_stream[base:base + n] = e_src.astype(np.int16)
                        fl = np.full(nt * 128, 255.0, np.float32)
                        fl[:n] = e_slot.astype(np.float32)
                        slot_stream[:, t_glob:t_glob + nt] = (
                            fl.reshape(nt, 128).T)
                    t_glob += nt
        assert t_glob == TOT
        # window-relative slots (255 padding stays out of range for any W)
        real = slot_stream < 255.0
        slot_stream = slot_stream - wp0[None, :].astype(np.float32) * real
        # wrap-16 the index stream, replicate across the 8 16-part groups
        wrapped = idx_stream.reshape(-1, 16).T  # [16, TOT*8]
        idx_arr = np.tile(wrapped, (8, 1)).copy()  # [128, TOT*8]
        # host-built 0/1 one-hot for ACT tiles, window-relative and
        # variable-width (width ww[t] per tile, concatenated)
        f8 = mybir.dt.np(FP8)
        p01 = np.zeros((128, max(P01W, 1)), f8)
        for i, t in enumerate(act_idx):
            W = int(ww[t])
            off = int(act_woff[i])
            oh = (slot_stream[:, t][:, None] ==
                  np.arange(W, dtype=np.float32)[None, :])
            p01[:, off:off + W] = oh.astype(f8)
        per_core.append(dict(idx=idx_arr, slots=slot_stream, p01=p01))

    sched = dict(groups=groups, TOT=TOT, Th=Th, Tb=Tb, perm=perm,
                 eng=eng, act_pos=act_pos, NACT=NACT,
                 wp0=wp0, ww=ww, act_woff=act_woff, P01W=P01W)
    return sched, per_core


def build_consts(cfg, gate_w, gate_b):
    """[128, 258] f32: iota | W_bcast | ones | b"""
    C = np.zeros((128, 258), np.float32)
    C[:, 0:128] = np.arange(128, dtype=np.float32)[None, :]
    C[:, 128:256] = np.asarray(gate_w, np.float32).reshape(1, 128)
    C[:, 256] = 1.0
    # gate bias is a uniform score shift and cancels in the softmax, so
    # the device never applies it; column 257 is kept but unused
    C[:, 257] = np.float32(np.asarray(gate_b).reshape(-1)[0])
    return C


def build_program(cfg, sched, do_main=True, do_compute=True):
    nc = bacc.Bacc("TRN2", num_devices=cfg.NC,
                   dynamic_dma_scratch_size=49152)
    NPC, NBUK, GROUP = cfg.NPC, cfg.NBUK, cfg.GROUP
    TOT = sched["TOT"]
    groups = sched["groups"]
    eng_map = sched["eng"]
    act_pos = sched["act_pos"]
    NACT = sched["NACT"]
    wp0 = sched["wp0"]
    ww = sched["ww"]
    act_woff = sched["act_woff"]

    xlo = nc.dram_tensor("xlo", [cfg.LO_MAX, 128], F32,
                         kind="ExternalInput")
    xhi = nc.dram_tensor("xhi", [cfg.N - cfg.HI_BASE, 128], F32,
                         kind="ExternalInput")
    idx = nc.dram_tensor("idx", [128, TOT * 8], I16, kind="ExternalInput")
    slt = nc.dram_tensor("slt", [128, TOT], F32, kind="ExternalInput")
    p01 = nc.dram_tensor("p01", [128, max(sched["P01W"], 1)], FP8,
                         kind="ExternalInput")
    cst = nc.dram_tensor("cst", [128, 258], F32, kind="ExternalInput")
    # agg output, slot-major: out[p, b*128 + d] = agg[b*128 + p, d], bf16
    out = nc.dram_tensor("out", [128, NBUK * 128], BF16,
                         kind="ExternalOutput")

    with tile.TileContext(nc) as tc:
        with (
            tc.tile_pool(name="const", bufs=1) as cpool,
            tc.tile_pool(name="meta", bufs=1) as mpool,
            tc.tile_pool(name="gather", bufs=5) as gpool,
            tc.tile_pool(name="sc", bufs=6) as scpool,
            tc.tile_pool(name="grp", bufs=3) as grpool,
            tc.tile_pool(name="pp", bufs=8) as ppool,
            tc.tile_pool(name="fl", bufs=4) as flpool,
            tc.tile_pool(name="ag", bufs=3) as agpool,
            tc.tile_pool(name="pnum", bufs=8, space="PSUM") as pnum,
        ):
            C = cpool.tile([128, 258], F32)
            iota_ap = C[:, 0:128]
            w_ap = C[:, 128:256]
            ones_ap = C[:, 256:257]
            b_ap = C[:, 257:258]

            # the idx stream is uploaded in chunks emitted just ahead of
            # the gathers that read them: a single monolithic upload held
            # the first gather batch (and the whole pipeline) back ~10us
            idx_sb = mpool.tile([128, TOT * 8], I16)
            slt_sb = mpool.tile([128, TOT], F32)

            # ---- main loop over groups (gathers prefetched one group
            # ahead so the in-order Pool queue never parks later groups'
            # gathers behind P'-builds that wait on cross-engine deps) ----
            # each dma_gather is capped at GMAX tiles: the SWDGE ring
            # holds scratch/16 descriptors and one instruction's
            # descriptor set must fit entirely
            GMAX = 8
            work = [g for g in (groups if do_main else [])
                    if g["lo"] + g["hi"] > 0]
            tg_of = {}
            acc = 0
            for g in (groups if do_main else []):
                tg_of[id(g)] = acc
                acc += g["lo"] + g["hi"]

            gbufs = {}

            # idx chunks cover CHUNK_GROUPS consecutive work groups (kept
            # >=512B per partition to dodge the small-transfer penalty)
            CHUNK_GROUPS = 3
            chunk_of = {}
            chunk_rng = []
            for wi, g in enumerate(work):
                ci = wi // CHUNK_GROUPS
                chunk_of[id(g)] = ci
                tg0 = tg_of[id(g)]
                TGg = g["lo"] + g["hi"]
                if ci == len(chunk_rng):
                    chunk_rng.append([tg0, tg0 + TGg])
                else:
                    chunk_rng[ci][1] = tg0 + TGg
            idx_chunk_done = set()

            def emit_idx_chunk(ci):
                if ci in idx_chunk_done:
                    return
                idx_chunk_done.add(ci)
                r0, r1 = chunk_rng[ci]
                nc.sync.dma_start(idx_sb[:, r0 * 8:r1 * 8],
                                  idx[:, r0 * 8:r1 * 8])

            def emit_gathers(g, first_small=False):
                emit_idx_chunk(chunk_of[id(g)])
                TG = g["lo"] + g["hi"]
                tg0 = tg_of[id(g)]
                # extra zeroed tile at the end lets the last tile's matmul
                # read a 256-wide rhs (junk cols accumulate into psum cols
                # 128:256, never read)
                gbuf = gpool.tile([128, TG, 128], F32, tag="gbuf")
                for half, n_t, base in ((0, g["lo"], 0),
                                        (1, g["hi"], g["lo"])):
                    tbl = xlo if half == 0 else xhi
                    # a small leading batch lets the first scores start
                    # as soon as 4 tiles land instead of a full batch
                    cuts = list(range(0, n_t, GMAX))
                    if first_small and half == 0 and n_t > 4:
                        cuts = [0, 4] + [c for c in cuts if c > 4]
                    for ci, q0 in enumerate(cuts):
                        q1 = cuts[ci + 1] if ci + 1 < len(cuts) else min(
                            q0 + GMAX, n_t)
                        b0t = base + q0
                        g0 = tg0 + b0t
                        nc.gpsimd.dma_gather(
                            out_ap=gbuf[:, b0t:base + q1, :],
                            in_ap=tbl[:],
                            idxs_ap=idx_sb[:, g0 * 8:(g0 + q1 - q0) * 8],
                            num_idxs=(q1 - q0) * 128,
                            num_idxs_reg=(q1 - q0) * 128,
                            elem_size=128,
                            single_packet=False,
                        )
                gbufs[id(g)] = gbuf

            # zero-fill output for empty groups up front
            for g in (groups if do_main else []):
                if g["lo"] + g["hi"] == 0:
                    nbk = g["b1"] - g["b0"]
                    aggg = agpool.tile([128, nbk * 128], BF16, tag="aggg")
                    nc.vector.memset(aggg[:], 0.0)
                    nc.sync.dma_start(
                        out[:, g["b0"] * 128:(g["b0"] + nbk) * 128], aggg[:])

            PREFETCH = 3  # groups of gathers in flight ahead (gbuf bufs-1)
            for wi, g in enumerate(work[:PREFETCH]):
                emit_gathers(g)
            # consts and slot stream issue after the pipeline-critical
            # first gathers (each DMA issue serializes ~0.6us on HWDGE)
            nc.sync.dma_start(C[:], cst[:])
            nc.sync.dma_start(slt_sb[:], slt[:])
            for wi, g in enumerate(work):
                if wi + PREFETCH < len(work):
                    emit_gathers(work[wi + PREFETCH])
                TG = g["lo"] + g["hi"]
                nbk = g["b1"] - g["b0"]
                t_glob = tg_of[id(g)]
                gbuf = gbufs.pop(id(g))
                aggg = agpool.tile([128, nbk * 128], BF16, tag="aggg")

                # p01 stream for this group's ACT-assigned tiles
                # (variable widths, contiguous in the global ACT ordering)
                acts = [t for t in range(TG) if eng_map[t_glob + t] == ENG_ACT]
                nact = len(acts)
                if nact > 0:
                    ai0 = int(act_pos[t_glob + acts[0]])
                    w0 = int(act_woff[ai0])
                    w1 = int(act_woff[ai0 + nact])
                    p01_sb = grpool.tile([128, w1 - w0], FP8, tag="p01sb")
                    nc.sync.dma_start(p01_sb[:], p01[:, w0:w1])

                if not do_compute:
                    # still consume gbuf so Tile keeps the gathers
                    sink = flpool.tile([128, 1], F32, tag="sink")
                    nc.vector.tensor_scalar(
                        out=sink[:], in0=gbuf[:, 0, 0:1], scalar1=1.0,
                        scalar2=None, op0=OP.mult)
                    nc.sync.dma_start(out[0:128, 0:1], sink[:].bitcast(BF16)[:, 0:1])
                    continue
                # scores for all tiles of the group
                sg = grpool.tile([128, TG], F32, tag="sg")
                eg = grpool.tile([128, TG], F32, tag="eg")
                for t in range(TG):
                    scr = scpool.tile([128, 128], F32, tag="scr",
                                      name="scr_v")
                    sc_eng = (nc.gpsimd if (t_glob + t) % 100 <
                              SC_POOL * 100 else nc.vector)
                    sc_eng.scalar_tensor_tensor(
                        out=scr[:], in0=gbuf[:, t, :], scalar=1.0,
                        in1=w_ap, op0=OP.mult, op1=OP.mult,
                        accum_out=sg[:, t:t + 1])
                    # per-16-tile exp keeps the pipeline fine-grained
                    if t % 16 == 15 or t == TG - 1:
                        lo8 = (t // 16) * 16
                        nc.scalar.activation(out=eg[:, lo8:t + 1],
                                             in_=sg[:, lo8:t + 1],
                                             func=AF.Exp,
                                             bias=0.0, scale=1.0)

                # per-bucket psum tiles: cols 0:128 numerator, col 128 denom
                psums = {}
                for j in range(nbk):
                    if sched["Tb"][g["b0"] + j] > 0:
                        psums[j] = pnum.tile([128, 129], F32, tag="pn",
                                             name=f"pn_{g['b0']}_{j}")

                for t, ti in enumerate(g["tiles"]):
                    j = ti["j"]
                    tg = t_glob + t
                    p0 = int(wp0[tg])
                    W = int(ww[tg])
                    Pp = ppool.tile([128, W], F32, tag=f"pp{W}")
                    ek = eng_map[tg]
                    if ek == ENG_ACT:
                        # ACT path: P' = host-built 0/1 one-hot (fp8,
                        # exact, window-relative) scaled by e in one Copy
                        ai = int(act_pos[tg])
                        o0 = int(act_woff[ai]) - int(
                            act_woff[int(act_pos[t_glob + acts[0]])])
                        nc.scalar.activation(
                            out=Pp[:], in_=p01_sb[:, o0:o0 + W],
                            func=AF.Copy, scale=eg[:, t:t + 1])
                    else:
                        nc.vector.tensor_scalar(
                            out=Pp[:], in0=iota_ap[:, 0:W],
                            scalar1=slt_sb[:, tg:tg + 1],
                            scalar2=eg[:, t:t + 1],
                            op0=OP.is_equal, op1=OP.mult)
                    # slot-sorted tiles cover a narrow aligned window
                    # [p0, p0+W) of the bucket's 128 slots; the matmul
                    # writes just those psum partitions. The bucket's
                    # first tile is forced to the full window so its
                    # start=True clears every partition of the bank.
                    nc.tensor.matmul(
                        out=psums[j][p0:p0 + W, 0:128], lhsT=Pp[:],
                        rhs=gbuf[:, t, :],
                        start=ti["first"], stop=False)
                    nc.tensor.matmul(
                        out=psums[j][p0:p0 + W, 128:129], lhsT=Pp[:],
                        rhs=ones_ap,
                        start=False, stop=ti["last"])

                # flush group: per-bucket reciprocal + scale into the
                # group's bf16 slot-major agg tile, then one DMA out
                for j in range(nbk):
                    if j in psums:
                        # no epsilon guard: slots with zero in-degree give
                        # den=0 -> inf/NaN rows, which the host overwrites
                        # with zeros (it knows the in-degrees from
                        # edge_index); skipping the max() op saves ~6us DVE
                        rcp = flpool.tile([128, 1], F32, tag="rcp")
                        nc.vector.reciprocal(out=rcp[:],
                                             in_=psums[j][:, 128:129])
                        nc.scalar.activation(
                            out=aggg[:, j * 128:(j + 1) * 128],
                            in_=psums[j][:, 0:128],
                            func=AF.Copy, scale=rcp[:, 0:1])
                    else:
                        nc.vector.memset(aggg[:, j * 128:(j + 1) * 128], 0.0)
                nc.sync.dma_start(
                    out[:, g["b0"] * 128:(g["b0"] + nbk) * 128], aggg[:])

    nc.compile()
    return nc


def make_in_maps(cfg, sched, per_core, x, gate_w, gate_b):
    x = np.asarray(x, np.float32)
    consts = build_consts(cfg, gate_w, gate_b)
    in_maps = []
    for core in range(cfg.NC):
        in_maps.append({
            "xlo": x[:cfg.LO_MAX],
            "xhi": x[cfg.HI_BASE:],
            "idx": per_core[core]["idx"],
            "slt": per_core[core]["slots"],
            "p01": per_core[core]["p01"],
            "cst": consts,
        })
    return in_maps


def _kernel_impl(x, gate_w, gate_b, edge_index, cfg=None, return_nc=False):
    from concourse.bass_utils import run_bass_kernel_spmd
    if cfg is None:
        cfg = Config()
    sched, per_core = build_schedule(cfg, edge_index[0], edge_index[1])
    nc = build_program(cfg, sched)
    in_maps = make_in_maps(cfg, sched, per_core, x, gate_w, gate_b)
    res = run_bass_kernel_spmd(nc, in_maps, core_ids=list(range(cfg.NC)))
    perm = sched["perm"]
    outp = np.zeros((cfg.N, 256), np.float32)
    outp[:, 0:128] = x
    indeg = np.bincount(np.asarray(edge_index[1], np.int64),
                        minlength=cfg.N)
    for core in range(cfg.NC):
        # o: [128, NBUK*128] bf16 slot-major -> [NBUK, 128, 128] agg
        o = np.asarray(res.results[core]["out"], dtype=np.float32)
        o = o.reshape(128, cfg.NBUK, 128).transpose(1, 0, 2)
        base = core * cfg.NPC
        for k in range(cfg.NBUK):
            b = int(perm[core, k])
            v = min(128, cfg.NPC - b * 128)
            outp[base + b * 128:base + b * 128 + v, 128:256] = o[k, :v]
    outp[indeg == 0, 128:256] = 0.0
    if return_nc:
        return outp, nc
    return outp


def kernel(**inputs):
    """Harness entry: full unsharded inputs -> full [50000, 256] f32 output.

    Shards edges by destination-node range across the 8 NeuronCores
    (each core computes its 6250-node output slice fully locally),
    compiles the Bass program, and runs it via run_bass_kernel_spmd.
    """
    x = np.ascontiguousarray(np.asarray(inputs["x"], np.float32))
    gate_w = np.asarray(inputs["gate_w"], np.float32)
    gate_b = np.asarray(inputs["gate_b"], np.float32)
    edge_index = np.asarray(inputs["edge_index"])
    return _kernel_impl(x, gate_w, gate_b, edge_index)
